# revision 1
# baseline (speedup 1.0000x reference)
"""Trainium2 Bass kernel for the GRU autoencoder.

Distribution strategy (8 NeuronCores):
  Encode : chain-parallel x batch-parallel. Core j handles GRU chain j//2
           (xf, xb, ef, eb) on batch half j%2 (128 rows), running a uniform
           100-step loop. The 50-step x-chains get 50 leading exact identity
           steps (update-gate pre-activation forced to +BIG => z=1 => h'=h).
           Input-side gates, biases and the pad flag ride the same matmul via
           augmented stationary rows (features + ones-row + flag-row).
  Reshard: AllToAll moves 16-row slices so every core assembles the hidden
           states for its own 32-row decode shard at *fixed* (SPMD-uniform)
           indices: core j decodes global rows [16j:16j+16] u [128+16j:+16].
  Middle : per-shard MLP (em1/em2/eo) + decoder const precompute (bf16).
  Decode : 60 autoregressive steps per core on its 32-row shard (fp32r).

All GRU-path matmuls use fp32r (full PE rate at N>=256, ~1e-4 rounding); the
middle MLP uses bf16 weights. PSUM accumulates in fp32 throughout.

PSUM budget (8 banks): gA 2 | gB 2x1 | gC 2x1 | tr 2x1.
"""

import sys

sys.path.insert(0, "/opt/trn_rl_repo")

import numpy as np

import concourse.bass as bass
import concourse.mybir as mybir
import concourse.tile as tile
from concourse import bacc
from concourse.masks import make_identity

dt = mybir.dt
AF = mybir.ActivationFunctionType
OP = mybir.AluOpType

B, TX, TY, NX, NY, H, HOR = 256, 50, 100, 64, 64, 512, 60
M1, M2 = 1024, 512
G = 3 * H
NCORE = 8
BE = 128   # encoder batch rows per core
BD = 32    # decoder batch rows per core
BIG = 30000.0

F32, F32R, BF16 = dt.float32, dt.float32r, dt.bfloat16


def build_nc(et=100, x_real=50, hor=60):
    nc = bacc.Bacc("TRN2", target_bir_lowering=False, debug=False,
                   num_devices=NCORE)

    # ---- DRAM parameters (identical names on every core; content differs) --
    d_xin = nc.dram_tensor("xin", [66, et * BE], F32, kind="ExternalInput")
    d_wih = nc.dram_tensor("wih_aug", [66, G], F32, kind="ExternalInput")
    d_whh = nc.dram_tensor("whh_t", [H, G], F32, kind="ExternalInput")
    d_bhhn = nc.dram_tensor("bhhn_row", [1, H], F32, kind="ExternalInput")

    d_em1 = nc.dram_tensor("em_w1t", [2 * H, M1], F32, kind="ExternalInput")
    d_em1b = nc.dram_tensor("em_b1row", [1, M1], F32, kind="ExternalInput")
    d_em2 = nc.dram_tensor("em_w2t", [M1, M2], F32, kind="ExternalInput")
    d_em2b = nc.dram_tensor("em_b2row", [1, M2], F32, kind="ExternalInput")
    d_eow = nc.dram_tensor("eo_wt", [M2, H], F32, kind="ExternalInput")
    d_eob = nc.dram_tensor("eo_brow", [1, H], F32, kind="ExternalInput")
    d_dcw = nc.dram_tensor("dc_wt", [2 * H, G], F32, kind="ExternalInput")
    d_dcb = nc.dram_tensor("dc_brow", [1, G], F32, kind="ExternalInput")

    d_dwy = nc.dram_tensor("dwy_t", [NY, G], F32, kind="ExternalInput")
    d_dwhh = nc.dram_tensor("dwhh_t", [H, G], F32, kind="ExternalInput")
    d_dbhhn = nc.dram_tensor("dbhhn_row", [1, H], F32, kind="ExternalInput")
    d_dm1 = nc.dram_tensor("dm_w1t", [H, M1], F32, kind="ExternalInput")
    d_dm1b = nc.dram_tensor("dm_b1row", [1, M1], F32, kind="ExternalInput")
    d_dm2 = nc.dram_tensor("dm_w2t", [M1, M2], F32, kind="ExternalInput")
    d_dm2b = nc.dram_tensor("dm_b2row", [1, M2], F32, kind="ExternalInput")
    d_dow = nc.dram_tensor("do_wt", [M2, NY], F32, kind="ExternalInput")
    d_dob = nc.dram_tensor("do_brow", [1, NY], F32, kind="ExternalInput")
    d_xlast = nc.dram_tensor("xlast_t", [NX, BD], F32, kind="ExternalInput")

    d_out = nc.dram_tensor("out", [BD, hor * NY], F32, kind="ExternalOutput")

    cc_in = nc.dram_tensor("cc_in", [BE, H], F32)
    cc_out = nc.dram_tensor("cc_out", [NCORE, 16, H], F32)

    with tile.TileContext(nc) as tc:
        with tc.tile_pool(name="wenc", bufs=1) as wenc, \
             tc.tile_pool(name="wstream", bufs=2) as ws, \
             tc.tile_pool(name="wdec", bufs=1) as wdec, \
             tc.tile_pool(name="state", bufs=2) as st, \
             tc.tile_pool(name="tmp", bufs=2) as tp, \
             tc.tile_pool(name="tmp1", bufs=1) as tq, \
             tc.tile_pool(name="mid", bufs=1) as md, \
             tc.tile_pool(name="ldtmp", bufs=2) as ld, \
             tc.tile_pool(name="persist", bufs=1) as pe, \
             tc.tile_pool(name="psA", bufs=2, space="PSUM") as psA, \
             tc.tile_pool(name="psB", bufs=1, space="PSUM") as psB, \
             tc.tile_pool(name="psC", bufs=2, space="PSUM") as psC, \
             tc.tile_pool(name="psTR", bufs=1, space="PSUM") as psTR:

            # ---------- constants ----------
            idf = pe.tile([128, 128], F32, tag="idf")
            make_identity(nc, idf[:])
            id32 = idf[0:32, 0:32]
            ones_f = pe.tile([1, 128], F32, tag="ones_f")
            nc.gpsimd.memset(ones_f[:], 1.0)
            ones_r = pe.tile([1, 128], F32R, tag="ones_r")
            nc.vector.tensor_copy(ones_r[:], ones_f[:])
            zero_f = pe.tile([128, 128], F32, tag="zero_f")
            nc.gpsimd.memset(zero_f[:], 0.0)
            id_r = pe.tile([32, 32], F32R, tag="id_r")
            nc.vector.tensor_copy(id_r[:], id32)
            ones_b = pe.tile([1, 128], BF16, tag="ones_b")
            nc.gpsimd.tensor_copy(ones_b[:], ones_f[:])

            def load_r(pool, dram_ap, rows, cols, tag, rdt=F32R, eng=None):
                r = pool.tile([rows, cols], rdt, tag=tag)
                for lo in range(0, cols, 768):
                    hi = min(cols, lo + 768)
                    t = ld.tile([rows, hi - lo], F32, tag="ldtmp")
                    nc.sync.dma_start(t[:], dram_ap[:, lo:hi])
                    (eng or nc.gpsimd).tensor_copy(r[:, lo:hi], t[:])
                return r

            # ---------- encoder weights (needed immediately; DVE rounds) ----
            wih_r = load_r(wenc, d_wih[:], 66, G, "wih", eng=nc.vector)
            whh_r = [load_r(wenc, d_whh[128 * c:128 * (c + 1), :], 128, G,
                            f"whh{c}", eng=nc.vector) for c in range(4)]

            # ---------- encoder state ----------
            hT = pe.tile([128, H], F32R, tag="hT0")
            for c in range(4):
                nc.vector.tensor_copy(hT[:, 128 * c:128 * (c + 1)], zero_f[:])
            h_bh = pe.tile([BE, H], F32, tag="h0")
            nc.gpsimd.memset(h_bh[:], 0.0)

            # ---------- middle/decoder weights (gpsimd rounds in background)
            # column-packed bf16 biases: dcb|em1b|em2b|eob
            bias_b = pe.tile([1, 3584], BF16, tag="bias_b")
            for dtn, base, w in ((d_dcb, 0, G), (d_em1b, G, M1),
                                 (d_em2b, G + M1, M2), (d_eob, 3072, M2)):
                for lo in range(0, w, 768):
                    hi = min(w, lo + 768)
                    t = ld.tile([1, hi - lo], F32, tag="ldtmp")
                    nc.sync.dma_start(t[:], dtn[:, lo:hi])
                    nc.gpsimd.tensor_copy(bias_b[0:1, base + lo:base + hi],
                                          t[:])

            dwyc = wdec.tile([96, G], F32R, tag="dwyc")
            for lo in range(0, G, 768):
                hi = lo + 768
                t = ld.tile([NY, 768], F32, tag="ldtmp")
                nc.sync.dma_start(t[:], d_dwy[:, lo:hi])
                nc.gpsimd.tensor_copy(dwyc[0:NY, lo:hi], t[:])
            # identity block staged at partitions 64:96 (for ypT_aug rows)
            id_hi_f = pe.tile([96, 32], F32, tag="id_hi_f")
            nc.sync.dma_start(id_hi_f[64:96, :], idf[0:32, 0:32])
            dwhh_r = [load_r(wdec, d_dwhh[128 * c:128 * (c + 1), :], 128, G,
                             f"dwhh{c}") for c in range(4)]
            dm1_r = [load_r(wdec, d_dm1[128 * c:128 * (c + 1), :], 128, M1,
                            f"dm1_{c}") for c in range(4)]
            dm2_r = [load_r(wdec, d_dm2[128 * c:128 * (c + 1), :], 128, M2,
                            f"dm2_{c}") for c in range(8)]
            dow_r = [load_r(wdec, d_dow[128 * c:128 * (c + 1), :], 128, NY,
                            f"dow_{c}", BF16) for c in range(4)]
            # column-packed f32r biases: bhhn|dbhhn|dm1b|dm2b|dob
            bias_r = pe.tile([1, 2624], F32R, tag="bias_r")
            for dtn, base, w in ((d_bhhn, 0, H), (d_dbhhn, H, H),
                                 (d_dm1b, 1024, M1), (d_dm2b, 2048, M2),
                                 (d_dob, 2560, NY)):
                for lo in range(0, w, 768):
                    hi = min(w, lo + 768)
                    t = ld.tile([1, hi - lo], F32, tag="ldtmp")
                    nc.sync.dma_start(t[:], dtn[:, lo:hi])
                    nc.gpsimd.tensor_copy(bias_r[0:1, base + lo:base + hi],
                                          t[:])
            xlast_r = load_r(wdec, d_xlast[:], NX, BD, "xlastr")

            # ---------- encode loop ----------
            for t in range(et):
                xs_f = tp.tile([66, 128], F32, tag="xs_f")
                nc.sync.dma_start(xs_f[:], d_xin[:, t * BE:(t + 1) * BE])
                xs = tp.tile([66, 128], F32R, tag="xs_r")
                nc.vector.tensor_copy(xs[:], xs_f[:])

                g1a = psA.tile([BE, 512], F32, tag="gA1")
                g1b = psA.tile([BE, 512], F32, tag="gA2")
                g2 = psB.tile([BE, 512], F32, tag="gB")
                g3 = psC.tile([BE, 512], F32, tag="gC")
                # bank-grouped: finish each PSUM bank before switching
                nc.tensor.matmul(g1a[:], xs[:], wih_r[:, 0:512],
                                 start=True, stop=False)
                for c in range(4):
                    nc.tensor.matmul(g1a[:], hT[:, 128 * c:128 * (c + 1)],
                                     whh_r[c][:, 0:512],
                                     start=False, stop=(c == 3))
                nc.tensor.matmul(g1b[:], xs[:], wih_r[:, 512:1024],
                                 start=True, stop=False)
                for c in range(4):
                    nc.tensor.matmul(g1b[:], hT[:, 128 * c:128 * (c + 1)],
                                     whh_r[c][:, 512:1024],
                                     start=False, stop=(c == 3))
                for c in range(4):
                    nc.tensor.matmul(g2[:], hT[:, 128 * c:128 * (c + 1)],
                                     whh_r[c][:, 1024:1536],
                                     start=(c == 0), stop=False)
                nc.tensor.matmul(g2[:], ones_r[0:1, 0:BE],
                                 bias_r[0:1, 0:512], start=False, stop=True)
                nc.tensor.matmul(g3[:], xs[:], wih_r[:, 1024:1536],
                                 start=True, stop=True)

                r_t = tp.tile([BE, 512], F32, tag="r")
                z_t = tp.tile([BE, 512], F32, tag="z")
                omz = tp.tile([BE, 512], F32, tag="omz")
                nc.scalar.activation(r_t[:], g1a[:], AF.Sigmoid)
                nc.scalar.activation(z_t[:], g1b[:], AF.Sigmoid)
                nc.scalar.activation(omz[:], g1b[:], AF.Sigmoid,
                                     scale=-1.0)
                rhn = tp.tile([BE, 512], F32, tag="rhn")
                nc.vector.tensor_mul(rhn[:], r_t[:], g2[:])
                npre = tp.tile([BE, 512], F32, tag="npre")
                nc.vector.tensor_add(npre[:], rhn[:], g3[:])
                n_t = tp.tile([BE, 512], F32, tag="n")
                nc.scalar.activation(n_t[:], npre[:], AF.Tanh)
                a_t = tp.tile([BE, 512], F32, tag="a")
                nc.vector.tensor_mul(a_t[:], omz[:], n_t[:])
                b_t = tp.tile([BE, 512], F32, tag="b")
                nc.vector.tensor_mul(b_t[:], z_t[:], h_bh[:])
                h_new = st.tile([BE, H], F32, tag="h")
                nc.vector.tensor_add(h_new[:], a_t[:], b_t[:])

                ptr = psTR.tile([128, 512], F32, tag="tr")
                for c in range(4):
                    nc.tensor.transpose(ptr[:, 128 * c:128 * (c + 1)],
                                        h_new[:, 128 * c:128 * (c + 1)],
                                        idf[:])
                hT_new = st.tile([128, H], F32R, tag="hT")
                nc.scalar.copy(hT_new[:], ptr[:])
                hT, h_bh = hT_new, h_new

            # ---------- reshard: AllToAll of 16-row slices ----------
            nc.sync.dma_start(cc_in[:], h_bh[:])
            nc.gpsimd.collective_compute(
                "AllToAll", OP.bypass,
                replica_groups=[list(range(NCORE))],
                ins=[cc_in[:]], outs=[cc_out[:]])

            pxa = md.tile([BD, H], F32, tag="pA")
            pxb = md.tile([BD, H], F32, tag="pB")
            pya = md.tile([BD, H], F32, tag="pA")
            pyb = md.tile([BD, H], F32, tag="pB")
            nc.sync.dma_start(pxa[0:16, :], cc_out[0][:])
            nc.sync.dma_start(pxa[16:32, :], cc_out[1][:])
            nc.sync.dma_start(pxb[0:16, :], cc_out[2][:])
            nc.sync.dma_start(pxb[16:32, :], cc_out[3][:])
            nc.sync.dma_start(pya[0:16, :], cc_out[4][:])
            nc.sync.dma_start(pya[16:32, :], cc_out[5][:])
            nc.sync.dma_start(pyb[0:16, :], cc_out[6][:])
            nc.sync.dma_start(pyb[16:32, :], cc_out[7][:])
            hx = md.tile([BD, H], F32, tag="hx")
            hy = md.tile([BD, H], F32, tag="hy")
            nc.vector.tensor_add(hx[:], pxa[:], pxb[:])
            nc.vector.tensor_add(hy[:], pya[:], pyb[:])

            def trsp_b(src, cols, tag):
                """src [BD, cols] f32 -> bf16 [128, (cols//128)*BD]."""
                nch = cols // 128
                p = psTR.tile([128, nch * BD], F32, tag="tr")
                for c in range(nch):
                    nc.tensor.transpose(p[:, BD * c:BD * (c + 1)],
                                        src[:, 128 * c:128 * (c + 1)], id32)
                o = pe.tile([128, nch * BD], BF16, tag=tag)
                nc.scalar.copy(o[:], p[:])
                return o

            hxT = trsp_b(hx, H, "hxT")
            hyT = trsp_b(hy, H, "hyT")

            m1a = psA.tile([BD, 512], F32, tag="gA1")
            m1b = psA.tile([BD, 512], F32, tag="gA2")
            for c in range(8):
                wt = load_r(ws, d_em1[128 * c:128 * (c + 1), :], 128, M1,
                            f"wstr{c % 2}", BF16)
                s = (hxT if c < 4 else hyT)[:, BD * (c % 4):BD * (c % 4 + 1)]
                nc.tensor.matmul(m1a[:], s, wt[:, 0:512],
                                 start=(c == 0), stop=False)
                nc.tensor.matmul(m1b[:], s, wt[:, 512:1024],
                                 start=(c == 0), stop=False)
            nc.tensor.matmul(m1a[:], ones_b[0:1, 0:BD],
                             bias_b[0:1, 1536:2048], start=False, stop=True)
            nc.tensor.matmul(m1b[:], ones_b[0:1, 0:BD],
                             bias_b[0:1, 2048:2560], start=False, stop=True)
            hm1 = tq.tile([BD, M1], F32, tag="hm1")
            nc.scalar.activation(hm1[:, 0:512], m1a[:], AF.Relu)
            nc.scalar.activation(hm1[:, 512:1024], m1b[:], AF.Relu)
            hm1T = trsp_b(hm1, M1, "hm1T_m")

            m2 = psB.tile([BD, M2], F32, tag="gB")
            for c in range(8):
                wt = load_r(ws, d_em2[128 * c:128 * (c + 1), :], 128, M2,
                            f"wstr{c % 2}", BF16)
                nc.tensor.matmul(m2[:], hm1T[:, BD * c:BD * (c + 1)],
                                 wt[:], start=(c == 0), stop=False)
            nc.tensor.matmul(m2[:], ones_b[0:1, 0:BD], bias_b[0:1, 2560:3072],
                             start=False, stop=True)
            hm2 = tq.tile([BD, M2], F32, tag="hm2")
            nc.scalar.activation(hm2[:], m2[:], AF.Relu)
            hm2T = trsp_b(hm2, M2, "hm2T_m")

            zp = psC.tile([BD, H], F32, tag="gC")
            for c in range(4):
                wt = load_r(ws, d_eow[128 * c:128 * (c + 1), :], 128, H,
                            f"wstr{c % 2}", BF16)
                nc.tensor.matmul(zp[:], hm2T[:, BD * c:BD * (c + 1)],
                                 wt[:], start=(c == 0), stop=False)
            nc.tensor.matmul(zp[:], ones_b[0:1, 0:BD], bias_b[0:1, 3072:3584],
                             start=False, stop=True)
            z_sb = md.tile([BD, H], F32, tag="z_sb")
            nc.scalar.copy(z_sb[:], zp[:])
            zT = trsp_b(z_sb, H, "zT")

            # const = cat(h_x, z) @ d_Wih[:, :2H].T + d_bih + d_bhh(r,z)
            cpa = psA.tile([96, 512], F32, tag="gA1")
            cpa = cpa[64:96, :]
            cpb = psA.tile([96, 512], F32, tag="gA2")
            cpb = cpb[64:96, :]
            cpn = psB.tile([96, 512], F32, tag="gB")
            cpn = cpn[64:96, :]
            for c in range(8):
                wt = load_r(ws, d_dcw[128 * c:128 * (c + 1), :], 128, G,
                            f"wstr{c % 2}", BF16)
                s = (hxT if c < 4 else zT)[:, BD * (c % 4):BD * (c % 4 + 1)]
                nc.tensor.matmul(cpa[:], s, wt[:, 0:512],
                                 start=(c == 0), stop=False)
                nc.tensor.matmul(cpb[:], s, wt[:, 512:1024],
                                 start=(c == 0), stop=False)
                nc.tensor.matmul(cpn[:], s, wt[:, 1024:1536],
                                 start=(c == 0), stop=False)
            nc.tensor.matmul(cpa[:], ones_b[0:1, 0:BD],
                             bias_b[0:1, 0:512], start=False, stop=True)
            nc.tensor.matmul(cpb[:], ones_b[0:1, 0:BD],
                             bias_b[0:1, 512:1024], start=False, stop=True)
            nc.tensor.matmul(cpn[:], ones_b[0:1, 0:BD],
                             bias_b[0:1, 1024:1536], start=False, stop=True)
            nc.vector.tensor_copy(dwyc[64:96, 0:512], cpa[:])
            nc.vector.tensor_copy(dwyc[64:96, 512:1024], cpb[:])
            nc.vector.tensor_copy(dwyc[64:96, 1024:1536], cpn[:])

            # decoder init
            hdT = st.tile([128, 4 * BD], F32R, tag="hdT")
            nc.vector.tensor_copy(hdT[:], zero_f[:])
            hd = st.tile([BD, H], F32, tag="hd")
            nc.gpsimd.memset(hd[:], 0.0)
            ypT = st.tile([96, BD], F32R, tag="ypT")
            nc.vector.tensor_copy(ypT[0:NX, :], xlast_r[:])
            nc.vector.tensor_copy(ypT[64:96, :], id_hi_f[64:96, :])

            # ---------- decode loop ----------
            for t in range(hor):
                g1a = psA.tile([BD, 512], F32, tag="gA1")
                g1b = psA.tile([BD, 512], F32, tag="gA2")
                g2 = psB.tile([BD, 512], F32, tag="gB")
                g3 = psC.tile([BD, 512], F32, tag="gC")
                # h-side first: depends only on hdT (ready since last GRU
                # phase), so these stream during the previous step's MLP.
                # The yp/const matmuls close each group once ypT lands.
                for c in range(4):
                    nc.tensor.matmul(g1a[:], hdT[:, BD * c:BD * (c + 1)],
                                     dwhh_r[c][:, 0:512],
                                     start=(c == 0), stop=False)
                for c in range(4):
                    nc.tensor.matmul(g2[:], hdT[:, BD * c:BD * (c + 1)],
                                     dwhh_r[c][:, 1024:1536],
                                     start=(c == 0), stop=False)
                nc.tensor.matmul(g2[:], ones_r[0:1, 0:BD],
                                 bias_r[0:1, 512:1024], start=False, stop=True)
                for c in range(4):
                    nc.tensor.matmul(g1b[:], hdT[:, BD * c:BD * (c + 1)],
                                     dwhh_r[c][:, 512:1024],
                                     start=(c == 0), stop=False)
                nc.tensor.matmul(g1a[:], ypT[:], dwyc[:, 0:512],
                                 start=False, stop=True)
                nc.tensor.matmul(g3[:], ypT[:], dwyc[:, 1024:1536],
                                 start=True, stop=True)
                nc.tensor.matmul(g1b[:], ypT[:], dwyc[:, 512:1024],
                                 start=False, stop=True)

                r_t = tp.tile([BD, 512], F32, tag="r")
                z_t = tp.tile([BD, 512], F32, tag="z")
                omz = tp.tile([BD, 512], F32, tag="omz")
                nc.scalar.activation(r_t[:], g1a[:], AF.Sigmoid)
                nc.scalar.activation(z_t[:], g1b[:], AF.Sigmoid)
                nc.scalar.activation(omz[:], g1b[:], AF.Sigmoid,
                                     scale=-1.0)
                rhn = tp.tile([BD, 512], F32, tag="rhn")
                nc.vector.tensor_mul(rhn[:], r_t[:], g2[:])
                npre = tp.tile([BD, 512], F32, tag="npre")
                nc.vector.tensor_add(npre[:], rhn[:], g3[:])
                n_t = tp.tile([BD, 512], F32, tag="n")
                nc.scalar.activation(n_t[:], npre[:], AF.Tanh)
                a_t = tp.tile([BD, 512], F32, tag="a")
                nc.vector.tensor_mul(a_t[:], omz[:], n_t[:])
                b_t = tp.tile([BD, 512], F32, tag="b")
                nc.gpsimd.tensor_mul(b_t[:], z_t[:], hd[:])
                hd_new = st.tile([BD, H], F32, tag="hd")
                nc.vector.tensor_add(hd_new[:], a_t[:], b_t[:])

                ptr = psTR.tile([128, 4 * BD], F32, tag="tr")
                for c in range(4):
                    nc.tensor.transpose(ptr[:, BD * c:BD * (c + 1)],
                                        hd_new[:, 128 * c:128 * (c + 1)],
                                        id32)
                hdT_new = st.tile([128, 4 * BD], F32R, tag="hdT")
                nc.scalar.copy(hdT_new[:], ptr[:])
                hdT, hd = hdT_new, hd_new

                m1a = psA.tile([BD, 512], F32, tag="gA1")
                m1b = psA.tile([BD, 512], F32, tag="gA2")
                for c in range(4):
                    nc.tensor.matmul(m1a[:], hdT[:, BD * c:BD * (c + 1)],
                                     dm1_r[c][:, 0:512],
                                     start=(c == 0), stop=False)
                nc.tensor.matmul(m1a[:], ones_r[0:1, 0:BD],
                                 bias_r[0:1, 1024:1536], start=False, stop=True)
                hm1 = tq.tile([BD, M1], F32, tag="hm1")
                nc.scalar.activation(hm1[:, 0:512], m1a[:], AF.Relu)
                for c in range(4):
                    nc.tensor.matmul(m1b[:],
                                     hdT[:, BD * c:BD * (c + 1)],
                                     dm1_r[c][:, 512:1024],
                                     start=(c == 0), stop=False)
                nc.tensor.matmul(m1b[:], ones_r[0:1, 0:BD],
                                 bias_r[0:1, 1536:2048], start=False, stop=True)
                nc.scalar.activation(hm1[:, 512:1024], m1b[:], AF.Relu)
                hm1Ta = tq.tile([128, 4 * BD], F32R, tag="hm1Ta")
                hm1Tb = tq.tile([128, 4 * BD], F32R, tag="hm1Tb")
                p1 = psTR.tile([128, 4 * BD], F32, tag="tr")
                for c in range(4):
                    nc.tensor.transpose(p1[:, BD * c:BD * (c + 1)],
                                        hm1[:, 128 * c:128 * (c + 1)], id32)
                nc.vector.tensor_copy(hm1Ta[:], p1[:])
                p1b = psTR.tile([128, 4 * BD], F32, tag="tr")
                for c in range(4):
                    nc.tensor.transpose(p1b[:, BD * c:BD * (c + 1)],
                                        hm1[:, 512 + 128 * c:640 + 128 * c],
                                        id32)
                nc.vector.tensor_copy(hm1Tb[:], p1b[:])

                m2 = psB.tile([BD, M2], F32, tag="gB")
                for c in range(8):
                    s = (hm1Ta if c < 4 else hm1Tb)[:, BD * (c % 4):
                                                    BD * (c % 4 + 1)]
                    nc.tensor.matmul(m2[:], s, dm2_r[c][:],
                                     start=(c == 0), stop=False)
                nc.tensor.matmul(m2[:], ones_r[0:1, 0:BD],
                                 bias_r[0:1, 2048:2560], start=False, stop=True)
                hm2 = tq.tile([BD, M2], F32, tag="hm2")
                nc.scalar.activation(hm2[:], m2[:], AF.Relu)
                p2 = psTR.tile([128, 4 * BD], F32, tag="tr")
                for c in range(4):
                    nc.tensor.transpose(p2[:, BD * c:BD * (c + 1)],
                                        hm2[:, 128 * c:128 * (c + 1)], id32)
                hm2T = tq.tile([128, 4 * BD], BF16, tag="hm2T")
                nc.vector.tensor_copy(hm2T[:], p2[:])

                yp_ps = psC.tile([BD, NY], F32, tag="gC")
                for c in range(4):
                    nc.tensor.matmul(yp_ps[:], hm2T[:, BD * c:BD * (c + 1)],
                                     dow_r[c][:], start=(c == 0), stop=False)
                nc.tensor.matmul(yp_ps[:], ones_r[0:1, 0:BD],
                                 bias_r[0:1, 2560:2624],
                                 start=False, stop=True)
                y_sb = tp.tile([BD, NY], F32, tag="y_sb")
                nc.scalar.copy(y_sb[:], yp_ps[:])
                nc.sync.dma_start(d_out[:, NY * t:NY * (t + 1)], y_sb[:])
                if t + 1 < hor:
                    p3 = psTR.tile([NX, BD], F32, tag="tr")
                    nc.tensor.transpose(p3[:], y_sb[:], id32)
                    ypT_new = st.tile([96, BD], F32R, tag="ypT")
                    nc.scalar.copy(ypT_new[0:NX, :], p3[:])
                    nc.vector.tensor_copy(ypT_new[64:96, :],
                                          id_hi_f[64:96, :])
                    ypT = ypT_new

    nc.compile()
    return nc


# ---------------------------------------------------------------------------
# Host-side sharding
# ---------------------------------------------------------------------------

def shard_inputs(inp, et=100, x_real=50, hor=60):
    f32 = np.float32
    x, y = np.asarray(inp["x"], f32), np.asarray(inp["y"], f32)
    tx = x.shape[1]
    chains = [("xf", False, x), ("xb", True, x),
              ("ef", False, y), ("eb", True, y)]
    in_maps = []
    shared = {}

    def wih_aug(pre):
        wih = np.asarray(inp[pre + "_Wih"], f32)
        bih = np.asarray(inp[pre + "_bih"], f32)
        bhh = np.asarray(inp[pre + "_bhh"], f32)
        aug = np.zeros((66, G), f32)
        aug[0:64, :] = wih.T
        bias = bih.copy()
        bias[0:2 * H] += bhh[0:2 * H]
        aug[64, :] = bias
        aug[65, H:2 * H] = BIG
        return aug

    d_Wih = np.asarray(inp["d_Wih"], f32)
    d_bih = np.asarray(inp["d_bih"], f32)
    d_bhh = np.asarray(inp["d_bhh"], f32)
    dc_b = d_bih.copy()
    dc_b[0:2 * H] += d_bhh[0:2 * H]

    shared["em_w1t"] = np.ascontiguousarray(np.asarray(inp["em_W1"], f32).T)
    shared["em_b1row"] = np.asarray(inp["em_b1"], f32)[None, :]
    shared["em_w2t"] = np.ascontiguousarray(np.asarray(inp["em_W2"], f32).T)
    shared["em_b2row"] = np.asarray(inp["em_b2"], f32)[None, :]
    shared["eo_wt"] = np.ascontiguousarray(np.asarray(inp["eo_W"], f32).T)
    shared["eo_brow"] = np.asarray(inp["eo_b"], f32)[None, :]
    shared["dc_wt"] = np.ascontiguousarray(d_Wih[:, 0:2 * H].T)
    shared["dc_brow"] = dc_b[None, :]
    shared["dwy_t"] = np.ascontiguousarray(d_Wih[:, 2 * H:].T)
    shared["dwhh_t"] = np.ascontiguousarray(np.asarray(inp["d_Whh"], f32).T)
    shared["dbhhn_row"] = np.ascontiguousarray(d_bhh[None, 2 * H:])
    shared["dm_w1t"] = np.ascontiguousarray(np.asarray(inp["dm_W1"], f32).T)
    shared["dm_b1row"] = np.asarray(inp["dm_b1"], f32)[None, :]
    shared["dm_w2t"] = np.ascontiguousarray(np.asarray(inp["dm_W2"], f32).T)
    shared["dm_b2row"] = np.asarray(inp["dm_b2"], f32)[None, :]
    shared["do_wt"] = np.ascontiguousarray(np.asarray(inp["do_W"], f32).T)
    shared["do_brow"] = np.asarray(inp["do_b"], f32)[None, :]

    for j in range(NCORE):
        chain, half = j // 2, j % 2
        pre, rev, seq = chains[chain]
        T = seq.shape[1]
        s = seq[128 * half:128 * (half + 1)]          # [128, T, 64]
        xin = np.zeros((66, et, BE), f32)
        xin[64, :, :] = 1.0
        pad = et - T
        if pad:
            xin[65, 0:pad, :] = 1.0
        order = np.arange(T)[::-1] if rev else np.arange(T)
        xin[0:64, pad:, :] = s[:, order, :].transpose(2, 1, 0)
        m = dict(shared)
        m["xin"] = np.ascontiguousarray(xin.reshape(66, et * BE))
        m["wih_aug"] = wih_aug(pre)
        m["whh_t"] = np.ascontiguousarray(np.asarray(inp[pre + "_Whh"],
                                                     f32).T)
        m["bhhn_row"] = np.ascontiguousarray(
            np.asarray(inp[pre + "_bhh"], f32)[None, 2 * H:])
        xl = np.concatenate([x[16 * j:16 * j + 16, -1, :],
                             x[128 + 16 * j:128 + 16 * j + 16, -1, :]])
        m["xlast_t"] = np.ascontiguousarray(xl.T)
        in_maps.append(m)
    return in_maps


def unshard(results, hor=60):
    out = np.zeros((B, hor, NY), np.float32)
    for j in range(NCORE):
        o = results[j]["out"].reshape(BD, hor, NY)
        out[16 * j:16 * j + 16] = o[0:16]
        out[128 + 16 * j:128 + 16 * j + 16] = o[16:32]
    return out


_NC = None


def kernel(**inputs):
    global _NC
    from concourse.bass_utils import run_bass_kernel_spmd
    if _NC is None:
        _NC = build_nc()
    in_maps = shard_inputs(inputs)
    res = run_bass_kernel_spmd(_NC, in_maps, core_ids=list(range(NCORE)))
    return unshard(res.results)



# revision 4
# speedup vs baseline: 1.0762x; 1.0762x over previous
"""Trainium2 Bass kernel for the GRU autoencoder (bf16 edition).

Distribution strategy (8 NeuronCores):
  Encode : chain-parallel x batch-parallel. Core j handles GRU chain j//2
           (xf, xb, ef, eb) on batch half j%2 (128 rows), running a uniform
           100-step loop. The 50-step x-chains run steps 0..49 then 50 exact
           identity steps (update-gate pre-activation forced to +BIG => z=1
           => h'=h) so their hidden is final at step 50 and the x-reshard
           collective overlaps encode steps 51..99.
  Reshard: two AllToAlls of 16-row slices (x-parts early, y-parts at end);
           every core assembles hidden states for its own 32-row decode
           shard at fixed SPMD-uniform indices:
           core j decodes global rows [16j:16j+16] u [128+16j:+16].
  Middle : per-shard MLP (em1/em2/eo) + decoder const precompute.
  Decode : 60 autoregressive steps per core on its 32-row shard.

All matmuls use bf16 operands (fp32 PSUM accumulation): bf16 MMs measure
~2x faster than fp32r on this part (193ns vs 397ns warm at N=512) and the
per-step xs-side MMs are emitted one step ahead so the PE has fill work
during each step's elementwise tail.

PSUM budget (8 banks): gA 2 | gB 2 | gC 2 | tr 2.
"""

import sys

sys.path.insert(0, "/opt/trn_rl_repo")

import numpy as np
import ml_dtypes

import concourse.bass as bass
import concourse.mybir as mybir
import concourse.tile as tile
from concourse import bacc
from concourse.masks import make_identity

dt = mybir.dt
AF = mybir.ActivationFunctionType
OP = mybir.AluOpType

B, TX, TY, NX, NY, H, HOR = 256, 50, 100, 64, 64, 512, 60
M1, M2 = 1024, 512
G = 3 * H
NCORE = 8
BE = 128   # encoder batch rows per core
BD = 32    # decoder batch rows per core
BIG = 30000.0
XSYNC = 52  # encode step at which the x-part reshard fires

F32, BF16 = dt.float32, dt.bfloat16
NPBF = ml_dtypes.bfloat16

# packed bias row layout (single [1, 6208] bf16 tensor)
B_DC, B_EM1, B_EM2, B_EO = 0, G, G + M1, G + M1 + M2
B_BHN, B_DBHN = 3584, 3584 + H
B_DM1, B_DM2, B_DO = 4608, 5632, 6144
BIAS_W = 6208


def build_nc(et=100, hor=60):
    nc = bacc.Bacc("TRN2", target_bir_lowering=False, debug=False,
                   num_devices=NCORE)

    # ---- DRAM parameters (identical names on every core; content differs) --
    d_xin = nc.dram_tensor("xin", [66, et * BE], BF16, kind="ExternalInput")
    d_wih = nc.dram_tensor("wih_aug", [66, G], BF16, kind="ExternalInput")
    d_whh = nc.dram_tensor("whh_t", [H, G], BF16, kind="ExternalInput")

    d_em1 = nc.dram_tensor("em_w1t", [2 * H, M1], BF16, kind="ExternalInput")
    d_em2 = nc.dram_tensor("em_w2t", [M1, M2], BF16, kind="ExternalInput")
    d_eow = nc.dram_tensor("eo_wt", [M2, H], BF16, kind="ExternalInput")
    d_dcw = nc.dram_tensor("dc_wt", [2 * H, G], BF16, kind="ExternalInput")

    d_dwy = nc.dram_tensor("dwy_t", [NY, G], BF16, kind="ExternalInput")
    d_dwhh = nc.dram_tensor("dwhh_t", [H, G], BF16, kind="ExternalInput")
    d_dm1 = nc.dram_tensor("dm_w1t", [H, M1], BF16, kind="ExternalInput")
    d_dm2 = nc.dram_tensor("dm_w2t", [M1, M2], BF16, kind="ExternalInput")
    d_dow = nc.dram_tensor("do_wt", [M2, NY], BF16, kind="ExternalInput")
    d_bias = nc.dram_tensor("biases", [1, BIAS_W], BF16, kind="ExternalInput")
    d_xlast = nc.dram_tensor("xlast_t", [NX, BD], BF16, kind="ExternalInput")

    d_out = nc.dram_tensor("out", [BD, hor * NY], F32, kind="ExternalOutput")

    cc_in_x = nc.dram_tensor("cc_in_x", [BE, H], BF16)
    cc_out_x = nc.dram_tensor("cc_out_x", [NCORE, 16, H], BF16)
    cc_in_y = nc.dram_tensor("cc_in_y", [BE, H], BF16)
    cc_out_y = nc.dram_tensor("cc_out_y", [NCORE, 16, H], BF16)

    with tile.TileContext(nc) as tc:
        with tc.tile_pool(name="wts", bufs=1) as wt, \
             tc.tile_pool(name="state", bufs=2) as st, \
             tc.tile_pool(name="xs", bufs=3) as xsp, \
             tc.tile_pool(name="tmp", bufs=2) as tp, \
             tc.tile_pool(name="tmp1", bufs=1) as tq, \
             tc.tile_pool(name="mid", bufs=1) as md, \
             tc.tile_pool(name="persist", bufs=1) as pe, \
             tc.tile_pool(name="psA", bufs=1, space="PSUM") as psA, \
             tc.tile_pool(name="psB", bufs=1, space="PSUM") as psB, \
             tc.tile_pool(name="psC", bufs=1, space="PSUM") as psC, \
             tc.tile_pool(name="psTR", bufs=2, space="PSUM") as psTR:

            # ---------- constants ----------
            idf = pe.tile([128, 128], F32, tag="idf")
            make_identity(nc, idf[:])
            idb = pe.tile([128, 128], BF16, tag="idb")
            nc.gpsimd.tensor_copy(idb[:], idf[:])
            id32f = idf[0:32, 0:32]
            id32b = idb[0:32, 0:32]
            ones_b = pe.tile([1, 128], BF16, tag="ones_b")
            nc.gpsimd.memset(ones_b[:], 1.0)
            zero_b = pe.tile([128, 512], BF16, tag="zero_b")
            nc.gpsimd.memset(zero_b[:], 0.0)

            # ---------- encoder weights (needed immediately) ----------
            wih = wt.tile([66, G], BF16, tag="wih")
            nc.sync.dma_start(wih[:], d_wih[:])
            whh = [wt.tile([128, G], BF16, tag=f"whh{c}", name=f"whh{c}")
                   for c in range(4)]
            for c in range(4):
                nc.sync.dma_start(whh[c][:], d_whh[128 * c:128 * (c + 1), :])
            bias = pe.tile([1, BIAS_W], BF16, tag="bias")
            nc.sync.dma_start(bias[:], d_bias[:])

            # ---------- encoder state ----------
            hT = pe.tile([128, H], BF16, tag="hT0")
            nc.vector.tensor_copy(hT[:], zero_b[:])
            h_bh = pe.tile([BE, H], BF16, tag="h0")
            nc.gpsimd.tensor_copy(h_bh[:], zero_b[:])

            # ---------- middle/decoder weights (DMA during encode) ----------
            em1 = [wt.tile([128, M1], BF16, tag=f"em1_{c}", name=f"em1_{c}")
                   for c in range(8)]
            for c in range(8):
                nc.sync.dma_start(em1[c][:], d_em1[128 * c:128 * (c + 1), :])
            em2 = [wt.tile([128, M2], BF16, tag=f"em2_{c}", name=f"em2_{c}")
                   for c in range(8)]
            for c in range(8):
                nc.sync.dma_start(em2[c][:], d_em2[128 * c:128 * (c + 1), :])
            eow = [wt.tile([128, H], BF16, tag=f"eow{c}", name=f"eow{c}")
                   for c in range(4)]
            for c in range(4):
                nc.sync.dma_start(eow[c][:], d_eow[128 * c:128 * (c + 1), :])
            dcw = [wt.tile([128, G], BF16, tag=f"dcw{c}", name=f"dcw{c}")
                   for c in range(8)]
            for c in range(8):
                nc.sync.dma_start(dcw[c][:], d_dcw[128 * c:128 * (c + 1), :])
            dwyc = wt.tile([96, G], BF16, tag="dwyc")
            nc.sync.dma_start(dwyc[0:NY, :], d_dwy[:])
            dwhh = [wt.tile([128, G], BF16, tag=f"dwhh{c}", name=f"dwhh{c}")
                    for c in range(4)]
            for c in range(4):
                nc.sync.dma_start(dwhh[c][:], d_dwhh[128 * c:128 * (c + 1), :])
            dm1 = [wt.tile([128, M1], BF16, tag=f"dm1_{c}", name=f"dm1_{c}")
                   for c in range(4)]
            for c in range(4):
                nc.sync.dma_start(dm1[c][:], d_dm1[128 * c:128 * (c + 1), :])
            dm2 = [wt.tile([128, M2], BF16, tag=f"dm2_{c}", name=f"dm2_{c}")
                   for c in range(8)]
            for c in range(8):
                nc.sync.dma_start(dm2[c][:], d_dm2[128 * c:128 * (c + 1), :])
            dow = [wt.tile([128, NY], BF16, tag=f"dow_{c}", name=f"dow_{c}")
                   for c in range(4)]
            for c in range(4):
                nc.sync.dma_start(dow[c][:], d_dow[128 * c:128 * (c + 1), :])
            xlast = wt.tile([NX, BD], BF16, tag="xlast")
            nc.sync.dma_start(xlast[:], d_xlast[:])
            # identity block staged at partitions 64:96 (for ypT rows)
            id_hi = pe.tile([96, 32], BF16, tag="id_hi")
            nc.sync.dma_start(id_hi[64:96, :], idb[0:32, 0:32])

            # ---------- encode loop ----------
            # Rolling PSUM groups: the xs-side (input-gate) MMs for step t+1
            # are emitted during step t so they fill the PE while the
            # elementwise tail runs.  gA1=r, gA2=z, gB=h-side n, gC=x-side n.
            def open_groups(t):
                xs = xsp.tile([66, 128], BF16, tag="xs")
                nc.sync.dma_start(xs[:], d_xin[:, t * BE:(t + 1) * BE])
                g1a = psA.tile([BE, 512], F32, tag="gA1")
                g1b = psA.tile([BE, 512], F32, tag="gA2")
                g3 = psC.tile([BE, 512], F32, tag="gC")
                nc.tensor.matmul(g1a[:], xs[:], wih[:, 0:512],
                                 start=True, stop=False)
                nc.tensor.matmul(g1b[:], xs[:], wih[:, 512:1024],
                                 start=True, stop=False)
                nc.tensor.matmul(g3[:], xs[:], wih[:, 1024:1536],
                                 start=True, stop=True)
                return g1a, g1b, g3

            groups = open_groups(0)
            for t in range(et):
                g1a, g1b, g3 = groups
                g2 = psB.tile([BE, 512], F32, tag="gB")
                # close the accumulation groups with the h-recurrent MMs
                for c in range(4):
                    nc.tensor.matmul(g1a[:], hT[:, 128 * c:128 * (c + 1)],
                                     whh[c][:, 0:512],
                                     start=False, stop=(c == 3))
                nc.tensor.matmul(g2[:], ones_b[0:1, 0:BE],
                                 bias[0:1, B_BHN:B_BHN + 512],
                                 start=True, stop=False)
                for c in range(4):
                    nc.tensor.matmul(g2[:], hT[:, 128 * c:128 * (c + 1)],
                                     whh[c][:, 1024:1536],
                                     start=False, stop=(c == 3))
                for c in range(4):
                    nc.tensor.matmul(g1b[:], hT[:, 128 * c:128 * (c + 1)],
                                     whh[c][:, 512:1024],
                                     start=False, stop=(c == 3))
                # open next step's groups (independent fill work for the PE)
                if t + 1 < et:
                    groups = open_groups(t + 1)

                r_t = tp.tile([BE, 512], BF16, tag="r")
                z_t = tp.tile([BE, 512], BF16, tag="z")
                omz = tp.tile([BE, 512], BF16, tag="omz")
                nc.scalar.activation(r_t[:], g1a[:], AF.Sigmoid)
                rhn = tp.tile([BE, 512], F32, tag="rhn")
                nc.vector.tensor_mul(rhn[:], r_t[:], g2[:])
                npre = tp.tile([BE, 512], F32, tag="npre")
                nc.vector.tensor_add(npre[:], rhn[:], g3[:])
                n_t = tp.tile([BE, 512], BF16, tag="n")
                nc.scalar.activation(n_t[:], npre[:], AF.Tanh)
                nc.scalar.activation(z_t[:], g1b[:], AF.Sigmoid)
                nc.scalar.activation(omz[:], g1b[:], AF.Sigmoid, scale=-1.0)
                a_t = tp.tile([BE, 512], BF16, tag="a")
                nc.vector.tensor_mul(a_t[:], omz[:], n_t[:])
                b_t = tp.tile([BE, 512], BF16, tag="b")
                nc.gpsimd.tensor_mul(b_t[:], z_t[:], h_bh[:])
                h_new = st.tile([BE, H], BF16, tag="h")
                nc.vector.tensor_add(h_new[:], a_t[:], b_t[:])

                ptr = psTR.tile([128, 512], BF16, tag="tr")
                for c in range(4):
                    nc.tensor.transpose(ptr[:, 128 * c:128 * (c + 1)],
                                        h_new[:, 128 * c:128 * (c + 1)],
                                        idb[:])
                hT_new = st.tile([128, H], BF16, tag="hT")
                nc.scalar.copy(hT_new[:], ptr[:])
                hT, h_bh = hT_new, h_new

                if t == XSYNC:
                    # x-chain hiddens are final after step 50: reshard them
                    # now so the collective overlaps the rest of encode.
                    hx_snap = pe.tile([BE, H], BF16, tag="hx_snap")
                    nc.vector.tensor_copy(hx_snap[:], h_bh[:])
                    nc.sync.dma_start(cc_in_x[:], hx_snap[:])
                    nc.gpsimd.collective_compute(
                        "AllToAll", OP.bypass,
                        replica_groups=[list(range(NCORE))],
                        ins=[cc_in_x[:]], outs=[cc_out_x[:]])

            # ---------- reshard: y-part AllToAll ----------
            nc.sync.dma_start(cc_in_y[:], h_bh[:])
            nc.gpsimd.collective_compute(
                "AllToAll", OP.bypass,
                replica_groups=[list(range(NCORE))],
                ins=[cc_in_y[:]], outs=[cc_out_y[:]])

            pxa = md.tile([BD, H], BF16, tag="pA")
            pxb = md.tile([BD, H], BF16, tag="pB")
            pya = md.tile([BD, H], BF16, tag="pC")
            pyb = md.tile([BD, H], BF16, tag="pD")
            nc.sync.dma_start(pxa[0:16, :], cc_out_x[0][:])
            nc.sync.dma_start(pxa[16:32, :], cc_out_x[1][:])
            nc.sync.dma_start(pxb[0:16, :], cc_out_x[2][:])
            nc.sync.dma_start(pxb[16:32, :], cc_out_x[3][:])
            nc.sync.dma_start(pya[0:16, :], cc_out_y[4][:])
            nc.sync.dma_start(pya[16:32, :], cc_out_y[5][:])
            nc.sync.dma_start(pyb[0:16, :], cc_out_y[6][:])
            nc.sync.dma_start(pyb[16:32, :], cc_out_y[7][:])
            hx = md.tile([BD, H], BF16, tag="hx")
            hy = md.tile([BD, H], BF16, tag="hy")
            nc.vector.tensor_add(hx[:], pxa[:], pxb[:])
            nc.vector.tensor_add(hy[:], pya[:], pyb[:])

            def trsp_b(src, cols, tag):
                """src [BD, cols] bf16 -> bf16 [128, (cols//128)*BD]."""
                nch = cols // 128
                p = psTR.tile([128, nch * BD], BF16, tag="tr")
                for c in range(nch):
                    nc.tensor.transpose(p[:, BD * c:BD * (c + 1)],
                                        src[:, 128 * c:128 * (c + 1)], id32b)
                o = pe.tile([128, nch * BD], BF16, tag=tag)
                nc.scalar.copy(o[:], p[:])
                return o

            hxT = trsp_b(hx, H, "hxT")
            hyT = trsp_b(hy, H, "hyT")

            m1a = psA.tile([BD, 512], F32, tag="gA1")
            m1b = psA.tile([BD, 512], F32, tag="gA2")
            for c in range(8):
                s = (hxT if c < 4 else hyT)[:, BD * (c % 4):BD * (c % 4 + 1)]
                nc.tensor.matmul(m1a[:], s, em1[c][:, 0:512],
                                 start=(c == 0), stop=False)
                nc.tensor.matmul(m1b[:], s, em1[c][:, 512:1024],
                                 start=(c == 0), stop=False)
            nc.tensor.matmul(m1a[:], ones_b[0:1, 0:BD],
                             bias[0:1, B_EM1:B_EM1 + 512],
                             start=False, stop=True)
            nc.tensor.matmul(m1b[:], ones_b[0:1, 0:BD],
                             bias[0:1, B_EM1 + 512:B_EM1 + 1024],
                             start=False, stop=True)
            hm1 = tq.tile([BD, M1], BF16, tag="hm1m")
            nc.scalar.activation(hm1[:, 0:512], m1a[:], AF.Relu)
            nc.scalar.activation(hm1[:, 512:1024], m1b[:], AF.Relu)
            hm1T = trsp_b(hm1, M1, "hm1T_m")

            m2 = psB.tile([BD, M2], F32, tag="gB")
            for c in range(8):
                nc.tensor.matmul(m2[:], hm1T[:, BD * c:BD * (c + 1)],
                                 em2[c][:], start=(c == 0), stop=False)
            nc.tensor.matmul(m2[:], ones_b[0:1, 0:BD],
                             bias[0:1, B_EM2:B_EM2 + 512],
                             start=False, stop=True)
            hm2 = tq.tile([BD, M2], BF16, tag="hm2m")
            nc.scalar.activation(hm2[:], m2[:], AF.Relu)
            hm2T = trsp_b(hm2, M2, "hm2T_m")

            zp = psC.tile([BD, H], F32, tag="gC")
            for c in range(4):
                nc.tensor.matmul(zp[:], hm2T[:, BD * c:BD * (c + 1)],
                                 eow[c][:], start=(c == 0), stop=False)
            nc.tensor.matmul(zp[:], ones_b[0:1, 0:BD],
                             bias[0:1, B_EO:B_EO + 512],
                             start=False, stop=True)
            z_sb = md.tile([BD, H], BF16, tag="z_sb")
            nc.scalar.copy(z_sb[:], zp[:])
            zT = trsp_b(z_sb, H, "zT")

            # const = cat(h_x, z) @ d_Wih[:, :2H].T + d_bih + d_bhh(r,z)
            cpa = psA.tile([96, 512], F32, tag="gA1")
            cpa = cpa[64:96, :]
            cpb = psA.tile([96, 512], F32, tag="gA2")
            cpb = cpb[64:96, :]
            cpn = psB.tile([96, 512], F32, tag="gB")
            cpn = cpn[64:96, :]
            for c in range(8):
                s = (hxT if c < 4 else zT)[:, BD * (c % 4):BD * (c % 4 + 1)]
                nc.tensor.matmul(cpa[:], s, dcw[c][:, 0:512],
                                 start=(c == 0), stop=False)
                nc.tensor.matmul(cpb[:], s, dcw[c][:, 512:1024],
                                 start=(c == 0), stop=False)
                nc.tensor.matmul(cpn[:], s, dcw[c][:, 1024:1536],
                                 start=(c == 0), stop=False)
            nc.tensor.matmul(cpa[:], ones_b[0:1, 0:BD],
                             bias[0:1, B_DC:B_DC + 512], start=False, stop=True)
            nc.tensor.matmul(cpb[:], ones_b[0:1, 0:BD],
                             bias[0:1, B_DC + 512:B_DC + 1024],
                             start=False, stop=True)
            nc.tensor.matmul(cpn[:], ones_b[0:1, 0:BD],
                             bias[0:1, B_DC + 1024:B_DC + 1536],
                             start=False, stop=True)
            nc.vector.tensor_copy(dwyc[64:96, 0:512], cpa[:])
            nc.vector.tensor_copy(dwyc[64:96, 512:1024], cpb[:])
            nc.vector.tensor_copy(dwyc[64:96, 1024:1536], cpn[:])

            # decoder init
            hdT = st.tile([128, 4 * BD], BF16, tag="hdT")
            nc.vector.tensor_copy(hdT[:], zero_b[:, 0:4 * BD])
            hd = st.tile([BD, H], BF16, tag="hd")
            nc.gpsimd.tensor_copy(hd[:], zero_b[0:BD, :])
            ypT = st.tile([96, BD], BF16, tag="ypT")
            nc.vector.tensor_copy(ypT[0:NX, :], xlast[:])
            nc.vector.tensor_copy(ypT[64:96, :], id_hi[64:96, :])

            # ---------- decode loop ----------
            for t in range(hor):
                g1a = psA.tile([BD, 512], F32, tag="gA1")
                g1b = psA.tile([BD, 512], F32, tag="gA2")
                g2 = psB.tile([BD, 512], F32, tag="gB")
                g3 = psC.tile([BD, 512], F32, tag="gC")
                # h-side first: depends only on hdT (ready since last GRU
                # phase), so these stream during the previous step's MLP.
                # The yp/const matmuls close each group once ypT lands.
                for c in range(4):
                    nc.tensor.matmul(g1a[:], hdT[:, BD * c:BD * (c + 1)],
                                     dwhh[c][:, 0:512],
                                     start=(c == 0), stop=False)
                for c in range(4):
                    nc.tensor.matmul(g2[:], hdT[:, BD * c:BD * (c + 1)],
                                     dwhh[c][:, 1024:1536],
                                     start=(c == 0), stop=False)
                nc.tensor.matmul(g2[:], ones_b[0:1, 0:BD],
                                 bias[0:1, B_DBHN:B_DBHN + 512],
                                 start=False, stop=True)
                for c in range(4):
                    nc.tensor.matmul(g1b[:], hdT[:, BD * c:BD * (c + 1)],
                                     dwhh[c][:, 512:1024],
                                     start=(c == 0), stop=False)
                nc.tensor.matmul(g1a[:], ypT[:], dwyc[:, 0:512],
                                 start=False, stop=True)
                nc.tensor.matmul(g3[:], ypT[:], dwyc[:, 1024:1536],
                                 start=True, stop=True)
                nc.tensor.matmul(g1b[:], ypT[:], dwyc[:, 512:1024],
                                 start=False, stop=True)

                r_t = tp.tile([BD, 512], BF16, tag="r")
                z_t = tp.tile([BD, 512], BF16, tag="z")
                omz = tp.tile([BD, 512], BF16, tag="omz")
                nc.scalar.activation(r_t[:], g1a[:], AF.Sigmoid)
                rhn = tp.tile([BD, 512], F32, tag="rhn")
                nc.vector.tensor_mul(rhn[:], r_t[:], g2[:])
                npre = tp.tile([BD, 512], F32, tag="npre")
                nc.vector.tensor_add(npre[:], rhn[:], g3[:])
                n_t = tp.tile([BD, 512], BF16, tag="n")
                nc.scalar.activation(n_t[:], npre[:], AF.Tanh)
                nc.scalar.activation(z_t[:], g1b[:], AF.Sigmoid)
                nc.scalar.activation(omz[:], g1b[:], AF.Sigmoid, scale=-1.0)
                a_t = tp.tile([BD, 512], BF16, tag="a")
                nc.vector.tensor_mul(a_t[:], omz[:], n_t[:])
                b_t = tp.tile([BD, 512], BF16, tag="b")
                nc.gpsimd.tensor_mul(b_t[:], z_t[:], hd[:])
                hd_new = st.tile([BD, H], BF16, tag="hd")
                nc.vector.tensor_add(hd_new[:], a_t[:], b_t[:])

                ptr = psTR.tile([128, 4 * BD], BF16, tag="tr")
                for c in range(4):
                    nc.tensor.transpose(ptr[:, BD * c:BD * (c + 1)],
                                        hd_new[:, 128 * c:128 * (c + 1)],
                                        id32b)
                hdT_new = st.tile([128, 4 * BD], BF16, tag="hdT")
                nc.scalar.copy(hdT_new[:], ptr[:])
                hdT, hd = hdT_new, hd_new

                m1a = psA.tile([BD, 512], F32, tag="gA1")
                m1b = psA.tile([BD, 512], F32, tag="gA2")
                for c in range(4):
                    nc.tensor.matmul(m1a[:], hdT[:, BD * c:BD * (c + 1)],
                                     dm1[c][:, 0:512],
                                     start=(c == 0), stop=False)
                nc.tensor.matmul(m1a[:], ones_b[0:1, 0:BD],
                                 bias[0:1, B_DM1:B_DM1 + 512],
                                 start=False, stop=True)
                hm1 = tq.tile([BD, M1], BF16, tag="hm1")
                nc.scalar.activation(hm1[:, 0:512], m1a[:], AF.Relu)
                for c in range(4):
                    nc.tensor.matmul(m1b[:],
                                     hdT[:, BD * c:BD * (c + 1)],
                                     dm1[c][:, 512:1024],
                                     start=(c == 0), stop=False)
                nc.tensor.matmul(m1b[:], ones_b[0:1, 0:BD],
                                 bias[0:1, B_DM1 + 512:B_DM1 + 1024],
                                 start=False, stop=True)
                nc.scalar.activation(hm1[:, 512:1024], m1b[:], AF.Relu)
                hm1Ta = tq.tile([128, 4 * BD], BF16, tag="hm1Ta")
                hm1Tb = tq.tile([128, 4 * BD], BF16, tag="hm1Tb")
                p1 = psTR.tile([128, 4 * BD], BF16, tag="tr")
                for c in range(4):
                    nc.tensor.transpose(p1[:, BD * c:BD * (c + 1)],
                                        hm1[:, 128 * c:128 * (c + 1)], id32b)
                nc.vector.tensor_copy(hm1Ta[:], p1[:])
                p1b = psTR.tile([128, 4 * BD], BF16, tag="tr")
                for c in range(4):
                    nc.tensor.transpose(p1b[:, BD * c:BD * (c + 1)],
                                        hm1[:, 512 + 128 * c:640 + 128 * c],
                                        id32b)
                nc.vector.tensor_copy(hm1Tb[:], p1b[:])

                m2 = psB.tile([BD, M2], F32, tag="gB")
                for c in range(8):
                    s = (hm1Ta if c < 4 else hm1Tb)[:, BD * (c % 4):
                                                    BD * (c % 4 + 1)]
                    nc.tensor.matmul(m2[:], s, dm2[c][:],
                                     start=(c == 0), stop=False)
                nc.tensor.matmul(m2[:], ones_b[0:1, 0:BD],
                                 bias[0:1, B_DM2:B_DM2 + 512],
                                 start=False, stop=True)
                hm2 = tq.tile([BD, M2], BF16, tag="hm2")
                nc.scalar.activation(hm2[:], m2[:], AF.Relu)
                p2 = psTR.tile([128, 4 * BD], BF16, tag="tr")
                for c in range(4):
                    nc.tensor.transpose(p2[:, BD * c:BD * (c + 1)],
                                        hm2[:, 128 * c:128 * (c + 1)], id32b)
                hm2T = tq.tile([128, 4 * BD], BF16, tag="hm2T")
                nc.vector.tensor_copy(hm2T[:], p2[:])

                yp_ps = psC.tile([BD, NY], F32, tag="gC")
                for c in range(4):
                    nc.tensor.matmul(yp_ps[:], hm2T[:, BD * c:BD * (c + 1)],
                                     dow[c][:], start=(c == 0), stop=False)
                nc.tensor.matmul(yp_ps[:], ones_b[0:1, 0:BD],
                                 bias[0:1, B_DO:B_DO + NY],
                                 start=False, stop=True)
                y_sb = tp.tile([BD, NY], F32, tag="y_sb")
                nc.scalar.copy(y_sb[:], yp_ps[:])
                nc.sync.dma_start(d_out[:, NY * t:NY * (t + 1)], y_sb[:])
                if t + 1 < hor:
                    yb = tp.tile([BD, NY], BF16, tag="yb")
                    nc.vector.tensor_copy(yb[:], yp_ps[:])
                    p3 = psTR.tile([NX, BD], BF16, tag="tr")
                    nc.tensor.transpose(p3[:], yb[:], id32b)
                    ypT_new = st.tile([96, BD], BF16, tag="ypT")
                    nc.scalar.copy(ypT_new[0:NX, :], p3[:])
                    nc.vector.tensor_copy(ypT_new[64:96, :],
                                          id_hi[64:96, :])
                    ypT = ypT_new

    nc.compile()
    return nc


# ---------------------------------------------------------------------------
# Host-side sharding
# ---------------------------------------------------------------------------

def shard_inputs(inp, et=100, hor=60):
    f32 = np.float32
    x, y = np.asarray(inp["x"], f32), np.asarray(inp["y"], f32)
    chains = [("xf", False, x), ("xb", True, x),
              ("ef", False, y), ("eb", True, y)]
    in_maps = []
    shared = {}

    def bf(a):
        return np.ascontiguousarray(np.asarray(a, f32)).astype(NPBF)

    def wih_aug(pre):
        wih = np.asarray(inp[pre + "_Wih"], f32)
        bih = np.asarray(inp[pre + "_bih"], f32)
        bhh = np.asarray(inp[pre + "_bhh"], f32)
        aug = np.zeros((66, G), f32)
        aug[0:64, :] = wih.T
        b = bih.copy()
        b[0:2 * H] += bhh[0:2 * H]
        aug[64, :] = b
        aug[65, H:2 * H] = BIG
        return bf(aug)

    d_Wih = np.asarray(inp["d_Wih"], f32)
    d_bih = np.asarray(inp["d_bih"], f32)
    d_bhh = np.asarray(inp["d_bhh"], f32)
    dc_b = d_bih.copy()
    dc_b[0:2 * H] += d_bhh[0:2 * H]

    shared["em_w1t"] = bf(np.asarray(inp["em_W1"], f32).T)
    shared["em_w2t"] = bf(np.asarray(inp["em_W2"], f32).T)
    shared["eo_wt"] = bf(np.asarray(inp["eo_W"], f32).T)
    shared["dc_wt"] = bf(d_Wih[:, 0:2 * H].T)
    shared["dwy_t"] = bf(d_Wih[:, 2 * H:].T)
    shared["dwhh_t"] = bf(np.asarray(inp["d_Whh"], f32).T)
    shared["dm_w1t"] = bf(np.asarray(inp["dm_W1"], f32).T)
    shared["dm_w2t"] = bf(np.asarray(inp["dm_W2"], f32).T)
    shared["do_wt"] = bf(np.asarray(inp["do_W"], f32).T)

    def bias_pack(pre):
        bz = np.zeros((1, BIAS_W), f32)
        bz[0, B_DC:B_DC + G] = dc_b
        bz[0, B_EM1:B_EM1 + M1] = np.asarray(inp["em_b1"], f32)
        bz[0, B_EM2:B_EM2 + M2] = np.asarray(inp["em_b2"], f32)
        bz[0, B_EO:B_EO + H] = np.asarray(inp["eo_b"], f32)
        bz[0, B_BHN:B_BHN + H] = np.asarray(inp[pre + "_bhh"], f32)[2 * H:]
        bz[0, B_DBHN:B_DBHN + H] = d_bhh[2 * H:]
        bz[0, B_DM1:B_DM1 + M1] = np.asarray(inp["dm_b1"], f32)
        bz[0, B_DM2:B_DM2 + M2] = np.asarray(inp["dm_b2"], f32)
        bz[0, B_DO:B_DO + NY] = np.asarray(inp["do_b"], f32)
        return bf(bz)

    for j in range(NCORE):
        chain, half = j // 2, j % 2
        pre, rev, seq = chains[chain]
        T = seq.shape[1]
        s = seq[128 * half:128 * (half + 1)]          # [128, T, 64]
        xin = np.zeros((66, et, BE), f32)
        xin[64, :, :] = 1.0
        if T < et:
            xin[65, T:, :] = 1.0                      # end padding: hold h
        order = np.arange(T)[::-1] if rev else np.arange(T)
        xin[0:64, :T, :] = s[:, order, :].transpose(2, 1, 0)
        m = dict(shared)
        m["xin"] = bf(xin.reshape(66, et * BE))
        m["wih_aug"] = wih_aug(pre)
        m["whh_t"] = bf(np.asarray(inp[pre + "_Whh"], f32).T)
        m["biases"] = bias_pack(pre)
        xl = np.concatenate([x[16 * j:16 * j + 16, -1, :],
                             x[128 + 16 * j:128 + 16 * j + 16, -1, :]])
        m["xlast_t"] = bf(xl.T)
        in_maps.append(m)
    return in_maps


def unshard(results, hor=60):
    out = np.zeros((B, hor, NY), np.float32)
    for j in range(NCORE):
        o = results[j]["out"].reshape(BD, hor, NY)
        out[16 * j:16 * j + 16] = o[0:16]
        out[128 + 16 * j:128 + 16 * j + 16] = o[16:32]
    return out


_NC = None


def kernel(**inputs):
    global _NC
    from concourse.bass_utils import run_bass_kernel_spmd
    if _NC is None:
        _NC = build_nc()
    in_maps = shard_inputs(inputs)
    res = run_bass_kernel_spmd(_NC, in_maps, core_ids=list(range(NCORE)))
    return unshard(res.results)


# revision 6
# speedup vs baseline: 1.2799x; 1.1893x over previous
"""Trainium2 Bass kernel for the GRU autoencoder (bf16 edition).

Distribution strategy (8 NeuronCores):
  Encode : chain-parallel x batch-parallel. Core j handles GRU chain j//2
           (xf, xb, ef, eb) on batch half j%2 (128 rows), running a uniform
           100-step loop. The 50-step x-chains run steps 0..49 then 50 exact
           identity steps (update-gate pre-activation forced to +BIG => z=1
           => h'=h) so their hidden is final at step 50 and the x-reshard
           collective overlaps encode steps 51..99.
  Reshard: two AllToAlls of 16-row slices (x-parts early, y-parts at end);
           every core assembles hidden states for its own 32-row decode
           shard at fixed SPMD-uniform indices:
           core j decodes global rows [16j:16j+16] u [128+16j:+16].
  Middle : per-shard MLP (em1/em2/eo) + decoder const precompute.
  Decode : 60 autoregressive steps per core on its 32-row shard.

All matmuls use bf16 operands (fp32 PSUM accumulation): bf16 MMs measure
~2x faster than fp32r on this part (193ns vs 397ns warm at N=512) and the
per-step xs-side MMs are emitted one step ahead so the PE has fill work
during each step's elementwise tail.

PSUM budget (8 banks): gA 2 | gB 2 | gC 2 | tr 2.
"""

import sys

sys.path.insert(0, "/opt/trn_rl_repo")

import numpy as np
import ml_dtypes

import concourse.bass as bass
import concourse.mybir as mybir
import concourse.tile as tile
from concourse import bacc
from concourse.masks import make_identity

dt = mybir.dt
AF = mybir.ActivationFunctionType
OP = mybir.AluOpType

B, TX, TY, NX, NY, H, HOR = 256, 50, 100, 64, 64, 512, 60
M1, M2 = 1024, 512
G = 3 * H
NCORE = 8
BE = 128   # encoder batch rows per core
BD = 32    # decoder batch rows per core
BIG = 30000.0
XSYNC = 52  # encode step at which the x-part reshard fires

F32, BF16 = dt.float32, dt.bfloat16
NPBF = ml_dtypes.bfloat16

# packed bias row layout (single [1, 6208] bf16 tensor)
B_DC, B_EM1, B_EM2, B_EO = 0, G, G + M1, G + M1 + M2
B_BHN, B_DBHN = 3584, 3584 + H
B_DM1, B_DM2, B_DO = 4608, 5632, 6144
BIAS_W = 6208


def build_nc(et=100, hor=60):
    nc = bacc.Bacc("TRN2", target_bir_lowering=False, debug=False,
                   num_devices=NCORE)

    # ---- DRAM parameters (identical names on every core; content differs) --
    d_xin = nc.dram_tensor("xin", [66, et * BE], BF16, kind="ExternalInput")
    d_wih = nc.dram_tensor("wih_aug", [66, G], BF16, kind="ExternalInput")
    d_whh = nc.dram_tensor("whh_t", [H, G], BF16, kind="ExternalInput")

    d_em1 = nc.dram_tensor("em_w1t", [2 * H, M1], BF16, kind="ExternalInput")
    d_em2 = nc.dram_tensor("em_w2t", [M1, M2], BF16, kind="ExternalInput")
    d_eow = nc.dram_tensor("eo_wt", [M2, H], BF16, kind="ExternalInput")
    d_dcw = nc.dram_tensor("dc_wt", [2 * H, G], BF16, kind="ExternalInput")

    d_dwy = nc.dram_tensor("dwy_t", [NY, G], BF16, kind="ExternalInput")
    d_dwhh = nc.dram_tensor("dwhh_t", [H, G], BF16, kind="ExternalInput")
    d_dm1 = nc.dram_tensor("dm_w1t", [H, M1], BF16, kind="ExternalInput")
    d_dm2 = nc.dram_tensor("dm_w2t", [M1, M2], BF16, kind="ExternalInput")
    d_dow = nc.dram_tensor("do_wt", [M2, NY], BF16, kind="ExternalInput")
    d_bias = nc.dram_tensor("biases", [1, BIAS_W], BF16, kind="ExternalInput")
    d_xlast = nc.dram_tensor("xlast_t", [NX, BD], BF16, kind="ExternalInput")

    d_out = nc.dram_tensor("out", [BD, hor * NY], F32, kind="ExternalOutput")

    cc_in_x = nc.dram_tensor("cc_in_x", [BE, H], BF16)
    cc_out_x = nc.dram_tensor("cc_out_x", [NCORE, 16, H], BF16)
    cc_in_y = nc.dram_tensor("cc_in_y", [BE, H], BF16)
    cc_out_y = nc.dram_tensor("cc_out_y", [NCORE, 16, H], BF16)

    with tile.TileContext(nc) as tc:
        with tc.tile_pool(name="wts", bufs=1) as wt, \
             tc.tile_pool(name="state", bufs=2) as st, \
             tc.tile_pool(name="xs", bufs=3) as xsp, \
             tc.tile_pool(name="tmp", bufs=2) as tp, \
             tc.tile_pool(name="tmp1", bufs=1) as tq, \
             tc.tile_pool(name="mid", bufs=1) as md, \
             tc.tile_pool(name="persist", bufs=1) as pe, \
             tc.tile_pool(name="psA", bufs=1, space="PSUM") as psA, \
             tc.tile_pool(name="psB", bufs=1, space="PSUM") as psB, \
             tc.tile_pool(name="psC", bufs=1, space="PSUM") as psC, \
             tc.tile_pool(name="psTR", bufs=2, space="PSUM") as psTR:

            # ---------- constants ----------
            idf = pe.tile([128, 128], F32, tag="idf")
            make_identity(nc, idf[:])
            idb = pe.tile([128, 128], BF16, tag="idb")
            nc.gpsimd.tensor_copy(idb[:], idf[:])
            id32f = idf[0:32, 0:32]
            id32b = idb[0:32, 0:32]
            ones_b = pe.tile([1, 128], BF16, tag="ones_b")
            nc.gpsimd.memset(ones_b[:], 1.0)
            zero_b = pe.tile([128, 512], BF16, tag="zero_b")
            nc.gpsimd.memset(zero_b[:], 0.0)
            ones_full = pe.tile([128, 512], BF16, tag="ones_full")
            nc.gpsimd.memset(ones_full[:], 1.0)

            # ---------- encoder weights (needed immediately) ----------
            wih = wt.tile([66, G], BF16, tag="wih")
            nc.sync.dma_start(wih[:], d_wih[:])
            whh = [wt.tile([128, G], BF16, tag=f"whh{c}", name=f"whh{c}")
                   for c in range(4)]
            for c in range(4):
                nc.sync.dma_start(whh[c][:], d_whh[128 * c:128 * (c + 1), :])
            bias = pe.tile([1, BIAS_W], BF16, tag="bias")
            nc.sync.dma_start(bias[:], d_bias[:])

            # ---------- encoder state ----------
            hT = pe.tile([128, H], BF16, tag="hT0")
            nc.vector.tensor_copy(hT[:], zero_b[:])
            h_bh = pe.tile([BE, H], BF16, tag="h0")
            nc.gpsimd.tensor_copy(h_bh[:], zero_b[:])

            # ---------- middle/decoder weights (DMA during encode) ----------
            em1 = [wt.tile([128, M1], BF16, tag=f"em1_{c}", name=f"em1_{c}")
                   for c in range(8)]
            for c in range(8):
                nc.sync.dma_start(em1[c][:], d_em1[128 * c:128 * (c + 1), :])
            em2 = [wt.tile([128, M2], BF16, tag=f"em2_{c}", name=f"em2_{c}")
                   for c in range(8)]
            for c in range(8):
                nc.sync.dma_start(em2[c][:], d_em2[128 * c:128 * (c + 1), :])
            eow = [wt.tile([128, H], BF16, tag=f"eow{c}", name=f"eow{c}")
                   for c in range(4)]
            for c in range(4):
                nc.sync.dma_start(eow[c][:], d_eow[128 * c:128 * (c + 1), :])
            dcw = [wt.tile([128, G], BF16, tag=f"dcw{c}", name=f"dcw{c}")
                   for c in range(8)]
            for c in range(8):
                nc.sync.dma_start(dcw[c][:], d_dcw[128 * c:128 * (c + 1), :])
            dwyc = wt.tile([96, G], BF16, tag="dwyc")
            nc.sync.dma_start(dwyc[0:NY, :], d_dwy[:])
            dwhh = [wt.tile([128, G], BF16, tag=f"dwhh{c}", name=f"dwhh{c}")
                    for c in range(4)]
            for c in range(4):
                nc.sync.dma_start(dwhh[c][:], d_dwhh[128 * c:128 * (c + 1), :])
            dm1 = [wt.tile([128, M1], BF16, tag=f"dm1_{c}", name=f"dm1_{c}")
                   for c in range(4)]
            for c in range(4):
                nc.sync.dma_start(dm1[c][:], d_dm1[128 * c:128 * (c + 1), :])
            dm2 = [wt.tile([128, M2], BF16, tag=f"dm2_{c}", name=f"dm2_{c}")
                   for c in range(8)]
            for c in range(8):
                nc.sync.dma_start(dm2[c][:], d_dm2[128 * c:128 * (c + 1), :])
            dow = [wt.tile([128, NY], BF16, tag=f"dow_{c}", name=f"dow_{c}")
                   for c in range(4)]
            for c in range(4):
                nc.sync.dma_start(dow[c][:], d_dow[128 * c:128 * (c + 1), :])
            xlast = wt.tile([NX, BD], BF16, tag="xlast")
            nc.sync.dma_start(xlast[:], d_xlast[:])
            # identity block staged at partitions 64:96 (for ypT rows)
            id_hi = pe.tile([96, 32], BF16, tag="id_hi")
            nc.sync.dma_start(id_hi[64:96, :], idb[0:32, 0:32])

            # ---------- encode loop ----------
            # Rolling PSUM groups: the xs-side (input-gate) MMs for step t+1
            # are emitted during step t so they fill the PE while the
            # elementwise tail runs.  gA1=r, gA2=z, gB=h-side n, gC=x-side n.
            def open_groups(t):
                xs = xsp.tile([66, 128], BF16, tag="xs")
                nc.sync.dma_start(xs[:], d_xin[:, t * BE:(t + 1) * BE])
                g1a = psA.tile([BE, 512], F32, tag="gA1")
                g1b = psA.tile([BE, 512], F32, tag="gA2")
                g3 = psC.tile([BE, 512], F32, tag="gC")
                nc.tensor.matmul(g1a[:], xs[:], wih[:, 0:512],
                                 start=True, stop=False)
                nc.tensor.matmul(g1b[:], xs[:], wih[:, 512:1024],
                                 start=True, stop=False)
                nc.tensor.matmul(g3[:], xs[:], wih[:, 1024:1536],
                                 start=True, stop=True)
                return g1a, g1b, g3

            groups = open_groups(0)
            for t in range(et):
                g1a, g1b, g3 = groups
                g2 = psB.tile([BE, 512], F32, tag="gB")
                # x-side n-gate was computed last step: stage it to SBUF now
                # (off the critical chain, DVE is idle here)
                g3b = tp.tile([BE, 512], BF16, tag="g3b")
                nc.vector.tensor_copy(g3b[:], g3[:])
                # close the accumulation groups with the h-recurrent MMs
                nc.tensor.matmul(g2[:], ones_b[0:1, 0:BE],
                                 bias[0:1, B_BHN:B_BHN + 512],
                                 start=True, stop=False)
                for c in range(4):
                    nc.tensor.matmul(g1a[:], hT[:, 128 * c:128 * (c + 1)],
                                     whh[c][:, 0:512],
                                     start=False, stop=(c == 3))
                for c in range(4):
                    nc.tensor.matmul(g2[:], hT[:, 128 * c:128 * (c + 1)],
                                     whh[c][:, 1024:1536],
                                     start=False, stop=(c == 3))
                for c in range(4):
                    nc.tensor.matmul(g1b[:], hT[:, 128 * c:128 * (c + 1)],
                                     whh[c][:, 512:1024],
                                     start=False, stop=(c == 3))

                r_t = tp.tile([BE, 512], BF16, tag="r")
                z_t = tp.tile([BE, 512], BF16, tag="z")
                omz = tp.tile([BE, 512], BF16, tag="omz")
                nc.scalar.activation(r_t[:], g1a[:], AF.Sigmoid)
                g2b = tp.tile([BE, 512], BF16, tag="g2b")
                nc.vector.tensor_copy(g2b[:], g2[:])
                rhn = tp.tile([BE, 512], BF16, tag="rhn")
                nc.vector.tensor_mul(rhn[:], r_t[:], g2b[:])
                npre = tp.tile([BE, 512], BF16, tag="npre")
                nc.vector.tensor_add(npre[:], rhn[:], g3b[:])
                n_t = tp.tile([BE, 512], BF16, tag="n")
                nc.scalar.activation(n_t[:], npre[:], AF.Tanh)
                nc.scalar.activation(z_t[:], g1b[:], AF.Sigmoid)
                nc.vector.scalar_tensor_tensor(
                    omz[:], ones_full[:], 1.0, z_t[:],
                    OP.mult, OP.subtract)
                a_t = tp.tile([BE, 512], BF16, tag="a")
                nc.vector.tensor_mul(a_t[:], omz[:], n_t[:])
                b_t = tp.tile([BE, 512], BF16, tag="b")
                nc.gpsimd.tensor_mul(b_t[:], z_t[:], h_bh[:])
                h_new = st.tile([BE, H], BF16, tag="h")
                nc.vector.tensor_add(h_new[:], a_t[:], b_t[:])

                # open next step's groups + PE warmers while the tail runs
                if t + 1 < et:
                    groups = open_groups(t + 1)
                for dk in range(3):
                    dum = psTR.tile([128, 512], F32, tag="dum")
                    nc.tensor.matmul(dum[:], idb[:],
                                     whh[dk][:, 0:512], start=True, stop=True)

                ptr = psTR.tile([128, 512], BF16, tag="tr")
                for c in range(4):
                    nc.tensor.transpose(ptr[:, 128 * c:128 * (c + 1)],
                                        h_new[:, 128 * c:128 * (c + 1)],
                                        idb[:])
                hT_new = st.tile([128, H], BF16, tag="hT")
                nc.vector.tensor_copy(hT_new[:], ptr[:])
                hT, h_bh = hT_new, h_new

                if t == XSYNC:
                    # x-chain hiddens are final after step 50: reshard them
                    # now so the collective overlaps the rest of encode.
                    hx_snap = pe.tile([BE, H], BF16, tag="hx_snap")
                    nc.vector.tensor_copy(hx_snap[:], h_bh[:])
                    nc.sync.dma_start(cc_in_x[:], hx_snap[:])
                    nc.gpsimd.collective_compute(
                        "AllToAll", OP.bypass,
                        replica_groups=[list(range(NCORE))],
                        ins=[cc_in_x[:]], outs=[cc_out_x[:]])

            # ---------- reshard: y-part AllToAll ----------
            nc.sync.dma_start(cc_in_y[:], h_bh[:])
            nc.gpsimd.collective_compute(
                "AllToAll", OP.bypass,
                replica_groups=[list(range(NCORE))],
                ins=[cc_in_y[:]], outs=[cc_out_y[:]])

            pxa = md.tile([BD, H], BF16, tag="pA")
            pxb = md.tile([BD, H], BF16, tag="pB")
            pya = md.tile([BD, H], BF16, tag="pC")
            pyb = md.tile([BD, H], BF16, tag="pD")
            nc.sync.dma_start(pxa[0:16, :], cc_out_x[0][:])
            nc.sync.dma_start(pxa[16:32, :], cc_out_x[1][:])
            nc.sync.dma_start(pxb[0:16, :], cc_out_x[2][:])
            nc.sync.dma_start(pxb[16:32, :], cc_out_x[3][:])
            nc.sync.dma_start(pya[0:16, :], cc_out_y[4][:])
            nc.sync.dma_start(pya[16:32, :], cc_out_y[5][:])
            nc.sync.dma_start(pyb[0:16, :], cc_out_y[6][:])
            nc.sync.dma_start(pyb[16:32, :], cc_out_y[7][:])
            hx = md.tile([BD, H], BF16, tag="hx")
            hy = md.tile([BD, H], BF16, tag="hy")
            nc.vector.tensor_add(hx[:], pxa[:], pxb[:])
            nc.vector.tensor_add(hy[:], pya[:], pyb[:])

            def trsp_b(src, cols, tag):
                """src [BD, cols] bf16 -> bf16 [128, (cols//128)*BD]."""
                nch = cols // 128
                p = psTR.tile([128, nch * BD], BF16, tag="tr")
                for c in range(nch):
                    nc.tensor.transpose(p[:, BD * c:BD * (c + 1)],
                                        src[:, 128 * c:128 * (c + 1)], id32b)
                o = pe.tile([128, nch * BD], BF16, tag=tag)
                nc.scalar.copy(o[:], p[:])
                return o

            hxT = trsp_b(hx, H, "hxT")
            hyT = trsp_b(hy, H, "hyT")

            m1a = psA.tile([BD, 512], F32, tag="gA1")
            m1b = psA.tile([BD, 512], F32, tag="gA2")
            for c in range(8):
                s = (hxT if c < 4 else hyT)[:, BD * (c % 4):BD * (c % 4 + 1)]
                nc.tensor.matmul(m1a[:], s, em1[c][:, 0:512],
                                 start=(c == 0), stop=False)
                nc.tensor.matmul(m1b[:], s, em1[c][:, 512:1024],
                                 start=(c == 0), stop=False)
            nc.tensor.matmul(m1a[:], ones_b[0:1, 0:BD],
                             bias[0:1, B_EM1:B_EM1 + 512],
                             start=False, stop=True)
            nc.tensor.matmul(m1b[:], ones_b[0:1, 0:BD],
                             bias[0:1, B_EM1 + 512:B_EM1 + 1024],
                             start=False, stop=True)
            hm1 = tq.tile([BD, M1], BF16, tag="hm1m")
            nc.scalar.activation(hm1[:, 0:512], m1a[:], AF.Relu)
            nc.scalar.activation(hm1[:, 512:1024], m1b[:], AF.Relu)
            hm1T = trsp_b(hm1, M1, "hm1T_m")

            m2 = psB.tile([BD, M2], F32, tag="gB")
            for c in range(8):
                nc.tensor.matmul(m2[:], hm1T[:, BD * c:BD * (c + 1)],
                                 em2[c][:], start=(c == 0), stop=False)
            nc.tensor.matmul(m2[:], ones_b[0:1, 0:BD],
                             bias[0:1, B_EM2:B_EM2 + 512],
                             start=False, stop=True)
            hm2 = tq.tile([BD, M2], BF16, tag="hm2m")
            nc.scalar.activation(hm2[:], m2[:], AF.Relu)
            hm2T = trsp_b(hm2, M2, "hm2T_m")

            zp = psC.tile([BD, H], F32, tag="gC")
            for c in range(4):
                nc.tensor.matmul(zp[:], hm2T[:, BD * c:BD * (c + 1)],
                                 eow[c][:], start=(c == 0), stop=False)
            nc.tensor.matmul(zp[:], ones_b[0:1, 0:BD],
                             bias[0:1, B_EO:B_EO + 512],
                             start=False, stop=True)
            z_sb = md.tile([BD, H], BF16, tag="z_sb")
            nc.scalar.copy(z_sb[:], zp[:])
            zT = trsp_b(z_sb, H, "zT")

            # const = cat(h_x, z) @ d_Wih[:, :2H].T + d_bih + d_bhh(r,z)
            cpa = psA.tile([96, 512], F32, tag="gA1")
            cpa = cpa[64:96, :]
            cpb = psA.tile([96, 512], F32, tag="gA2")
            cpb = cpb[64:96, :]
            cpn = psB.tile([96, 512], F32, tag="gB")
            cpn = cpn[64:96, :]
            for c in range(8):
                s = (hxT if c < 4 else zT)[:, BD * (c % 4):BD * (c % 4 + 1)]
                nc.tensor.matmul(cpa[:], s, dcw[c][:, 0:512],
                                 start=(c == 0), stop=False)
                nc.tensor.matmul(cpb[:], s, dcw[c][:, 512:1024],
                                 start=(c == 0), stop=False)
                nc.tensor.matmul(cpn[:], s, dcw[c][:, 1024:1536],
                                 start=(c == 0), stop=False)
            nc.tensor.matmul(cpa[:], ones_b[0:1, 0:BD],
                             bias[0:1, B_DC:B_DC + 512], start=False, stop=True)
            nc.tensor.matmul(cpb[:], ones_b[0:1, 0:BD],
                             bias[0:1, B_DC + 512:B_DC + 1024],
                             start=False, stop=True)
            nc.tensor.matmul(cpn[:], ones_b[0:1, 0:BD],
                             bias[0:1, B_DC + 1024:B_DC + 1536],
                             start=False, stop=True)
            nc.vector.tensor_copy(dwyc[64:96, 0:512], cpa[:])
            nc.vector.tensor_copy(dwyc[64:96, 512:1024], cpb[:])
            nc.vector.tensor_copy(dwyc[64:96, 1024:1536], cpn[:])

            # decoder init
            hdT = st.tile([128, 4 * BD], BF16, tag="hdT")
            nc.vector.tensor_copy(hdT[:], zero_b[:, 0:4 * BD])
            hd = st.tile([BD, H], BF16, tag="hd")
            nc.gpsimd.tensor_copy(hd[:], zero_b[0:BD, :])
            ypT = st.tile([96, BD], BF16, tag="ypT")
            nc.vector.tensor_copy(ypT[0:NX, :], xlast[:])
            nc.vector.tensor_copy(ypT[64:96, :], id_hi[64:96, :])

            # ---------- decode loop ----------
            for t in range(hor):
                g1a = psA.tile([BD, 512], F32, tag="gA1")
                g1b = psA.tile([BD, 512], F32, tag="gA2")
                g2 = psB.tile([BD, 512], F32, tag="gB")
                g3 = psC.tile([BD, 512], F32, tag="gC")
                # h-side first: depends only on hdT (ready since last GRU
                # phase), so these stream during the previous step's MLP.
                # The yp/const matmuls close each group once ypT lands.
                nc.tensor.matmul(g2[:], ones_b[0:1, 0:BD],
                                 bias[0:1, B_DBHN:B_DBHN + 512],
                                 start=True, stop=False)
                for c in range(4):
                    nc.tensor.matmul(g2[:], hdT[:, BD * c:BD * (c + 1)],
                                     dwhh[c][:, 1024:1536],
                                     start=False, stop=(c == 3))
                for c in range(4):
                    nc.tensor.matmul(g1a[:], hdT[:, BD * c:BD * (c + 1)],
                                     dwhh[c][:, 0:512],
                                     start=(c == 0), stop=False)
                for c in range(4):
                    nc.tensor.matmul(g1b[:], hdT[:, BD * c:BD * (c + 1)],
                                     dwhh[c][:, 512:1024],
                                     start=(c == 0), stop=False)
                g2b = tp.tile([BD, 512], BF16, tag="g2b")
                nc.vector.tensor_copy(g2b[:], g2[:])
                nc.tensor.matmul(g1a[:], ypT[:], dwyc[:, 0:512],
                                 start=False, stop=True)
                nc.tensor.matmul(g3[:], ypT[:], dwyc[:, 1024:1536],
                                 start=True, stop=True)
                nc.tensor.matmul(g1b[:], ypT[:], dwyc[:, 512:1024],
                                 start=False, stop=True)
                g3b = tp.tile([BD, 512], BF16, tag="g3b")
                nc.vector.tensor_copy(g3b[:], g3[:])
                for dk in range(6):
                    dum = psTR.tile([128, 512], F32, tag="dum")
                    nc.tensor.matmul(dum[:], idb[:],
                                     dwhh[dk % 4][:, 0:512],
                                     start=True, stop=True)

                r_t = tp.tile([BD, 512], BF16, tag="r")
                z_t = tp.tile([BD, 512], BF16, tag="z")
                omz = tp.tile([BD, 512], BF16, tag="omz")
                nc.scalar.activation(r_t[:], g1a[:], AF.Sigmoid)
                rhn = tp.tile([BD, 512], BF16, tag="rhn")
                nc.vector.tensor_mul(rhn[:], r_t[:], g2b[:])
                npre = tp.tile([BD, 512], BF16, tag="npre")
                nc.vector.tensor_add(npre[:], rhn[:], g3b[:])
                n_t = tp.tile([BD, 512], BF16, tag="n")
                nc.scalar.activation(n_t[:], npre[:], AF.Tanh)
                nc.scalar.activation(z_t[:], g1b[:], AF.Sigmoid)
                nc.vector.scalar_tensor_tensor(
                    omz[:], ones_full[0:BD, :], 1.0, z_t[:],
                    OP.mult, OP.subtract)
                a_t = tp.tile([BD, 512], BF16, tag="a")
                nc.vector.tensor_mul(a_t[:], omz[:], n_t[:])
                b_t = tp.tile([BD, 512], BF16, tag="b")
                nc.gpsimd.tensor_mul(b_t[:], z_t[:], hd[:])
                hd_new = st.tile([BD, H], BF16, tag="hd")
                nc.vector.tensor_add(hd_new[:], a_t[:], b_t[:])

                ptr = psTR.tile([128, 4 * BD], BF16, tag="tr")
                for c in range(4):
                    nc.tensor.transpose(ptr[:, BD * c:BD * (c + 1)],
                                        hd_new[:, 128 * c:128 * (c + 1)],
                                        id32b)
                hdT_new = st.tile([128, 4 * BD], BF16, tag="hdT")
                nc.vector.tensor_copy(hdT_new[:], ptr[:])
                hdT, hd = hdT_new, hd_new

                m1a = psA.tile([BD, 512], F32, tag="gA1")
                m1b = psA.tile([BD, 512], F32, tag="gA2")
                for c in range(4):
                    nc.tensor.matmul(m1a[:], hdT[:, BD * c:BD * (c + 1)],
                                     dm1[c][:, 0:512],
                                     start=(c == 0), stop=False)
                nc.tensor.matmul(m1a[:], ones_b[0:1, 0:BD],
                                 bias[0:1, B_DM1:B_DM1 + 512],
                                 start=False, stop=True)
                hm1 = tq.tile([BD, M1], BF16, tag="hm1")
                nc.scalar.activation(hm1[:, 0:512], m1a[:], AF.Relu)
                for c in range(4):
                    nc.tensor.matmul(m1b[:],
                                     hdT[:, BD * c:BD * (c + 1)],
                                     dm1[c][:, 512:1024],
                                     start=(c == 0), stop=False)
                nc.tensor.matmul(m1b[:], ones_b[0:1, 0:BD],
                                 bias[0:1, B_DM1 + 512:B_DM1 + 1024],
                                 start=False, stop=True)
                nc.scalar.activation(hm1[:, 512:1024], m1b[:], AF.Relu)
                hm1Ta = tq.tile([128, 4 * BD], BF16, tag="hm1Ta")
                hm1Tb = tq.tile([128, 4 * BD], BF16, tag="hm1Tb")
                p1 = psTR.tile([128, 4 * BD], BF16, tag="tr")
                for c in range(4):
                    nc.tensor.transpose(p1[:, BD * c:BD * (c + 1)],
                                        hm1[:, 128 * c:128 * (c + 1)], id32b)
                nc.vector.tensor_copy(hm1Ta[:], p1[:])
                p1b = psTR.tile([128, 4 * BD], BF16, tag="tr")
                for c in range(4):
                    nc.tensor.transpose(p1b[:, BD * c:BD * (c + 1)],
                                        hm1[:, 512 + 128 * c:640 + 128 * c],
                                        id32b)
                nc.vector.tensor_copy(hm1Tb[:], p1b[:])

                m2 = psB.tile([BD, M2], F32, tag="gB")
                for c in range(8):
                    s = (hm1Ta if c < 4 else hm1Tb)[:, BD * (c % 4):
                                                    BD * (c % 4 + 1)]
                    nc.tensor.matmul(m2[:], s, dm2[c][:],
                                     start=(c == 0), stop=False)
                nc.tensor.matmul(m2[:], ones_b[0:1, 0:BD],
                                 bias[0:1, B_DM2:B_DM2 + 512],
                                 start=False, stop=True)
                hm2 = tq.tile([BD, M2], BF16, tag="hm2")
                nc.scalar.activation(hm2[:], m2[:], AF.Relu)
                p2 = psTR.tile([128, 4 * BD], BF16, tag="tr")
                for c in range(4):
                    nc.tensor.transpose(p2[:, BD * c:BD * (c + 1)],
                                        hm2[:, 128 * c:128 * (c + 1)], id32b)
                hm2T = tq.tile([128, 4 * BD], BF16, tag="hm2T")
                nc.vector.tensor_copy(hm2T[:], p2[:])

                yp_ps = psC.tile([BD, NY], F32, tag="gC")
                for c in range(4):
                    nc.tensor.matmul(yp_ps[:], hm2T[:, BD * c:BD * (c + 1)],
                                     dow[c][:], start=(c == 0), stop=False)
                nc.tensor.matmul(yp_ps[:], ones_b[0:1, 0:BD],
                                 bias[0:1, B_DO:B_DO + NY],
                                 start=False, stop=True)
                y_sb = tp.tile([BD, NY], F32, tag="y_sb")
                nc.vector.tensor_copy(y_sb[:], yp_ps[:])
                nc.sync.dma_start(d_out[:, NY * t:NY * (t + 1)], y_sb[:])
                if t + 1 < hor:
                    yb = tp.tile([BD, NY], BF16, tag="yb")
                    nc.vector.tensor_copy(yb[:], yp_ps[:])
                    p3 = psTR.tile([NX, BD], BF16, tag="tr")
                    nc.tensor.transpose(p3[:], yb[:], id32b)
                    ypT_new = st.tile([96, BD], BF16, tag="ypT")
                    nc.vector.tensor_copy(ypT_new[0:NX, :], p3[:])
                    nc.vector.tensor_copy(ypT_new[64:96, :],
                                          id_hi[64:96, :])
                    ypT = ypT_new

    nc.compile()
    return nc


# ---------------------------------------------------------------------------
# Host-side sharding
# ---------------------------------------------------------------------------

def shard_inputs(inp, et=100, hor=60):
    f32 = np.float32
    x, y = np.asarray(inp["x"], f32), np.asarray(inp["y"], f32)
    chains = [("xf", False, x), ("xb", True, x),
              ("ef", False, y), ("eb", True, y)]
    in_maps = []
    shared = {}

    def bf(a):
        return np.ascontiguousarray(np.asarray(a, f32)).astype(NPBF)

    def wih_aug(pre):
        wih = np.asarray(inp[pre + "_Wih"], f32)
        bih = np.asarray(inp[pre + "_bih"], f32)
        bhh = np.asarray(inp[pre + "_bhh"], f32)
        aug = np.zeros((66, G), f32)
        aug[0:64, :] = wih.T
        b = bih.copy()
        b[0:2 * H] += bhh[0:2 * H]
        aug[64, :] = b
        aug[65, H:2 * H] = BIG
        return bf(aug)

    d_Wih = np.asarray(inp["d_Wih"], f32)
    d_bih = np.asarray(inp["d_bih"], f32)
    d_bhh = np.asarray(inp["d_bhh"], f32)
    dc_b = d_bih.copy()
    dc_b[0:2 * H] += d_bhh[0:2 * H]

    shared["em_w1t"] = bf(np.asarray(inp["em_W1"], f32).T)
    shared["em_w2t"] = bf(np.asarray(inp["em_W2"], f32).T)
    shared["eo_wt"] = bf(np.asarray(inp["eo_W"], f32).T)
    shared["dc_wt"] = bf(d_Wih[:, 0:2 * H].T)
    shared["dwy_t"] = bf(d_Wih[:, 2 * H:].T)
    shared["dwhh_t"] = bf(np.asarray(inp["d_Whh"], f32).T)
    shared["dm_w1t"] = bf(np.asarray(inp["dm_W1"], f32).T)
    shared["dm_w2t"] = bf(np.asarray(inp["dm_W2"], f32).T)
    shared["do_wt"] = bf(np.asarray(inp["do_W"], f32).T)

    def bias_pack(pre):
        bz = np.zeros((1, BIAS_W), f32)
        bz[0, B_DC:B_DC + G] = dc_b
        bz[0, B_EM1:B_EM1 + M1] = np.asarray(inp["em_b1"], f32)
        bz[0, B_EM2:B_EM2 + M2] = np.asarray(inp["em_b2"], f32)
        bz[0, B_EO:B_EO + H] = np.asarray(inp["eo_b"], f32)
        bz[0, B_BHN:B_BHN + H] = np.asarray(inp[pre + "_bhh"], f32)[2 * H:]
        bz[0, B_DBHN:B_DBHN + H] = d_bhh[2 * H:]
        bz[0, B_DM1:B_DM1 + M1] = np.asarray(inp["dm_b1"], f32)
        bz[0, B_DM2:B_DM2 + M2] = np.asarray(inp["dm_b2"], f32)
        bz[0, B_DO:B_DO + NY] = np.asarray(inp["do_b"], f32)
        return bf(bz)

    for j in range(NCORE):
        chain, half = j // 2, j % 2
        pre, rev, seq = chains[chain]
        T = seq.shape[1]
        s = seq[128 * half:128 * (half + 1)]          # [128, T, 64]
        xin = np.zeros((66, et, BE), f32)
        xin[64, :, :] = 1.0
        if T < et:
            xin[65, T:, :] = 1.0                      # end padding: hold h
        order = np.arange(T)[::-1] if rev else np.arange(T)
        xin[0:64, :T, :] = s[:, order, :].transpose(2, 1, 0)
        m = dict(shared)
        m["xin"] = bf(xin.reshape(66, et * BE))
        m["wih_aug"] = wih_aug(pre)
        m["whh_t"] = bf(np.asarray(inp[pre + "_Whh"], f32).T)
        m["biases"] = bias_pack(pre)
        xl = np.concatenate([x[16 * j:16 * j + 16, -1, :],
                             x[128 + 16 * j:128 + 16 * j + 16, -1, :]])
        m["xlast_t"] = bf(xl.T)
        in_maps.append(m)
    return in_maps


def unshard(results, hor=60):
    out = np.zeros((B, hor, NY), np.float32)
    for j in range(NCORE):
        o = results[j]["out"].reshape(BD, hor, NY)
        out[16 * j:16 * j + 16] = o[0:16]
        out[128 + 16 * j:128 + 16 * j + 16] = o[16:32]
    return out


_NC = None


def kernel(**inputs):
    global _NC
    from concourse.bass_utils import run_bass_kernel_spmd
    if _NC is None:
        _NC = build_nc()
    in_maps = shard_inputs(inputs)
    res = run_bass_kernel_spmd(_NC, in_maps, core_ids=list(range(NCORE)))
    return unshard(res.results)


# revision 7
# speedup vs baseline: 1.2890x; 1.0071x over previous
"""Trainium2 Bass kernel for the GRU autoencoder (bf16 edition).

Distribution strategy (8 NeuronCores):
  Encode : chain-parallel x batch-parallel. Core j handles GRU chain j//2
           (xf, xb, ef, eb) on batch half j%2 (128 rows), running a uniform
           100-step loop. The 50-step x-chains run steps 0..49 then 50 exact
           identity steps (update-gate pre-activation forced to +BIG => z=1
           => h'=h) so their hidden is final at step 50 and the x-reshard
           collective overlaps encode steps 51..99.
  Reshard: two AllToAlls of 16-row slices (x-parts early, y-parts at end);
           every core assembles hidden states for its own 32-row decode
           shard at fixed SPMD-uniform indices:
           core j decodes global rows [16j:16j+16] u [128+16j:+16].
  Middle : per-shard MLP (em1/em2/eo) + decoder const precompute.
  Decode : 60 autoregressive steps per core on its 32-row shard.

All matmuls use bf16 operands (fp32 PSUM accumulation): bf16 MMs measure
~2x faster than fp32r on this part (193ns vs 397ns warm at N=512) and the
per-step xs-side MMs are emitted one step ahead so the PE has fill work
during each step's elementwise tail.

PSUM budget (8 banks): gA 2 | gB 2 | gC 2 | tr 2.
"""

import sys

sys.path.insert(0, "/opt/trn_rl_repo")

import numpy as np
import ml_dtypes

import concourse.bass as bass
import concourse.mybir as mybir
import concourse.tile as tile
from concourse import bacc
from concourse.masks import make_identity

dt = mybir.dt
AF = mybir.ActivationFunctionType
OP = mybir.AluOpType

B, TX, TY, NX, NY, H, HOR = 256, 50, 100, 64, 64, 512, 60
M1, M2 = 1024, 512
G = 3 * H
NCORE = 8
BE = 128   # encoder batch rows per core
BD = 32    # decoder batch rows per core
BIG = 30000.0
XSYNC = 52  # encode step at which the x-part reshard fires

F32, BF16 = dt.float32, dt.bfloat16
NPBF = ml_dtypes.bfloat16

# packed bias row layout (single [1, 6208] bf16 tensor)
B_DC, B_EM1, B_EM2, B_EO = 0, G, G + M1, G + M1 + M2
B_BHN, B_DBHN = 3584, 3584 + H
B_DM1, B_DM2, B_DO = 4608, 5632, 6144
BIAS_W = 6208


def build_nc(et=100, hor=60):
    nc = bacc.Bacc("TRN2", target_bir_lowering=False, debug=False,
                   num_devices=NCORE)

    # ---- DRAM parameters (identical names on every core; content differs) --
    d_xin = nc.dram_tensor("xin", [66, et * BE], BF16, kind="ExternalInput")
    d_wih = nc.dram_tensor("wih_aug", [66, G], BF16, kind="ExternalInput")
    d_whh = nc.dram_tensor("whh_t", [H, G], BF16, kind="ExternalInput")

    d_em1 = nc.dram_tensor("em_w1t", [2 * H, M1], BF16, kind="ExternalInput")
    d_em2 = nc.dram_tensor("em_w2t", [M1, M2], BF16, kind="ExternalInput")
    d_eow = nc.dram_tensor("eo_wt", [M2, H], BF16, kind="ExternalInput")
    d_dcw = nc.dram_tensor("dc_wt", [2 * H, G], BF16, kind="ExternalInput")

    d_dwy = nc.dram_tensor("dwy_t", [NY, G], BF16, kind="ExternalInput")
    d_dwhh = nc.dram_tensor("dwhh_t", [H, G], BF16, kind="ExternalInput")
    d_dm1 = nc.dram_tensor("dm_w1t", [H, M1], BF16, kind="ExternalInput")
    d_dm2 = nc.dram_tensor("dm_w2t", [M1, M2], BF16, kind="ExternalInput")
    d_dow = nc.dram_tensor("do_wt", [M2, NY], BF16, kind="ExternalInput")
    d_bias = nc.dram_tensor("biases", [1, BIAS_W], BF16, kind="ExternalInput")
    d_xlast = nc.dram_tensor("xlast_t", [NX, BD], BF16, kind="ExternalInput")

    d_out = nc.dram_tensor("out", [BD, hor * NY], F32, kind="ExternalOutput")

    cc_in_x = nc.dram_tensor("cc_in_x", [BE, H], BF16)
    cc_out_x = nc.dram_tensor("cc_out_x", [NCORE, 16, H], BF16)
    cc_in_y = nc.dram_tensor("cc_in_y", [BE, H], BF16)
    cc_out_y = nc.dram_tensor("cc_out_y", [NCORE, 16, H], BF16)

    with tile.TileContext(nc) as tc:
        with tc.tile_pool(name="wts", bufs=1) as wt, \
             tc.tile_pool(name="state", bufs=2) as st, \
             tc.tile_pool(name="xs", bufs=3) as xsp, \
             tc.tile_pool(name="tmp", bufs=2) as tp, \
             tc.tile_pool(name="tmp1", bufs=1) as tq, \
             tc.tile_pool(name="mid", bufs=1) as md, \
             tc.tile_pool(name="persist", bufs=1) as pe, \
             tc.tile_pool(name="psA", bufs=1, space="PSUM") as psA, \
             tc.tile_pool(name="psB", bufs=1, space="PSUM") as psB, \
             tc.tile_pool(name="psC", bufs=1, space="PSUM") as psC, \
             tc.tile_pool(name="psTR", bufs=2, space="PSUM") as psTR:

            # ---------- constants ----------
            idf = pe.tile([128, 128], F32, tag="idf")
            make_identity(nc, idf[:])
            idb = pe.tile([128, 128], BF16, tag="idb")
            nc.gpsimd.tensor_copy(idb[:], idf[:])
            id32f = idf[0:32, 0:32]
            id32b = idb[0:32, 0:32]
            ones_b = pe.tile([1, 128], BF16, tag="ones_b")
            nc.gpsimd.memset(ones_b[:], 1.0)
            zero_b = pe.tile([128, 512], BF16, tag="zero_b")
            nc.gpsimd.memset(zero_b[:], 0.0)
            ones_full = pe.tile([128, 512], BF16, tag="ones_full")
            nc.gpsimd.memset(ones_full[:], 1.0)

            # ---------- encoder weights (needed immediately) ----------
            wih = wt.tile([66, G], BF16, tag="wih")
            nc.sync.dma_start(wih[:], d_wih[:])
            whh = [wt.tile([128, G], BF16, tag=f"whh{c}", name=f"whh{c}")
                   for c in range(4)]
            for c in range(4):
                nc.sync.dma_start(whh[c][:], d_whh[128 * c:128 * (c + 1), :])
            bias = pe.tile([1, BIAS_W], BF16, tag="bias")
            nc.sync.dma_start(bias[:], d_bias[:])

            # ---------- encoder state ----------
            hT = pe.tile([128, H], BF16, tag="hT0")
            nc.vector.tensor_copy(hT[:], zero_b[:])
            h_bh = pe.tile([BE, H], BF16, tag="h0")
            nc.gpsimd.tensor_copy(h_bh[:], zero_b[:])

            # ---------- middle/decoder weights (DMA during encode) ----------
            em1 = [wt.tile([128, M1], BF16, tag=f"em1_{c}", name=f"em1_{c}")
                   for c in range(8)]
            for c in range(8):
                nc.sync.dma_start(em1[c][:], d_em1[128 * c:128 * (c + 1), :])
            em2 = [wt.tile([128, M2], BF16, tag=f"em2_{c}", name=f"em2_{c}")
                   for c in range(8)]
            for c in range(8):
                nc.sync.dma_start(em2[c][:], d_em2[128 * c:128 * (c + 1), :])
            eow = [wt.tile([128, H], BF16, tag=f"eow{c}", name=f"eow{c}")
                   for c in range(4)]
            for c in range(4):
                nc.sync.dma_start(eow[c][:], d_eow[128 * c:128 * (c + 1), :])
            dcw = [wt.tile([128, G], BF16, tag=f"dcw{c}", name=f"dcw{c}")
                   for c in range(8)]
            for c in range(8):
                nc.sync.dma_start(dcw[c][:], d_dcw[128 * c:128 * (c + 1), :])
            dwyc = wt.tile([96, G], BF16, tag="dwyc")
            nc.sync.dma_start(dwyc[0:NY, :], d_dwy[:])
            dwhh = [wt.tile([128, G], BF16, tag=f"dwhh{c}", name=f"dwhh{c}")
                    for c in range(4)]
            for c in range(4):
                nc.sync.dma_start(dwhh[c][:], d_dwhh[128 * c:128 * (c + 1), :])
            dm1 = [wt.tile([128, M1], BF16, tag=f"dm1_{c}", name=f"dm1_{c}")
                   for c in range(4)]
            for c in range(4):
                nc.sync.dma_start(dm1[c][:], d_dm1[128 * c:128 * (c + 1), :])
            dm2 = [wt.tile([128, M2], BF16, tag=f"dm2_{c}", name=f"dm2_{c}")
                   for c in range(8)]
            for c in range(8):
                nc.sync.dma_start(dm2[c][:], d_dm2[128 * c:128 * (c + 1), :])
            dow = [wt.tile([128, NY], BF16, tag=f"dow_{c}", name=f"dow_{c}")
                   for c in range(4)]
            for c in range(4):
                nc.sync.dma_start(dow[c][:], d_dow[128 * c:128 * (c + 1), :])
            xlast = wt.tile([NX, BD], BF16, tag="xlast")
            nc.sync.dma_start(xlast[:], d_xlast[:])
            # identity block staged at partitions 64:96 (for ypT rows)
            id_hi = pe.tile([96, 32], BF16, tag="id_hi")
            nc.sync.dma_start(id_hi[64:96, :], idb[0:32, 0:32])

            # ---------- encode loop ----------
            # Rolling PSUM groups: the xs-side (input-gate) MMs for step t+1
            # are emitted during step t so they fill the PE while the
            # elementwise tail runs.  gA1=r, gA2=z, gB=h-side n, gC=x-side n.
            def open_groups(t):
                xs = xsp.tile([66, 128], BF16, tag="xs")
                nc.sync.dma_start(xs[:], d_xin[:, t * BE:(t + 1) * BE])
                g1a = psA.tile([BE, 512], F32, tag="gA1")
                g1b = psA.tile([BE, 512], F32, tag="gA2")
                g3 = psC.tile([BE, 512], F32, tag="gC")
                nc.tensor.matmul(g1a[:], xs[:], wih[:, 0:512],
                                 start=True, stop=False)
                nc.tensor.matmul(g1b[:], xs[:], wih[:, 512:1024],
                                 start=True, stop=False)
                nc.tensor.matmul(g3[:], xs[:], wih[:, 1024:1536],
                                 start=True, stop=True)
                return g1a, g1b, g3

            groups = open_groups(0)
            for t in range(et):
                g1a, g1b, g3 = groups
                g2 = psB.tile([BE, 512], F32, tag="gB")
                # x-side n-gate was computed last step: stage it to SBUF now
                # (off the critical chain, DVE is idle here)
                g3b = tp.tile([BE, 512], BF16, tag="g3b")
                nc.vector.tensor_copy(g3b[:], g3[:])
                # close the accumulation groups with the h-recurrent MMs
                nc.tensor.matmul(g2[:], ones_b[0:1, 0:BE],
                                 bias[0:1, B_BHN:B_BHN + 512],
                                 start=True, stop=False)
                for c in range(4):
                    nc.tensor.matmul(g1a[:], hT[:, 128 * c:128 * (c + 1)],
                                     whh[c][:, 0:512],
                                     start=False, stop=(c == 3))
                for c in range(4):
                    nc.tensor.matmul(g2[:], hT[:, 128 * c:128 * (c + 1)],
                                     whh[c][:, 1024:1536],
                                     start=False, stop=(c == 3))
                for c in range(4):
                    nc.tensor.matmul(g1b[:], hT[:, 128 * c:128 * (c + 1)],
                                     whh[c][:, 512:1024],
                                     start=False, stop=(c == 3))

                r_t = tp.tile([BE, 512], BF16, tag="r")
                z_t = tp.tile([BE, 512], BF16, tag="z")
                omz = tp.tile([BE, 512], BF16, tag="omz")
                nc.scalar.activation(r_t[:], g1a[:], AF.Sigmoid)
                nc.scalar.activation(z_t[:], g1b[:], AF.Sigmoid)
                g2b = tp.tile([BE, 512], BF16, tag="g2b")
                nc.vector.tensor_copy(g2b[:], g2[:])
                rhn = tp.tile([BE, 512], BF16, tag="rhn")
                nc.vector.tensor_mul(rhn[:], r_t[:], g2b[:])
                npre = tp.tile([BE, 512], BF16, tag="npre")
                nc.vector.tensor_add(npre[:], rhn[:], g3b[:])
                n_t = tp.tile([BE, 512], BF16, tag="n")
                nc.scalar.activation(n_t[:], npre[:], AF.Tanh)
                nc.vector.scalar_tensor_tensor(
                    omz[:], ones_full[:], 1.0, z_t[:],
                    OP.mult, OP.subtract)
                a_t = tp.tile([BE, 512], BF16, tag="a")
                nc.vector.tensor_mul(a_t[:], omz[:], n_t[:])
                b_t = tp.tile([BE, 512], BF16, tag="b")
                nc.gpsimd.tensor_mul(b_t[:], z_t[:], h_bh[:])
                h_new = st.tile([BE, H], BF16, tag="h")
                nc.vector.tensor_add(h_new[:], a_t[:], b_t[:])

                # PE warmers bridge the tail stall, then next step's x-side
                for dk in range(4):
                    dum = psTR.tile([128, 512], F32, tag="dum")
                    nc.tensor.matmul(dum[:], idb[:],
                                     whh[dk][:, 0:512], start=True, stop=True)
                if t + 1 < et:
                    groups = open_groups(t + 1)

                ptr = psTR.tile([128, 512], BF16, tag="tr")
                for c in range(4):
                    nc.tensor.transpose(ptr[:, 128 * c:128 * (c + 1)],
                                        h_new[:, 128 * c:128 * (c + 1)],
                                        idb[:])
                hT_new = st.tile([128, H], BF16, tag="hT")
                nc.vector.tensor_copy(hT_new[:], ptr[:])
                hT, h_bh = hT_new, h_new

                if t == XSYNC:
                    # x-chain hiddens are final after step 50: reshard them
                    # now so the collective overlaps the rest of encode.
                    hx_snap = pe.tile([BE, H], BF16, tag="hx_snap")
                    nc.vector.tensor_copy(hx_snap[:], h_bh[:])
                    nc.sync.dma_start(cc_in_x[:], hx_snap[:])
                    nc.gpsimd.collective_compute(
                        "AllToAll", OP.bypass,
                        replica_groups=[list(range(NCORE))],
                        ins=[cc_in_x[:]], outs=[cc_out_x[:]])

            # ---------- reshard: y-part AllToAll ----------
            nc.sync.dma_start(cc_in_y[:], h_bh[:])
            nc.gpsimd.collective_compute(
                "AllToAll", OP.bypass,
                replica_groups=[list(range(NCORE))],
                ins=[cc_in_y[:]], outs=[cc_out_y[:]])

            pxa = md.tile([BD, H], BF16, tag="pA")
            pxb = md.tile([BD, H], BF16, tag="pB")
            pya = md.tile([BD, H], BF16, tag="pC")
            pyb = md.tile([BD, H], BF16, tag="pD")
            nc.sync.dma_start(pxa[0:16, :], cc_out_x[0][:])
            nc.sync.dma_start(pxa[16:32, :], cc_out_x[1][:])
            nc.sync.dma_start(pxb[0:16, :], cc_out_x[2][:])
            nc.sync.dma_start(pxb[16:32, :], cc_out_x[3][:])
            nc.sync.dma_start(pya[0:16, :], cc_out_y[4][:])
            nc.sync.dma_start(pya[16:32, :], cc_out_y[5][:])
            nc.sync.dma_start(pyb[0:16, :], cc_out_y[6][:])
            nc.sync.dma_start(pyb[16:32, :], cc_out_y[7][:])
            hx = md.tile([BD, H], BF16, tag="hx")
            hy = md.tile([BD, H], BF16, tag="hy")
            nc.vector.tensor_add(hx[:], pxa[:], pxb[:])
            nc.vector.tensor_add(hy[:], pya[:], pyb[:])

            def trsp_b(src, cols, tag):
                """src [BD, cols] bf16 -> bf16 [128, (cols//128)*BD]."""
                nch = cols // 128
                p = psTR.tile([128, nch * BD], BF16, tag="tr")
                for c in range(nch):
                    nc.tensor.transpose(p[:, BD * c:BD * (c + 1)],
                                        src[:, 128 * c:128 * (c + 1)], id32b)
                o = pe.tile([128, nch * BD], BF16, tag=tag)
                nc.scalar.copy(o[:], p[:])
                return o

            hxT = trsp_b(hx, H, "hxT")
            hyT = trsp_b(hy, H, "hyT")

            m1a = psA.tile([BD, 512], F32, tag="gA1")
            m1b = psA.tile([BD, 512], F32, tag="gA2")
            for c in range(8):
                s = (hxT if c < 4 else hyT)[:, BD * (c % 4):BD * (c % 4 + 1)]
                nc.tensor.matmul(m1a[:], s, em1[c][:, 0:512],
                                 start=(c == 0), stop=False)
                nc.tensor.matmul(m1b[:], s, em1[c][:, 512:1024],
                                 start=(c == 0), stop=False)
            nc.tensor.matmul(m1a[:], ones_b[0:1, 0:BD],
                             bias[0:1, B_EM1:B_EM1 + 512],
                             start=False, stop=True)
            nc.tensor.matmul(m1b[:], ones_b[0:1, 0:BD],
                             bias[0:1, B_EM1 + 512:B_EM1 + 1024],
                             start=False, stop=True)
            hm1 = tq.tile([BD, M1], BF16, tag="hm1m")
            nc.scalar.activation(hm1[:, 0:512], m1a[:], AF.Relu)
            nc.scalar.activation(hm1[:, 512:1024], m1b[:], AF.Relu)
            hm1T = trsp_b(hm1, M1, "hm1T_m")

            m2 = psB.tile([BD, M2], F32, tag="gB")
            for c in range(8):
                nc.tensor.matmul(m2[:], hm1T[:, BD * c:BD * (c + 1)],
                                 em2[c][:], start=(c == 0), stop=False)
            nc.tensor.matmul(m2[:], ones_b[0:1, 0:BD],
                             bias[0:1, B_EM2:B_EM2 + 512],
                             start=False, stop=True)
            hm2 = tq.tile([BD, M2], BF16, tag="hm2m")
            nc.scalar.activation(hm2[:], m2[:], AF.Relu)
            hm2T = trsp_b(hm2, M2, "hm2T_m")

            zp = psC.tile([BD, H], F32, tag="gC")
            for c in range(4):
                nc.tensor.matmul(zp[:], hm2T[:, BD * c:BD * (c + 1)],
                                 eow[c][:], start=(c == 0), stop=False)
            nc.tensor.matmul(zp[:], ones_b[0:1, 0:BD],
                             bias[0:1, B_EO:B_EO + 512],
                             start=False, stop=True)
            z_sb = md.tile([BD, H], BF16, tag="z_sb")
            nc.scalar.copy(z_sb[:], zp[:])
            zT = trsp_b(z_sb, H, "zT")

            # const = cat(h_x, z) @ d_Wih[:, :2H].T + d_bih + d_bhh(r,z)
            cpa = psA.tile([96, 512], F32, tag="gA1")
            cpa = cpa[64:96, :]
            cpb = psA.tile([96, 512], F32, tag="gA2")
            cpb = cpb[64:96, :]
            cpn = psB.tile([96, 512], F32, tag="gB")
            cpn = cpn[64:96, :]
            for c in range(8):
                s = (hxT if c < 4 else zT)[:, BD * (c % 4):BD * (c % 4 + 1)]
                nc.tensor.matmul(cpa[:], s, dcw[c][:, 0:512],
                                 start=(c == 0), stop=False)
                nc.tensor.matmul(cpb[:], s, dcw[c][:, 512:1024],
                                 start=(c == 0), stop=False)
                nc.tensor.matmul(cpn[:], s, dcw[c][:, 1024:1536],
                                 start=(c == 0), stop=False)
            nc.tensor.matmul(cpa[:], ones_b[0:1, 0:BD],
                             bias[0:1, B_DC:B_DC + 512], start=False, stop=True)
            nc.tensor.matmul(cpb[:], ones_b[0:1, 0:BD],
                             bias[0:1, B_DC + 512:B_DC + 1024],
                             start=False, stop=True)
            nc.tensor.matmul(cpn[:], ones_b[0:1, 0:BD],
                             bias[0:1, B_DC + 1024:B_DC + 1536],
                             start=False, stop=True)
            nc.vector.tensor_copy(dwyc[64:96, 0:512], cpa[:])
            nc.vector.tensor_copy(dwyc[64:96, 512:1024], cpb[:])
            nc.vector.tensor_copy(dwyc[64:96, 1024:1536], cpn[:])

            # decoder init
            hdT = st.tile([128, 4 * BD], BF16, tag="hdT")
            nc.vector.tensor_copy(hdT[:], zero_b[:, 0:4 * BD])
            hd = st.tile([BD, H], BF16, tag="hd")
            nc.gpsimd.tensor_copy(hd[:], zero_b[0:BD, :])
            ypT0 = pe.tile([96, BD], BF16, tag="ypT0")
            ypT1 = pe.tile([96, BD], BF16, tag="ypT1")
            ypTs = [ypT0, ypT1]
            nc.vector.tensor_copy(ypT0[0:NX, :], xlast[:])
            nc.vector.tensor_copy(ypT0[64:96, :], id_hi[64:96, :])
            nc.vector.tensor_copy(ypT1[64:96, :], id_hi[64:96, :])
            ypT = ypT0

            # ---------- decode loop ----------
            for t in range(hor):
                g1a = psA.tile([BD, 512], F32, tag="gA1")
                g1b = psA.tile([BD, 512], F32, tag="gA2")
                g2 = psB.tile([BD, 512], F32, tag="gB")
                g3 = psC.tile([BD, 512], F32, tag="gC")
                # h-side first: depends only on hdT (ready since last GRU
                # phase), so these stream during the previous step's MLP.
                # The yp/const matmuls close each group once ypT lands.
                nc.tensor.matmul(g2[:], ones_b[0:1, 0:BD],
                                 bias[0:1, B_DBHN:B_DBHN + 512],
                                 start=True, stop=False)
                for c in range(4):
                    nc.tensor.matmul(g2[:], hdT[:, BD * c:BD * (c + 1)],
                                     dwhh[c][:, 1024:1536],
                                     start=False, stop=(c == 3))
                for c in range(4):
                    nc.tensor.matmul(g1a[:], hdT[:, BD * c:BD * (c + 1)],
                                     dwhh[c][:, 0:512],
                                     start=(c == 0), stop=False)
                for c in range(4):
                    nc.tensor.matmul(g1b[:], hdT[:, BD * c:BD * (c + 1)],
                                     dwhh[c][:, 512:1024],
                                     start=(c == 0), stop=False)
                g2b = tp.tile([BD, 512], BF16, tag="g2b")
                nc.vector.tensor_copy(g2b[:], g2[:])
                nc.tensor.matmul(g1a[:], ypT[:], dwyc[:, 0:512],
                                 start=False, stop=True)
                nc.tensor.matmul(g3[:], ypT[:], dwyc[:, 1024:1536],
                                 start=True, stop=True)
                nc.tensor.matmul(g1b[:], ypT[:], dwyc[:, 512:1024],
                                 start=False, stop=True)
                g3b = tp.tile([BD, 512], BF16, tag="g3b")
                nc.vector.tensor_copy(g3b[:], g3[:])
                for dk in range(6):
                    dum = psTR.tile([128, 512], F32, tag="dum")
                    nc.tensor.matmul(dum[:], idb[:],
                                     dwhh[dk % 4][:, 0:512],
                                     start=True, stop=True)

                r_t = tp.tile([BD, 512], BF16, tag="r")
                z_t = tp.tile([BD, 512], BF16, tag="z")
                omz = tp.tile([BD, 512], BF16, tag="omz")
                nc.scalar.activation(r_t[:], g1a[:], AF.Sigmoid)
                nc.scalar.activation(z_t[:], g1b[:], AF.Sigmoid)
                rhn = tp.tile([BD, 512], BF16, tag="rhn")
                nc.vector.tensor_mul(rhn[:], r_t[:], g2b[:])
                npre = tp.tile([BD, 512], BF16, tag="npre")
                nc.vector.tensor_add(npre[:], rhn[:], g3b[:])
                n_t = tp.tile([BD, 512], BF16, tag="n")
                nc.scalar.activation(n_t[:], npre[:], AF.Tanh)
                nc.vector.scalar_tensor_tensor(
                    omz[:], ones_full[0:BD, :], 1.0, z_t[:],
                    OP.mult, OP.subtract)
                a_t = tp.tile([BD, 512], BF16, tag="a")
                nc.vector.tensor_mul(a_t[:], omz[:], n_t[:])
                b_t = tp.tile([BD, 512], BF16, tag="b")
                nc.gpsimd.tensor_mul(b_t[:], z_t[:], hd[:])
                hd_new = st.tile([BD, H], BF16, tag="hd")
                nc.vector.tensor_add(hd_new[:], a_t[:], b_t[:])

                ptr = psTR.tile([128, 4 * BD], BF16, tag="tr")
                for c in range(4):
                    nc.tensor.transpose(ptr[:, BD * c:BD * (c + 1)],
                                        hd_new[:, 128 * c:128 * (c + 1)],
                                        id32b)
                hdT_new = st.tile([128, 4 * BD], BF16, tag="hdT")
                nc.vector.tensor_copy(hdT_new[:], ptr[:])
                hdT, hd = hdT_new, hd_new

                m1a = psA.tile([BD, 512], F32, tag="gA1")
                m1b = psA.tile([BD, 512], F32, tag="gA2")
                for c in range(4):
                    nc.tensor.matmul(m1a[:], hdT[:, BD * c:BD * (c + 1)],
                                     dm1[c][:, 0:512],
                                     start=(c == 0), stop=False)
                nc.tensor.matmul(m1a[:], ones_b[0:1, 0:BD],
                                 bias[0:1, B_DM1:B_DM1 + 512],
                                 start=False, stop=True)
                hm1 = tq.tile([BD, M1], BF16, tag="hm1")
                nc.scalar.activation(hm1[:, 0:512], m1a[:], AF.Relu)
                for c in range(4):
                    nc.tensor.matmul(m1b[:],
                                     hdT[:, BD * c:BD * (c + 1)],
                                     dm1[c][:, 512:1024],
                                     start=(c == 0), stop=False)
                nc.tensor.matmul(m1b[:], ones_b[0:1, 0:BD],
                                 bias[0:1, B_DM1 + 512:B_DM1 + 1024],
                                 start=False, stop=True)
                nc.scalar.activation(hm1[:, 512:1024], m1b[:], AF.Relu)
                hm1Ta = tq.tile([128, 4 * BD], BF16, tag="hm1Ta")
                hm1Tb = tq.tile([128, 4 * BD], BF16, tag="hm1Tb")
                p1 = psTR.tile([128, 4 * BD], BF16, tag="tr")
                for c in range(4):
                    nc.tensor.transpose(p1[:, BD * c:BD * (c + 1)],
                                        hm1[:, 128 * c:128 * (c + 1)], id32b)
                nc.vector.tensor_copy(hm1Ta[:], p1[:])
                p1b = psTR.tile([128, 4 * BD], BF16, tag="tr")
                for c in range(4):
                    nc.tensor.transpose(p1b[:, BD * c:BD * (c + 1)],
                                        hm1[:, 512 + 128 * c:640 + 128 * c],
                                        id32b)
                nc.vector.tensor_copy(hm1Tb[:], p1b[:])

                m2 = psB.tile([BD, M2], F32, tag="gB")
                for c in range(8):
                    s = (hm1Ta if c < 4 else hm1Tb)[:, BD * (c % 4):
                                                    BD * (c % 4 + 1)]
                    nc.tensor.matmul(m2[:], s, dm2[c][:],
                                     start=(c == 0), stop=False)
                nc.tensor.matmul(m2[:], ones_b[0:1, 0:BD],
                                 bias[0:1, B_DM2:B_DM2 + 512],
                                 start=False, stop=True)
                hm2 = tq.tile([BD, M2], BF16, tag="hm2")
                nc.scalar.activation(hm2[:], m2[:], AF.Relu)
                p2 = psTR.tile([128, 4 * BD], BF16, tag="tr")
                for c in range(4):
                    nc.tensor.transpose(p2[:, BD * c:BD * (c + 1)],
                                        hm2[:, 128 * c:128 * (c + 1)], id32b)
                hm2T = tq.tile([128, 4 * BD], BF16, tag="hm2T")
                nc.vector.tensor_copy(hm2T[:], p2[:])

                yp_ps = psC.tile([BD, NY], F32, tag="gC")
                for c in range(4):
                    nc.tensor.matmul(yp_ps[:], hm2T[:, BD * c:BD * (c + 1)],
                                     dow[c][:], start=(c == 0), stop=False)
                nc.tensor.matmul(yp_ps[:], ones_b[0:1, 0:BD],
                                 bias[0:1, B_DO:B_DO + NY],
                                 start=False, stop=True)
                y_sb = tp.tile([BD, NY], F32, tag="y_sb")
                nc.vector.tensor_copy(y_sb[:], yp_ps[:])
                nc.sync.dma_start(d_out[:, NY * t:NY * (t + 1)], y_sb[:])
                if t + 1 < hor:
                    yT_ps = psC.tile([NY, BD], F32, tag="gC")
                    for c in range(4):
                        nc.tensor.matmul(yT_ps[:], dow[c][:],
                                         hm2T[:, BD * c:BD * (c + 1)],
                                         start=(c == 0), stop=False)
                    nc.tensor.matmul(yT_ps[:], bias[0:1, B_DO:B_DO + NY],
                                     ones_b[0:1, 0:BD],
                                     start=False, stop=True)
                    ypT_new = ypTs[(t + 1) % 2]
                    nc.vector.tensor_copy(ypT_new[0:NX, :], yT_ps[:])
                    ypT = ypT_new

    nc.compile()
    return nc


# ---------------------------------------------------------------------------
# Host-side sharding
# ---------------------------------------------------------------------------

def shard_inputs(inp, et=100, hor=60):
    f32 = np.float32
    x, y = np.asarray(inp["x"], f32), np.asarray(inp["y"], f32)
    chains = [("xf", False, x), ("xb", True, x),
              ("ef", False, y), ("eb", True, y)]
    in_maps = []
    shared = {}

    def bf(a):
        return np.ascontiguousarray(np.asarray(a, f32)).astype(NPBF)

    def wih_aug(pre):
        wih = np.asarray(inp[pre + "_Wih"], f32)
        bih = np.asarray(inp[pre + "_bih"], f32)
        bhh = np.asarray(inp[pre + "_bhh"], f32)
        aug = np.zeros((66, G), f32)
        aug[0:64, :] = wih.T
        b = bih.copy()
        b[0:2 * H] += bhh[0:2 * H]
        aug[64, :] = b
        aug[65, H:2 * H] = BIG
        return bf(aug)

    d_Wih = np.asarray(inp["d_Wih"], f32)
    d_bih = np.asarray(inp["d_bih"], f32)
    d_bhh = np.asarray(inp["d_bhh"], f32)
    dc_b = d_bih.copy()
    dc_b[0:2 * H] += d_bhh[0:2 * H]

    shared["em_w1t"] = bf(np.asarray(inp["em_W1"], f32).T)
    shared["em_w2t"] = bf(np.asarray(inp["em_W2"], f32).T)
    shared["eo_wt"] = bf(np.asarray(inp["eo_W"], f32).T)
    shared["dc_wt"] = bf(d_Wih[:, 0:2 * H].T)
    shared["dwy_t"] = bf(d_Wih[:, 2 * H:].T)
    shared["dwhh_t"] = bf(np.asarray(inp["d_Whh"], f32).T)
    shared["dm_w1t"] = bf(np.asarray(inp["dm_W1"], f32).T)
    shared["dm_w2t"] = bf(np.asarray(inp["dm_W2"], f32).T)
    shared["do_wt"] = bf(np.asarray(inp["do_W"], f32).T)

    def bias_pack(pre):
        bz = np.zeros((1, BIAS_W), f32)
        bz[0, B_DC:B_DC + G] = dc_b
        bz[0, B_EM1:B_EM1 + M1] = np.asarray(inp["em_b1"], f32)
        bz[0, B_EM2:B_EM2 + M2] = np.asarray(inp["em_b2"], f32)
        bz[0, B_EO:B_EO + H] = np.asarray(inp["eo_b"], f32)
        bz[0, B_BHN:B_BHN + H] = np.asarray(inp[pre + "_bhh"], f32)[2 * H:]
        bz[0, B_DBHN:B_DBHN + H] = d_bhh[2 * H:]
        bz[0, B_DM1:B_DM1 + M1] = np.asarray(inp["dm_b1"], f32)
        bz[0, B_DM2:B_DM2 + M2] = np.asarray(inp["dm_b2"], f32)
        bz[0, B_DO:B_DO + NY] = np.asarray(inp["do_b"], f32)
        return bf(bz)

    for j in range(NCORE):
        chain, half = j // 2, j % 2
        pre, rev, seq = chains[chain]
        T = seq.shape[1]
        s = seq[128 * half:128 * (half + 1)]          # [128, T, 64]
        xin = np.zeros((66, et, BE), f32)
        xin[64, :, :] = 1.0
        if T < et:
            xin[65, T:, :] = 1.0                      # end padding: hold h
        order = np.arange(T)[::-1] if rev else np.arange(T)
        xin[0:64, :T, :] = s[:, order, :].transpose(2, 1, 0)
        m = dict(shared)
        m["xin"] = bf(xin.reshape(66, et * BE))
        m["wih_aug"] = wih_aug(pre)
        m["whh_t"] = bf(np.asarray(inp[pre + "_Whh"], f32).T)
        m["biases"] = bias_pack(pre)
        xl = np.concatenate([x[16 * j:16 * j + 16, -1, :],
                             x[128 + 16 * j:128 + 16 * j + 16, -1, :]])
        m["xlast_t"] = bf(xl.T)
        in_maps.append(m)
    return in_maps


def unshard(results, hor=60):
    out = np.zeros((B, hor, NY), np.float32)
    for j in range(NCORE):
        o = results[j]["out"].reshape(BD, hor, NY)
        out[16 * j:16 * j + 16] = o[0:16]
        out[128 + 16 * j:128 + 16 * j + 16] = o[16:32]
    return out


_NC = None


def kernel(**inputs):
    global _NC
    from concourse.bass_utils import run_bass_kernel_spmd
    if _NC is None:
        _NC = build_nc()
    in_maps = shard_inputs(inputs)
    res = run_bass_kernel_spmd(_NC, in_maps, core_ids=list(range(NCORE)))
    return unshard(res.results)


# revision 8
# speedup vs baseline: 1.3380x; 1.0380x over previous
"""Trainium2 Bass kernel for the GRU autoencoder (bf16 edition).

Distribution strategy (8 NeuronCores):
  Encode : chain-parallel x batch-parallel. Core j handles GRU chain j//2
           (xf, xb, ef, eb) on batch half j%2 (128 rows), running a uniform
           100-step loop. The 50-step x-chains run steps 0..49 then 50 exact
           identity steps (update-gate pre-activation forced to +BIG => z=1
           => h'=h) so their hidden is final at step 50 and the x-reshard
           collective overlaps encode steps 51..99.
  Reshard: two AllToAlls of 16-row slices (x-parts early, y-parts at end);
           every core assembles hidden states for its own 32-row decode
           shard at fixed SPMD-uniform indices:
           core j decodes global rows [16j:16j+16] u [128+16j:+16].
  Middle : per-shard MLP (em1/em2/eo) + decoder const precompute.
  Decode : 60 autoregressive steps per core on its 32-row shard.

All matmuls use bf16 operands (fp32 PSUM accumulation): bf16 MMs measure
~2x faster than fp32r on this part (193ns vs 397ns warm at N=512) and the
per-step xs-side MMs are emitted one step ahead so the PE has fill work
during each step's elementwise tail.

PSUM budget (8 banks): gA 2 | gB 2 | gC 2 | tr 2.
"""

import sys

sys.path.insert(0, "/opt/trn_rl_repo")

import numpy as np
import ml_dtypes

import concourse.bass as bass
import concourse.mybir as mybir
import concourse.tile as tile
from concourse import bacc
from concourse.masks import make_identity

dt = mybir.dt
AF = mybir.ActivationFunctionType
OP = mybir.AluOpType

B, TX, TY, NX, NY, H, HOR = 256, 50, 100, 64, 64, 512, 60
M1, M2 = 1024, 512
G = 3 * H
NCORE = 8
BE = 128   # encoder batch rows per core
BD = 32    # decoder batch rows per core
BIG = 30000.0
XSYNC = 52  # encode step at which the x-part reshard fires

F32, BF16 = dt.float32, dt.bfloat16
NPBF = ml_dtypes.bfloat16

# packed bias row layout (single [1, 6208] bf16 tensor)
B_DC, B_EM1, B_EM2, B_EO = 0, G, G + M1, G + M1 + M2
B_BHN, B_DBHN = 3584, 3584 + H
B_DM1, B_DM2, B_DO = 4608, 5632, 6144
BIAS_W = 6208


def build_nc(et=100, hor=60):
    nc = bacc.Bacc("TRN2", target_bir_lowering=False, debug=False,
                   num_devices=NCORE)

    # ---- DRAM parameters (identical names on every core; content differs) --
    d_xin = nc.dram_tensor("xin", [66, et * BE], BF16, kind="ExternalInput")
    d_wih = nc.dram_tensor("wih_aug", [66, G], BF16, kind="ExternalInput")
    d_whh = nc.dram_tensor("whh_t", [H, G], BF16, kind="ExternalInput")

    d_em1 = nc.dram_tensor("em_w1t", [2 * H, M1], BF16, kind="ExternalInput")
    d_em2 = nc.dram_tensor("em_w2t", [M1, M2], BF16, kind="ExternalInput")
    d_eow = nc.dram_tensor("eo_wt", [M2, H], BF16, kind="ExternalInput")
    d_dcw = nc.dram_tensor("dc_wt", [2 * H, G], BF16, kind="ExternalInput")

    d_dwy = nc.dram_tensor("dwy_t", [NY, G], BF16, kind="ExternalInput")
    d_dwhh = nc.dram_tensor("dwhh_t", [H, G], BF16, kind="ExternalInput")
    d_dm1 = nc.dram_tensor("dm_w1t", [H, M1], BF16, kind="ExternalInput")
    d_dm2 = nc.dram_tensor("dm_w2t", [M1, M2], BF16, kind="ExternalInput")
    d_dow = nc.dram_tensor("do_wt", [M2, NY], BF16, kind="ExternalInput")
    d_bias = nc.dram_tensor("biases", [1, BIAS_W], BF16, kind="ExternalInput")
    d_xlast = nc.dram_tensor("xlast_t", [NX, BD], BF16, kind="ExternalInput")

    d_out = nc.dram_tensor("out", [BD, hor * NY], F32, kind="ExternalOutput")

    cc_in_x = nc.dram_tensor("cc_in_x", [BE, H], BF16)
    cc_out_x = nc.dram_tensor("cc_out_x", [NCORE, 16, H], BF16)
    cc_in_y = nc.dram_tensor("cc_in_y", [BE, H], BF16)
    cc_out_y = nc.dram_tensor("cc_out_y", [NCORE, 16, H], BF16)

    with tile.TileContext(nc) as tc:
        with tc.tile_pool(name="wts", bufs=1) as wt, \
             tc.tile_pool(name="state", bufs=2) as st, \
             tc.tile_pool(name="xs", bufs=3) as xsp, \
             tc.tile_pool(name="tmp", bufs=2) as tp, \
             tc.tile_pool(name="tmp1", bufs=1) as tq, \
             tc.tile_pool(name="mid", bufs=1) as md, \
             tc.tile_pool(name="persist", bufs=1) as pe, \
             tc.tile_pool(name="psA", bufs=1, space="PSUM") as psA, \
             tc.tile_pool(name="psB", bufs=1, space="PSUM") as psB, \
             tc.tile_pool(name="psC", bufs=1, space="PSUM") as psC, \
             tc.tile_pool(name="psTR", bufs=2, space="PSUM") as psTR:

            # ---------- constants ----------
            idf = pe.tile([128, 128], F32, tag="idf")
            make_identity(nc, idf[:])
            idb = pe.tile([128, 128], BF16, tag="idb")
            nc.gpsimd.tensor_copy(idb[:], idf[:])
            id32f = idf[0:32, 0:32]
            id32b = idb[0:32, 0:32]
            ones_b = pe.tile([1, 128], BF16, tag="ones_b")
            nc.gpsimd.memset(ones_b[:], 1.0)
            zero_b = pe.tile([128, 512], BF16, tag="zero_b")
            nc.gpsimd.memset(zero_b[:], 0.0)
            ones_full = pe.tile([128, 512], BF16, tag="ones_full")
            nc.gpsimd.memset(ones_full[:], 1.0)

            # ---------- encoder weights (needed immediately) ----------
            wih = wt.tile([66, G], BF16, tag="wih")
            nc.sync.dma_start(wih[:], d_wih[:])
            whh = [wt.tile([128, G], BF16, tag=f"whh{c}", name=f"whh{c}")
                   for c in range(4)]
            for c in range(4):
                nc.sync.dma_start(whh[c][:], d_whh[128 * c:128 * (c + 1), :])
            bias = pe.tile([1, BIAS_W], BF16, tag="bias")
            nc.sync.dma_start(bias[:], d_bias[:])

            # ---------- encoder state ----------
            hT = pe.tile([128, H], BF16, tag="hT0")
            nc.vector.tensor_copy(hT[:], zero_b[:])
            h_bh = pe.tile([BE, H], BF16, tag="h0")
            nc.gpsimd.tensor_copy(h_bh[:], zero_b[:])

            # ---------- middle/decoder weights (DMA during encode) ----------
            em1 = [wt.tile([128, M1], BF16, tag=f"em1_{c}", name=f"em1_{c}")
                   for c in range(8)]
            for c in range(8):
                nc.sync.dma_start(em1[c][:], d_em1[128 * c:128 * (c + 1), :])
            em2 = [wt.tile([128, M2], BF16, tag=f"em2_{c}", name=f"em2_{c}")
                   for c in range(8)]
            for c in range(8):
                nc.sync.dma_start(em2[c][:], d_em2[128 * c:128 * (c + 1), :])
            eow = [wt.tile([128, H], BF16, tag=f"eow{c}", name=f"eow{c}")
                   for c in range(4)]
            for c in range(4):
                nc.sync.dma_start(eow[c][:], d_eow[128 * c:128 * (c + 1), :])
            dcw = [wt.tile([128, G], BF16, tag=f"dcw{c}", name=f"dcw{c}")
                   for c in range(8)]
            for c in range(8):
                nc.sync.dma_start(dcw[c][:], d_dcw[128 * c:128 * (c + 1), :])
            dwyc = wt.tile([96, G], BF16, tag="dwyc")
            nc.sync.dma_start(dwyc[0:NY, :], d_dwy[:])
            dwhh = [wt.tile([128, G], BF16, tag=f"dwhh{c}", name=f"dwhh{c}")
                    for c in range(4)]
            for c in range(4):
                nc.sync.dma_start(dwhh[c][:], d_dwhh[128 * c:128 * (c + 1), :])
            dm1 = [wt.tile([128, M1], BF16, tag=f"dm1_{c}", name=f"dm1_{c}")
                   for c in range(4)]
            for c in range(4):
                nc.sync.dma_start(dm1[c][:], d_dm1[128 * c:128 * (c + 1), :])
            dm2 = [wt.tile([128, M2], BF16, tag=f"dm2_{c}", name=f"dm2_{c}")
                   for c in range(8)]
            for c in range(8):
                nc.sync.dma_start(dm2[c][:], d_dm2[128 * c:128 * (c + 1), :])
            dow = [wt.tile([128, NY], BF16, tag=f"dow_{c}", name=f"dow_{c}")
                   for c in range(4)]
            for c in range(4):
                nc.sync.dma_start(dow[c][:], d_dow[128 * c:128 * (c + 1), :])
            xlast = wt.tile([NX, BD], BF16, tag="xlast")
            nc.sync.dma_start(xlast[:], d_xlast[:])
            # identity block staged at partitions 64:96 (for ypT rows)
            id_hi = pe.tile([96, 32], BF16, tag="id_hi")
            nc.sync.dma_start(id_hi[64:96, :], idb[0:32, 0:32])

            # ---------- encode loop ----------
            # Rolling PSUM groups: the xs-side (input-gate) MMs for step t+1
            # are emitted during step t so they fill the PE while the
            # elementwise tail runs.  gA1=r, gA2=z, gB=h-side n, gC=x-side n.
            def open_groups(t):
                xs = xsp.tile([66, 128], BF16, tag="xs")
                nc.sync.dma_start(xs[:], d_xin[:, t * BE:(t + 1) * BE])
                g1a = psA.tile([BE, 512], F32, tag="gA1")
                g1b = psA.tile([BE, 512], F32, tag="gA2")
                g3 = psC.tile([BE, 512], F32, tag="gC")
                nc.tensor.matmul(g1a[:], xs[:], wih[:, 0:512],
                                 start=True, stop=False)
                nc.tensor.matmul(g1b[:], xs[:], wih[:, 512:1024],
                                 start=True, stop=False)
                nc.tensor.matmul(g3[:], xs[:], wih[:, 1024:1536],
                                 start=True, stop=True)
                return g1a, g1b, g3

            groups = open_groups(0)
            for t in range(et):
                g1a, g1b, g3 = groups
                g2 = psB.tile([BE, 512], F32, tag="gB")
                # x-side n-gate was computed last step: stage it to SBUF now
                # (off the critical chain, DVE is idle here)
                g3b = tp.tile([BE, 512], BF16, tag="g3b")
                nc.vector.tensor_copy(g3b[:], g3[:])
                # close the accumulation groups with the h-recurrent MMs
                nc.tensor.matmul(g2[:], ones_b[0:1, 0:BE],
                                 bias[0:1, B_BHN:B_BHN + 512],
                                 start=True, stop=False)
                for c in range(4):
                    nc.tensor.matmul(g1a[:], hT[:, 128 * c:128 * (c + 1)],
                                     whh[c][:, 0:512],
                                     start=False, stop=(c == 3))
                for c in range(4):
                    nc.tensor.matmul(g2[:], hT[:, 128 * c:128 * (c + 1)],
                                     whh[c][:, 1024:1536],
                                     start=False, stop=(c == 3))
                for c in range(4):
                    nc.tensor.matmul(g1b[:], hT[:, 128 * c:128 * (c + 1)],
                                     whh[c][:, 512:1024],
                                     start=False, stop=(c == 3))

                r_t = tp.tile([BE, 512], BF16, tag="r")
                z_t = tp.tile([BE, 512], BF16, tag="z")
                nc.scalar.activation(r_t[:], g1a[:], AF.Sigmoid)
                nc.scalar.activation(z_t[:], g1b[:], AF.Sigmoid)
                g2b = tp.tile([BE, 512], BF16, tag="g2b")
                nc.vector.tensor_copy(g2b[:], g2[:])
                rhn = tp.tile([BE, 512], BF16, tag="rhn")
                nc.vector.tensor_mul(rhn[:], r_t[:], g2b[:])
                npre = tp.tile([BE, 512], BF16, tag="npre")
                nc.vector.tensor_add(npre[:], rhn[:], g3b[:])
                n_t = tp.tile([BE, 512], BF16, tag="n")
                nc.scalar.activation(n_t[:], npre[:], AF.Tanh)
                d_t = tp.tile([BE, 512], BF16, tag="d")
                nc.vector.tensor_sub(d_t[:], h_bh[:], n_t[:])
                e_t = tp.tile([BE, 512], BF16, tag="e")
                nc.vector.tensor_mul(e_t[:], z_t[:], d_t[:])
                h_new = st.tile([BE, H], BF16, tag="h")
                nc.vector.tensor_add(h_new[:], n_t[:], e_t[:])

                # PE warmers bridge the tail stall, then next step's x-side
                for dk in range(5):
                    dum = psTR.tile([128, 512], F32, tag="dum")
                    nc.tensor.matmul(dum[:], idb[:],
                                     whh[dk % 4][:, 0:512],
                                     start=True, stop=True)
                if t + 1 < et:
                    groups = open_groups(t + 1)

                ptr = psTR.tile([128, 512], BF16, tag="tr")
                for c in range(4):
                    nc.tensor.transpose(ptr[:, 128 * c:128 * (c + 1)],
                                        h_new[:, 128 * c:128 * (c + 1)],
                                        idb[:])
                hT_new = st.tile([128, H], BF16, tag="hT")
                nc.vector.tensor_copy(hT_new[:], ptr[:])
                hT, h_bh = hT_new, h_new

                if t == XSYNC:
                    # x-chain hiddens are final after step 50: reshard them
                    # now so the collective overlaps the rest of encode.
                    hx_snap = pe.tile([BE, H], BF16, tag="hx_snap")
                    nc.vector.tensor_copy(hx_snap[:], h_bh[:])
                    nc.sync.dma_start(cc_in_x[:], hx_snap[:])
                    nc.gpsimd.collective_compute(
                        "AllToAll", OP.bypass,
                        replica_groups=[list(range(NCORE))],
                        ins=[cc_in_x[:]], outs=[cc_out_x[:]])

            # ---------- reshard: y-part AllToAll ----------
            nc.sync.dma_start(cc_in_y[:], h_bh[:])
            nc.gpsimd.collective_compute(
                "AllToAll", OP.bypass,
                replica_groups=[list(range(NCORE))],
                ins=[cc_in_y[:]], outs=[cc_out_y[:]])

            pxa = md.tile([BD, H], BF16, tag="pA")
            pxb = md.tile([BD, H], BF16, tag="pB")
            pya = md.tile([BD, H], BF16, tag="pC")
            pyb = md.tile([BD, H], BF16, tag="pD")
            nc.sync.dma_start(pxa[0:16, :], cc_out_x[0][:])
            nc.sync.dma_start(pxa[16:32, :], cc_out_x[1][:])
            nc.sync.dma_start(pxb[0:16, :], cc_out_x[2][:])
            nc.sync.dma_start(pxb[16:32, :], cc_out_x[3][:])
            nc.sync.dma_start(pya[0:16, :], cc_out_y[4][:])
            nc.sync.dma_start(pya[16:32, :], cc_out_y[5][:])
            nc.sync.dma_start(pyb[0:16, :], cc_out_y[6][:])
            nc.sync.dma_start(pyb[16:32, :], cc_out_y[7][:])
            hx = md.tile([BD, H], BF16, tag="hx")
            hy = md.tile([BD, H], BF16, tag="hy")
            nc.vector.tensor_add(hx[:], pxa[:], pxb[:])
            nc.vector.tensor_add(hy[:], pya[:], pyb[:])

            def trsp_b(src, cols, tag):
                """src [BD, cols] bf16 -> bf16 [128, (cols//128)*BD]."""
                nch = cols // 128
                p = psTR.tile([128, nch * BD], BF16, tag="tr")
                for c in range(nch):
                    nc.tensor.transpose(p[:, BD * c:BD * (c + 1)],
                                        src[:, 128 * c:128 * (c + 1)], id32b)
                o = pe.tile([128, nch * BD], BF16, tag=tag)
                nc.scalar.copy(o[:], p[:])
                return o

            hxT = trsp_b(hx, H, "hxT")
            hyT = trsp_b(hy, H, "hyT")

            m1a = psA.tile([BD, 512], F32, tag="gA1")
            m1b = psA.tile([BD, 512], F32, tag="gA2")
            for c in range(8):
                s = (hxT if c < 4 else hyT)[:, BD * (c % 4):BD * (c % 4 + 1)]
                nc.tensor.matmul(m1a[:], s, em1[c][:, 0:512],
                                 start=(c == 0), stop=False)
                nc.tensor.matmul(m1b[:], s, em1[c][:, 512:1024],
                                 start=(c == 0), stop=False)
            nc.tensor.matmul(m1a[:], ones_b[0:1, 0:BD],
                             bias[0:1, B_EM1:B_EM1 + 512],
                             start=False, stop=True)
            nc.tensor.matmul(m1b[:], ones_b[0:1, 0:BD],
                             bias[0:1, B_EM1 + 512:B_EM1 + 1024],
                             start=False, stop=True)
            hm1 = tq.tile([BD, M1], BF16, tag="hm1m")
            nc.scalar.activation(hm1[:, 0:512], m1a[:], AF.Relu)
            nc.scalar.activation(hm1[:, 512:1024], m1b[:], AF.Relu)
            hm1T = trsp_b(hm1, M1, "hm1T_m")

            m2 = psB.tile([BD, M2], F32, tag="gB")
            for c in range(8):
                nc.tensor.matmul(m2[:], hm1T[:, BD * c:BD * (c + 1)],
                                 em2[c][:], start=(c == 0), stop=False)
            nc.tensor.matmul(m2[:], ones_b[0:1, 0:BD],
                             bias[0:1, B_EM2:B_EM2 + 512],
                             start=False, stop=True)
            hm2 = tq.tile([BD, M2], BF16, tag="hm2m")
            nc.scalar.activation(hm2[:], m2[:], AF.Relu)
            hm2T = trsp_b(hm2, M2, "hm2T_m")

            zp = psC.tile([BD, H], F32, tag="gC")
            for c in range(4):
                nc.tensor.matmul(zp[:], hm2T[:, BD * c:BD * (c + 1)],
                                 eow[c][:], start=(c == 0), stop=False)
            nc.tensor.matmul(zp[:], ones_b[0:1, 0:BD],
                             bias[0:1, B_EO:B_EO + 512],
                             start=False, stop=True)
            z_sb = md.tile([BD, H], BF16, tag="z_sb")
            nc.scalar.copy(z_sb[:], zp[:])
            zT = trsp_b(z_sb, H, "zT")

            # const = cat(h_x, z) @ d_Wih[:, :2H].T + d_bih + d_bhh(r,z)
            cpa = psA.tile([96, 512], F32, tag="gA1")
            cpa = cpa[64:96, :]
            cpb = psA.tile([96, 512], F32, tag="gA2")
            cpb = cpb[64:96, :]
            cpn = psB.tile([96, 512], F32, tag="gB")
            cpn = cpn[64:96, :]
            for c in range(8):
                s = (hxT if c < 4 else zT)[:, BD * (c % 4):BD * (c % 4 + 1)]
                nc.tensor.matmul(cpa[:], s, dcw[c][:, 0:512],
                                 start=(c == 0), stop=False)
                nc.tensor.matmul(cpb[:], s, dcw[c][:, 512:1024],
                                 start=(c == 0), stop=False)
                nc.tensor.matmul(cpn[:], s, dcw[c][:, 1024:1536],
                                 start=(c == 0), stop=False)
            nc.tensor.matmul(cpa[:], ones_b[0:1, 0:BD],
                             bias[0:1, B_DC:B_DC + 512], start=False, stop=True)
            nc.tensor.matmul(cpb[:], ones_b[0:1, 0:BD],
                             bias[0:1, B_DC + 512:B_DC + 1024],
                             start=False, stop=True)
            nc.tensor.matmul(cpn[:], ones_b[0:1, 0:BD],
                             bias[0:1, B_DC + 1024:B_DC + 1536],
                             start=False, stop=True)
            nc.vector.tensor_copy(dwyc[64:96, 0:512], cpa[:])
            nc.vector.tensor_copy(dwyc[64:96, 512:1024], cpb[:])
            nc.vector.tensor_copy(dwyc[64:96, 1024:1536], cpn[:])

            # decoder init
            hdT = st.tile([128, 4 * BD], BF16, tag="hdT")
            nc.vector.tensor_copy(hdT[:], zero_b[:, 0:4 * BD])
            hd = st.tile([BD, H], BF16, tag="hd")
            nc.gpsimd.tensor_copy(hd[:], zero_b[0:BD, :])
            ypT0 = pe.tile([96, BD], BF16, tag="ypT0")
            ypT1 = pe.tile([96, BD], BF16, tag="ypT1")
            ypTs = [ypT0, ypT1]
            nc.vector.tensor_copy(ypT0[0:NX, :], xlast[:])
            nc.vector.tensor_copy(ypT0[64:96, :], id_hi[64:96, :])
            nc.vector.tensor_copy(ypT1[64:96, :], id_hi[64:96, :])
            ypT = ypT0

            # ---------- decode loop ----------
            for t in range(hor):
                g1a = psA.tile([BD, 512], F32, tag="gA1")
                g1b = psA.tile([BD, 512], F32, tag="gA2")
                g2 = psB.tile([BD, 512], F32, tag="gB")
                g3 = psC.tile([BD, 512], F32, tag="gC")
                # h-side first: depends only on hdT (ready since last GRU
                # phase), so these stream during the previous step's MLP.
                # The yp/const matmuls close each group once ypT lands.
                nc.tensor.matmul(g2[:], ones_b[0:1, 0:BD],
                                 bias[0:1, B_DBHN:B_DBHN + 512],
                                 start=True, stop=False)
                for c in range(4):
                    nc.tensor.matmul(g2[:], hdT[:, BD * c:BD * (c + 1)],
                                     dwhh[c][:, 1024:1536],
                                     start=False, stop=(c == 3))
                for c in range(4):
                    nc.tensor.matmul(g1a[:], hdT[:, BD * c:BD * (c + 1)],
                                     dwhh[c][:, 0:512],
                                     start=(c == 0), stop=False)
                for c in range(4):
                    nc.tensor.matmul(g1b[:], hdT[:, BD * c:BD * (c + 1)],
                                     dwhh[c][:, 512:1024],
                                     start=(c == 0), stop=False)
                g2b = tp.tile([BD, 512], BF16, tag="g2b")
                nc.vector.tensor_copy(g2b[:], g2[:])
                nc.tensor.matmul(g1a[:], ypT[:], dwyc[:, 0:512],
                                 start=False, stop=True)
                nc.tensor.matmul(g3[:], ypT[:], dwyc[:, 1024:1536],
                                 start=True, stop=True)
                nc.tensor.matmul(g1b[:], ypT[:], dwyc[:, 512:1024],
                                 start=False, stop=True)
                g3b = tp.tile([BD, 512], BF16, tag="g3b")
                nc.vector.tensor_copy(g3b[:], g3[:])
                for dk in range(6):
                    dum = psTR.tile([128, 512], F32, tag="dum")
                    nc.tensor.matmul(dum[:], idb[:],
                                     dwhh[dk % 4][:, 0:512],
                                     start=True, stop=True)

                r_t = tp.tile([BD, 512], BF16, tag="r")
                z_t = tp.tile([BD, 512], BF16, tag="z")
                nc.scalar.activation(r_t[:], g1a[:], AF.Sigmoid)
                nc.scalar.activation(z_t[:], g1b[:], AF.Sigmoid)
                rhn = tp.tile([BD, 512], BF16, tag="rhn")
                nc.vector.tensor_mul(rhn[:], r_t[:], g2b[:])
                npre = tp.tile([BD, 512], BF16, tag="npre")
                nc.vector.tensor_add(npre[:], rhn[:], g3b[:])
                n_t = tp.tile([BD, 512], BF16, tag="n")
                nc.scalar.activation(n_t[:], npre[:], AF.Tanh)
                d_t = tp.tile([BD, 512], BF16, tag="d")
                nc.vector.tensor_sub(d_t[:], hd[:], n_t[:])
                e_t = tp.tile([BD, 512], BF16, tag="e")
                nc.vector.tensor_mul(e_t[:], z_t[:], d_t[:])
                hd_new = st.tile([BD, H], BF16, tag="hd")
                nc.vector.tensor_add(hd_new[:], n_t[:], e_t[:])

                ptr = psTR.tile([128, 4 * BD], BF16, tag="tr")
                for c in range(4):
                    nc.tensor.transpose(ptr[:, BD * c:BD * (c + 1)],
                                        hd_new[:, 128 * c:128 * (c + 1)],
                                        id32b)
                hdT_new = st.tile([128, 4 * BD], BF16, tag="hdT")
                nc.vector.tensor_copy(hdT_new[:], ptr[:])
                hdT, hd = hdT_new, hd_new

                m1a = psA.tile([BD, 512], F32, tag="gA1")
                m1b = psA.tile([BD, 512], F32, tag="gA2")
                for c in range(4):
                    nc.tensor.matmul(m1a[:], hdT[:, BD * c:BD * (c + 1)],
                                     dm1[c][:, 0:512],
                                     start=(c == 0), stop=False)
                nc.tensor.matmul(m1a[:], ones_b[0:1, 0:BD],
                                 bias[0:1, B_DM1:B_DM1 + 512],
                                 start=False, stop=True)
                hm1 = tq.tile([BD, M1], BF16, tag="hm1")
                nc.scalar.activation(hm1[:, 0:512], m1a[:], AF.Relu)
                for c in range(4):
                    nc.tensor.matmul(m1b[:],
                                     hdT[:, BD * c:BD * (c + 1)],
                                     dm1[c][:, 512:1024],
                                     start=(c == 0), stop=False)
                nc.tensor.matmul(m1b[:], ones_b[0:1, 0:BD],
                                 bias[0:1, B_DM1 + 512:B_DM1 + 1024],
                                 start=False, stop=True)
                nc.scalar.activation(hm1[:, 512:1024], m1b[:], AF.Relu)
                hm1T = tq.tile([128, 8 * BD], BF16, tag="hm1T")
                p1 = psTR.tile([128, 8 * BD], BF16, tag="tr")
                for c in range(8):
                    nc.tensor.transpose(p1[:, BD * c:BD * (c + 1)],
                                        hm1[:, 128 * c:128 * (c + 1)], id32b)
                nc.vector.tensor_copy(hm1T[:], p1[:])

                m2 = psB.tile([BD, M2], F32, tag="gB")
                for c in range(8):
                    nc.tensor.matmul(m2[:], hm1T[:, BD * c:BD * (c + 1)],
                                     dm2[c][:],
                                     start=(c == 0), stop=False)
                nc.tensor.matmul(m2[:], ones_b[0:1, 0:BD],
                                 bias[0:1, B_DM2:B_DM2 + 512],
                                 start=False, stop=True)
                hm2 = tq.tile([BD, M2], BF16, tag="hm2")
                nc.scalar.activation(hm2[:], m2[:], AF.Relu)
                p2 = psTR.tile([128, 4 * BD], BF16, tag="tr")
                for c in range(4):
                    nc.tensor.transpose(p2[:, BD * c:BD * (c + 1)],
                                        hm2[:, 128 * c:128 * (c + 1)], id32b)
                hm2T = tq.tile([128, 4 * BD], BF16, tag="hm2T")
                nc.vector.tensor_copy(hm2T[:], p2[:])

                yp_ps = psC.tile([BD, NY], F32, tag="gC")
                for c in range(4):
                    nc.tensor.matmul(yp_ps[:], hm2T[:, BD * c:BD * (c + 1)],
                                     dow[c][:], start=(c == 0), stop=False)
                nc.tensor.matmul(yp_ps[:], ones_b[0:1, 0:BD],
                                 bias[0:1, B_DO:B_DO + NY],
                                 start=False, stop=True)
                y_sb = tp.tile([BD, NY], F32, tag="y_sb")
                nc.vector.tensor_copy(y_sb[:], yp_ps[:])
                nc.sync.dma_start(d_out[:, NY * t:NY * (t + 1)], y_sb[:])
                if t + 1 < hor:
                    yT_ps = psC.tile([NY, BD], F32, tag="gC")
                    for c in range(4):
                        nc.tensor.matmul(yT_ps[:], dow[c][:],
                                         hm2T[:, BD * c:BD * (c + 1)],
                                         start=(c == 0), stop=False)
                    nc.tensor.matmul(yT_ps[:], bias[0:1, B_DO:B_DO + NY],
                                     ones_b[0:1, 0:BD],
                                     start=False, stop=True)
                    ypT_new = ypTs[(t + 1) % 2]
                    nc.vector.tensor_copy(ypT_new[0:NX, :], yT_ps[:])
                    ypT = ypT_new

    nc.compile()
    return nc


# ---------------------------------------------------------------------------
# Host-side sharding
# ---------------------------------------------------------------------------

def shard_inputs(inp, et=100, hor=60):
    f32 = np.float32
    x, y = np.asarray(inp["x"], f32), np.asarray(inp["y"], f32)
    chains = [("xf", False, x), ("xb", True, x),
              ("ef", False, y), ("eb", True, y)]
    in_maps = []
    shared = {}

    def bf(a):
        return np.ascontiguousarray(np.asarray(a, f32)).astype(NPBF)

    def wih_aug(pre):
        wih = np.asarray(inp[pre + "_Wih"], f32)
        bih = np.asarray(inp[pre + "_bih"], f32)
        bhh = np.asarray(inp[pre + "_bhh"], f32)
        aug = np.zeros((66, G), f32)
        aug[0:64, :] = wih.T
        b = bih.copy()
        b[0:2 * H] += bhh[0:2 * H]
        aug[64, :] = b
        aug[65, H:2 * H] = BIG
        return bf(aug)

    d_Wih = np.asarray(inp["d_Wih"], f32)
    d_bih = np.asarray(inp["d_bih"], f32)
    d_bhh = np.asarray(inp["d_bhh"], f32)
    dc_b = d_bih.copy()
    dc_b[0:2 * H] += d_bhh[0:2 * H]

    shared["em_w1t"] = bf(np.asarray(inp["em_W1"], f32).T)
    shared["em_w2t"] = bf(np.asarray(inp["em_W2"], f32).T)
    shared["eo_wt"] = bf(np.asarray(inp["eo_W"], f32).T)
    shared["dc_wt"] = bf(d_Wih[:, 0:2 * H].T)
    shared["dwy_t"] = bf(d_Wih[:, 2 * H:].T)
    shared["dwhh_t"] = bf(np.asarray(inp["d_Whh"], f32).T)
    shared["dm_w1t"] = bf(np.asarray(inp["dm_W1"], f32).T)
    shared["dm_w2t"] = bf(np.asarray(inp["dm_W2"], f32).T)
    shared["do_wt"] = bf(np.asarray(inp["do_W"], f32).T)

    def bias_pack(pre):
        bz = np.zeros((1, BIAS_W), f32)
        bz[0, B_DC:B_DC + G] = dc_b
        bz[0, B_EM1:B_EM1 + M1] = np.asarray(inp["em_b1"], f32)
        bz[0, B_EM2:B_EM2 + M2] = np.asarray(inp["em_b2"], f32)
        bz[0, B_EO:B_EO + H] = np.asarray(inp["eo_b"], f32)
        bz[0, B_BHN:B_BHN + H] = np.asarray(inp[pre + "_bhh"], f32)[2 * H:]
        bz[0, B_DBHN:B_DBHN + H] = d_bhh[2 * H:]
        bz[0, B_DM1:B_DM1 + M1] = np.asarray(inp["dm_b1"], f32)
        bz[0, B_DM2:B_DM2 + M2] = np.asarray(inp["dm_b2"], f32)
        bz[0, B_DO:B_DO + NY] = np.asarray(inp["do_b"], f32)
        return bf(bz)

    for j in range(NCORE):
        chain, half = j // 2, j % 2
        pre, rev, seq = chains[chain]
        T = seq.shape[1]
        s = seq[128 * half:128 * (half + 1)]          # [128, T, 64]
        xin = np.zeros((66, et, BE), f32)
        xin[64, :, :] = 1.0
        if T < et:
            xin[65, T:, :] = 1.0                      # end padding: hold h
        order = np.arange(T)[::-1] if rev else np.arange(T)
        xin[0:64, :T, :] = s[:, order, :].transpose(2, 1, 0)
        m = dict(shared)
        m["xin"] = bf(xin.reshape(66, et * BE))
        m["wih_aug"] = wih_aug(pre)
        m["whh_t"] = bf(np.asarray(inp[pre + "_Whh"], f32).T)
        m["biases"] = bias_pack(pre)
        xl = np.concatenate([x[16 * j:16 * j + 16, -1, :],
                             x[128 + 16 * j:128 + 16 * j + 16, -1, :]])
        m["xlast_t"] = bf(xl.T)
        in_maps.append(m)
    return in_maps


def unshard(results, hor=60):
    out = np.zeros((B, hor, NY), np.float32)
    for j in range(NCORE):
        o = results[j]["out"].reshape(BD, hor, NY)
        out[16 * j:16 * j + 16] = o[0:16]
        out[128 + 16 * j:128 + 16 * j + 16] = o[16:32]
    return out


_NC = None


def kernel(**inputs):
    global _NC
    from concourse.bass_utils import run_bass_kernel_spmd
    if _NC is None:
        _NC = build_nc()
    in_maps = shard_inputs(inputs)
    res = run_bass_kernel_spmd(_NC, in_maps, core_ids=list(range(NCORE)))
    return unshard(res.results)


# revision 9
# speedup vs baseline: 1.3736x; 1.0266x over previous
"""Trainium2 Bass kernel for the GRU autoencoder (bf16 edition).

Distribution strategy (8 NeuronCores):
  Encode : chain-parallel x batch-parallel. Core j handles GRU chain j//2
           (xf, xb, ef, eb) on batch half j%2 (128 rows), running a uniform
           100-step loop. The 50-step x-chains run steps 0..49 then 50 exact
           identity steps (update-gate pre-activation forced to +BIG => z=1
           => h'=h) so their hidden is final at step 50 and the x-reshard
           collective overlaps encode steps 51..99.
  Reshard: two AllToAlls of 16-row slices (x-parts early, y-parts at end);
           every core assembles hidden states for its own 32-row decode
           shard at fixed SPMD-uniform indices:
           core j decodes global rows [16j:16j+16] u [128+16j:+16].
  Middle : per-shard MLP (em1/em2/eo) + decoder const precompute.
  Decode : 60 autoregressive steps per core on its 32-row shard.

All matmuls use bf16 operands (fp32 PSUM accumulation): bf16 MMs measure
~2x faster than fp32r on this part (193ns vs 397ns warm at N=512) and the
per-step xs-side MMs are emitted one step ahead so the PE has fill work
during each step's elementwise tail.

PSUM budget (8 banks): gA 2 | gB 2 | gC 2 | tr 2.
"""

import sys

sys.path.insert(0, "/opt/trn_rl_repo")

import numpy as np
import ml_dtypes

import concourse.bass as bass
import concourse.mybir as mybir
import concourse.tile as tile
from concourse import bacc
from concourse.masks import make_identity

dt = mybir.dt
AF = mybir.ActivationFunctionType
OP = mybir.AluOpType

B, TX, TY, NX, NY, H, HOR = 256, 50, 100, 64, 64, 512, 60
M1, M2 = 1024, 512
G = 3 * H
NCORE = 8
BE = 128   # encoder batch rows per core
BD = 32    # decoder batch rows per core
BIG = 30000.0
XSYNC = 52  # encode step at which the x-part reshard fires

F32, BF16 = dt.float32, dt.bfloat16
NPBF = ml_dtypes.bfloat16

# packed bias row layout (single [1, 6208] bf16 tensor)
B_DC, B_EM1, B_EM2, B_EO = 0, G, G + M1, G + M1 + M2
B_BHN, B_DBHN = 3584, 3584 + H
B_DM1, B_DM2, B_DO = 4608, 5632, 6144
BIAS_W = 6208


def build_nc(et=100, hor=60):
    nc = bacc.Bacc("TRN2", target_bir_lowering=False, debug=False,
                   num_devices=NCORE)

    # ---- DRAM parameters (identical names on every core; content differs) --
    d_xin = nc.dram_tensor("xin", [66, et * BE], BF16, kind="ExternalInput")
    d_wih = nc.dram_tensor("wih_aug", [66, G], BF16, kind="ExternalInput")
    d_whh = nc.dram_tensor("whh_t", [H, G], BF16, kind="ExternalInput")

    d_em1 = nc.dram_tensor("em_w1t", [2 * H, M1], BF16, kind="ExternalInput")
    d_em2 = nc.dram_tensor("em_w2t", [M1, M2], BF16, kind="ExternalInput")
    d_eow = nc.dram_tensor("eo_wt", [M2, H], BF16, kind="ExternalInput")
    d_dcw = nc.dram_tensor("dc_wt", [2 * H, G], BF16, kind="ExternalInput")

    d_dwy = nc.dram_tensor("dwy_t", [NY, G], BF16, kind="ExternalInput")
    d_dwhh = nc.dram_tensor("dwhh_t", [H, G], BF16, kind="ExternalInput")
    d_dm1 = nc.dram_tensor("dm_w1t", [H, M1], BF16, kind="ExternalInput")
    d_dm2 = nc.dram_tensor("dm_w2t", [M1, M2], BF16, kind="ExternalInput")
    d_dow = nc.dram_tensor("do_wt", [M2, NY], BF16, kind="ExternalInput")
    d_bias = nc.dram_tensor("biases", [1, BIAS_W], BF16, kind="ExternalInput")
    d_xlast = nc.dram_tensor("xlast_t", [NX, BD], BF16, kind="ExternalInput")

    d_out = nc.dram_tensor("out", [BD, hor * NY], F32, kind="ExternalOutput")

    cc_in_x = nc.dram_tensor("cc_in_x", [BE, H], BF16)
    cc_out_x = nc.dram_tensor("cc_out_x", [NCORE, 16, H], BF16)
    cc_in_y = nc.dram_tensor("cc_in_y", [BE, H], BF16)
    cc_out_y = nc.dram_tensor("cc_out_y", [NCORE, 16, H], BF16)

    with tile.TileContext(nc) as tc:
        with tc.tile_pool(name="wts", bufs=1) as wt, \
             tc.tile_pool(name="state", bufs=2) as st, \
             tc.tile_pool(name="xs", bufs=3) as xsp, \
             tc.tile_pool(name="tmp", bufs=2) as tp, \
             tc.tile_pool(name="tmp1", bufs=1) as tq, \
             tc.tile_pool(name="mid", bufs=1) as md, \
             tc.tile_pool(name="persist", bufs=1) as pe, \
             tc.tile_pool(name="psA", bufs=1, space="PSUM") as psA, \
             tc.tile_pool(name="psB", bufs=1, space="PSUM") as psB, \
             tc.tile_pool(name="psC", bufs=1, space="PSUM") as psC, \
             tc.tile_pool(name="psTR", bufs=2, space="PSUM") as psTR:

            # ---------- constants ----------
            idf = pe.tile([128, 128], F32, tag="idf")
            make_identity(nc, idf[:])
            idb = pe.tile([128, 128], BF16, tag="idb")
            nc.gpsimd.tensor_copy(idb[:], idf[:])
            id32f = idf[0:32, 0:32]
            id32b = idb[0:32, 0:32]
            ones_b = pe.tile([1, 128], BF16, tag="ones_b")
            nc.gpsimd.memset(ones_b[:], 1.0)
            zero_b = pe.tile([128, 512], BF16, tag="zero_b")
            nc.gpsimd.memset(zero_b[:], 0.0)
            ones_full = pe.tile([128, 512], BF16, tag="ones_full")
            nc.gpsimd.memset(ones_full[:], 1.0)

            # ---------- encoder weights (needed immediately) ----------
            wih = wt.tile([66, G], BF16, tag="wih")
            nc.sync.dma_start(wih[:], d_wih[:])
            whh = [wt.tile([128, G], BF16, tag=f"whh{c}", name=f"whh{c}")
                   for c in range(4)]
            for c in range(4):
                nc.sync.dma_start(whh[c][:], d_whh[128 * c:128 * (c + 1), :])
            bias = pe.tile([1, BIAS_W], BF16, tag="bias")
            nc.sync.dma_start(bias[:], d_bias[:])

            # ---------- encoder state ----------
            hT = pe.tile([128, H], BF16, tag="hT0")
            nc.vector.tensor_copy(hT[:], zero_b[:])
            h_bh = pe.tile([BE, H], BF16, tag="h0")
            nc.gpsimd.tensor_copy(h_bh[:], zero_b[:])

            # ---------- middle/decoder weights (DMA during encode) ----------
            em1 = [wt.tile([128, M1], BF16, tag=f"em1_{c}", name=f"em1_{c}")
                   for c in range(8)]
            for c in range(8):
                nc.sync.dma_start(em1[c][:], d_em1[128 * c:128 * (c + 1), :])
            em2 = [wt.tile([128, M2], BF16, tag=f"em2_{c}", name=f"em2_{c}")
                   for c in range(8)]
            for c in range(8):
                nc.sync.dma_start(em2[c][:], d_em2[128 * c:128 * (c + 1), :])
            eow = [wt.tile([128, H], BF16, tag=f"eow{c}", name=f"eow{c}")
                   for c in range(4)]
            for c in range(4):
                nc.sync.dma_start(eow[c][:], d_eow[128 * c:128 * (c + 1), :])
            dcw = [wt.tile([128, G], BF16, tag=f"dcw{c}", name=f"dcw{c}")
                   for c in range(8)]
            for c in range(8):
                nc.sync.dma_start(dcw[c][:], d_dcw[128 * c:128 * (c + 1), :])
            dwyc = wt.tile([96, G], BF16, tag="dwyc")
            nc.sync.dma_start(dwyc[0:NY, :], d_dwy[:])
            dwhh = [wt.tile([128, G], BF16, tag=f"dwhh{c}", name=f"dwhh{c}")
                    for c in range(4)]
            for c in range(4):
                nc.sync.dma_start(dwhh[c][:], d_dwhh[128 * c:128 * (c + 1), :])
            dm1 = [wt.tile([128, M1], BF16, tag=f"dm1_{c}", name=f"dm1_{c}")
                   for c in range(4)]
            for c in range(4):
                nc.sync.dma_start(dm1[c][:], d_dm1[128 * c:128 * (c + 1), :])
            dm2 = [wt.tile([128, M2], BF16, tag=f"dm2_{c}", name=f"dm2_{c}")
                   for c in range(8)]
            for c in range(8):
                nc.sync.dma_start(dm2[c][:], d_dm2[128 * c:128 * (c + 1), :])
            dow = [wt.tile([128, NY], BF16, tag=f"dow_{c}", name=f"dow_{c}")
                   for c in range(4)]
            for c in range(4):
                nc.sync.dma_start(dow[c][:], d_dow[128 * c:128 * (c + 1), :])
            xlast = wt.tile([NX, BD], BF16, tag="xlast")
            nc.sync.dma_start(xlast[:], d_xlast[:])
            # identity block staged at partitions 64:96 (for ypT rows)
            id_hi = pe.tile([96, 32], BF16, tag="id_hi")
            nc.sync.dma_start(id_hi[64:96, :], idb[0:32, 0:32])

            # ---------- encode loop ----------
            # Rolling PSUM groups: the xs-side (input-gate) MMs for step t+1
            # are emitted during step t so they fill the PE while the
            # elementwise tail runs.  gA1=r, gA2=z, gB=h-side n, gC=x-side n.
            def open_groups(t):
                xs = xsp.tile([66, 128], BF16, tag="xs")
                nc.sync.dma_start(xs[:], d_xin[:, t * BE:(t + 1) * BE])
                g1a = psA.tile([BE, 512], F32, tag="gA1")
                g1b = psA.tile([BE, 512], F32, tag="gA2")
                g3 = psC.tile([BE, 512], F32, tag="gC")
                nc.tensor.matmul(g1a[:], xs[:], wih[:, 0:512],
                                 start=True, stop=False)
                nc.tensor.matmul(g1b[:], xs[:], wih[:, 512:1024],
                                 start=True, stop=False)
                nc.tensor.matmul(g3[:], xs[:], wih[:, 1024:1536],
                                 start=True, stop=True)
                return g1a, g1b, g3

            groups = open_groups(0)
            for t in range(et):
                g1a, g1b, g3 = groups
                g2 = psB.tile([BE, 512], F32, tag="gB")
                # x-side n-gate was computed last step: stage it to SBUF now
                # (off the critical chain, DVE is idle here)
                g3b = tp.tile([BE, 512], BF16, tag="g3b")
                nc.vector.tensor_copy(g3b[:], g3[:])
                # close the accumulation groups with the h-recurrent MMs
                nc.tensor.matmul(g2[:], ones_b[0:1, 0:BE],
                                 bias[0:1, B_BHN:B_BHN + 512],
                                 start=True, stop=False)
                for c in range(4):
                    nc.tensor.matmul(g1a[:], hT[:, 128 * c:128 * (c + 1)],
                                     whh[c][:, 0:512],
                                     start=False, stop=(c == 3))
                for c in range(4):
                    nc.tensor.matmul(g2[:], hT[:, 128 * c:128 * (c + 1)],
                                     whh[c][:, 1024:1536],
                                     start=False, stop=(c == 3))
                for c in range(4):
                    nc.tensor.matmul(g1b[:], hT[:, 128 * c:128 * (c + 1)],
                                     whh[c][:, 512:1024],
                                     start=False, stop=(c == 3))

                r_t = tp.tile([BE, 512], BF16, tag="r")
                z_t = tp.tile([BE, 512], BF16, tag="z")
                nc.scalar.activation(r_t[:], g1a[:], AF.Sigmoid)
                nc.scalar.activation(z_t[:], g1b[:], AF.Sigmoid)
                g2b = tp.tile([BE, 512], BF16, tag="g2b")
                nc.vector.tensor_copy(g2b[:], g2[:])
                rhn = tp.tile([BE, 512], BF16, tag="rhn")
                nc.vector.tensor_mul(rhn[:], r_t[:], g2b[:])
                npre = tp.tile([BE, 512], BF16, tag="npre")
                nc.vector.tensor_add(npre[:], rhn[:], g3b[:])
                n_t = tp.tile([BE, 512], BF16, tag="n")
                nc.scalar.activation(n_t[:], npre[:], AF.Tanh)
                d_t = tp.tile([BE, 512], BF16, tag="d")
                nc.vector.tensor_sub(d_t[:], h_bh[:], n_t[:])
                e_t = tp.tile([BE, 512], BF16, tag="e")
                nc.vector.tensor_mul(e_t[:], z_t[:], d_t[:])
                h_new = st.tile([BE, H], BF16, tag="h")
                nc.vector.tensor_add(h_new[:], n_t[:], e_t[:])

                # PE warmers bridge the tail stall, then next step's x-side
                for dk in range(5):
                    dum = psTR.tile([128, 512], F32, tag="dum")
                    nc.tensor.matmul(dum[:], idb[:],
                                     whh[dk % 4][:, 0:512],
                                     start=True, stop=True)
                if t + 1 < et:
                    groups = open_groups(t + 1)

                ptr = psTR.tile([128, 512], BF16, tag="tr")
                for c in range(4):
                    nc.tensor.transpose(ptr[:, 128 * c:128 * (c + 1)],
                                        h_new[:, 128 * c:128 * (c + 1)],
                                        idb[:])
                hT_new = st.tile([128, H], BF16, tag="hT")
                nc.vector.tensor_copy(hT_new[:, 0:128], ptr[:, 0:128])
                nc.vector.tensor_copy(hT_new[:, 128:512], ptr[:, 128:512])
                hT, h_bh = hT_new, h_new

                if t == XSYNC:
                    # x-chain hiddens are final after step 50: reshard them
                    # now so the collective overlaps the rest of encode.
                    hx_snap = pe.tile([BE, H], BF16, tag="hx_snap")
                    nc.vector.tensor_copy(hx_snap[:], h_bh[:])
                    nc.sync.dma_start(cc_in_x[:], hx_snap[:])
                    nc.gpsimd.collective_compute(
                        "AllToAll", OP.bypass,
                        replica_groups=[list(range(NCORE))],
                        ins=[cc_in_x[:]], outs=[cc_out_x[:]])

            # ---------- reshard: y-part AllToAll ----------
            nc.sync.dma_start(cc_in_y[:], h_bh[:])
            nc.gpsimd.collective_compute(
                "AllToAll", OP.bypass,
                replica_groups=[list(range(NCORE))],
                ins=[cc_in_y[:]], outs=[cc_out_y[:]])

            pxa = md.tile([BD, H], BF16, tag="pA")
            pxb = md.tile([BD, H], BF16, tag="pB")
            pya = md.tile([BD, H], BF16, tag="pC")
            pyb = md.tile([BD, H], BF16, tag="pD")
            nc.sync.dma_start(pxa[0:16, :], cc_out_x[0][:])
            nc.sync.dma_start(pxa[16:32, :], cc_out_x[1][:])
            nc.sync.dma_start(pxb[0:16, :], cc_out_x[2][:])
            nc.sync.dma_start(pxb[16:32, :], cc_out_x[3][:])
            nc.sync.dma_start(pya[0:16, :], cc_out_y[4][:])
            nc.sync.dma_start(pya[16:32, :], cc_out_y[5][:])
            nc.sync.dma_start(pyb[0:16, :], cc_out_y[6][:])
            nc.sync.dma_start(pyb[16:32, :], cc_out_y[7][:])
            hx = md.tile([BD, H], BF16, tag="hx")
            hy = md.tile([BD, H], BF16, tag="hy")
            nc.vector.tensor_add(hx[:], pxa[:], pxb[:])
            nc.vector.tensor_add(hy[:], pya[:], pyb[:])

            def trsp_b(src, cols, tag):
                """src [BD, cols] bf16 -> bf16 [128, (cols//128)*BD]."""
                nch = cols // 128
                p = psTR.tile([128, nch * BD], BF16, tag="tr")
                for c in range(nch):
                    nc.tensor.transpose(p[:, BD * c:BD * (c + 1)],
                                        src[:, 128 * c:128 * (c + 1)], id32b)
                o = pe.tile([128, nch * BD], BF16, tag=tag)
                nc.scalar.copy(o[:], p[:])
                return o

            hxT = trsp_b(hx, H, "hxT")
            hyT = trsp_b(hy, H, "hyT")

            m1a = psA.tile([BD, 512], F32, tag="gA1")
            m1b = psA.tile([BD, 512], F32, tag="gA2")
            for c in range(8):
                s = (hxT if c < 4 else hyT)[:, BD * (c % 4):BD * (c % 4 + 1)]
                nc.tensor.matmul(m1a[:], s, em1[c][:, 0:512],
                                 start=(c == 0), stop=False)
                nc.tensor.matmul(m1b[:], s, em1[c][:, 512:1024],
                                 start=(c == 0), stop=False)
            nc.tensor.matmul(m1a[:], ones_b[0:1, 0:BD],
                             bias[0:1, B_EM1:B_EM1 + 512],
                             start=False, stop=True)
            nc.tensor.matmul(m1b[:], ones_b[0:1, 0:BD],
                             bias[0:1, B_EM1 + 512:B_EM1 + 1024],
                             start=False, stop=True)
            hm1 = tq.tile([BD, M1], BF16, tag="hm1m")
            nc.scalar.activation(hm1[:, 0:512], m1a[:], AF.Relu)
            nc.scalar.activation(hm1[:, 512:1024], m1b[:], AF.Relu)
            hm1T = trsp_b(hm1, M1, "hm1T_m")

            m2 = psB.tile([BD, M2], F32, tag="gB")
            for c in range(8):
                nc.tensor.matmul(m2[:], hm1T[:, BD * c:BD * (c + 1)],
                                 em2[c][:], start=(c == 0), stop=False)
            nc.tensor.matmul(m2[:], ones_b[0:1, 0:BD],
                             bias[0:1, B_EM2:B_EM2 + 512],
                             start=False, stop=True)
            hm2 = tq.tile([BD, M2], BF16, tag="hm2m")
            nc.scalar.activation(hm2[:], m2[:], AF.Relu)
            hm2T = trsp_b(hm2, M2, "hm2T_m")

            zp = psC.tile([BD, H], F32, tag="gC")
            for c in range(4):
                nc.tensor.matmul(zp[:], hm2T[:, BD * c:BD * (c + 1)],
                                 eow[c][:], start=(c == 0), stop=False)
            nc.tensor.matmul(zp[:], ones_b[0:1, 0:BD],
                             bias[0:1, B_EO:B_EO + 512],
                             start=False, stop=True)
            z_sb = md.tile([BD, H], BF16, tag="z_sb")
            nc.scalar.copy(z_sb[:], zp[:])
            zT = trsp_b(z_sb, H, "zT")

            # const = cat(h_x, z) @ d_Wih[:, :2H].T + d_bih + d_bhh(r,z)
            cpa = psA.tile([96, 512], F32, tag="gA1")
            cpa = cpa[64:96, :]
            cpb = psA.tile([96, 512], F32, tag="gA2")
            cpb = cpb[64:96, :]
            cpn = psB.tile([96, 512], F32, tag="gB")
            cpn = cpn[64:96, :]
            for c in range(8):
                s = (hxT if c < 4 else zT)[:, BD * (c % 4):BD * (c % 4 + 1)]
                nc.tensor.matmul(cpa[:], s, dcw[c][:, 0:512],
                                 start=(c == 0), stop=False)
                nc.tensor.matmul(cpb[:], s, dcw[c][:, 512:1024],
                                 start=(c == 0), stop=False)
                nc.tensor.matmul(cpn[:], s, dcw[c][:, 1024:1536],
                                 start=(c == 0), stop=False)
            nc.tensor.matmul(cpa[:], ones_b[0:1, 0:BD],
                             bias[0:1, B_DC:B_DC + 512], start=False, stop=True)
            nc.tensor.matmul(cpb[:], ones_b[0:1, 0:BD],
                             bias[0:1, B_DC + 512:B_DC + 1024],
                             start=False, stop=True)
            nc.tensor.matmul(cpn[:], ones_b[0:1, 0:BD],
                             bias[0:1, B_DC + 1024:B_DC + 1536],
                             start=False, stop=True)
            nc.vector.tensor_copy(dwyc[64:96, 0:512], cpa[:])
            nc.vector.tensor_copy(dwyc[64:96, 512:1024], cpb[:])
            nc.vector.tensor_copy(dwyc[64:96, 1024:1536], cpn[:])

            # decoder init
            hdT = st.tile([128, 4 * BD], BF16, tag="hdT")
            nc.vector.tensor_copy(hdT[:], zero_b[:, 0:4 * BD])
            hd = st.tile([BD, H], BF16, tag="hd")
            nc.gpsimd.tensor_copy(hd[:], zero_b[0:BD, :])
            ypT0 = pe.tile([96, BD], BF16, tag="ypT0")
            ypT1 = pe.tile([96, BD], BF16, tag="ypT1")
            ypTs = [ypT0, ypT1]
            nc.vector.tensor_copy(ypT0[0:NX, :], xlast[:])
            nc.vector.tensor_copy(ypT0[64:96, :], id_hi[64:96, :])
            nc.vector.tensor_copy(ypT1[64:96, :], id_hi[64:96, :])
            ypT = ypT0

            # ---------- decode loop ----------
            for t in range(hor):
                g1a = psA.tile([BD, 512], F32, tag="gA1")
                g1b = psA.tile([BD, 512], F32, tag="gA2")
                g2 = psB.tile([BD, 512], F32, tag="gB")
                g3 = psC.tile([BD, 512], F32, tag="gC")
                # h-side first: depends only on hdT (ready since last GRU
                # phase), so these stream during the previous step's MLP.
                # The yp/const matmuls close each group once ypT lands.
                nc.tensor.matmul(g2[:], ones_b[0:1, 0:BD],
                                 bias[0:1, B_DBHN:B_DBHN + 512],
                                 start=True, stop=False)
                for c in range(4):
                    nc.tensor.matmul(g2[:], hdT[:, BD * c:BD * (c + 1)],
                                     dwhh[c][:, 1024:1536],
                                     start=False, stop=(c == 3))
                for c in range(4):
                    nc.tensor.matmul(g1a[:], hdT[:, BD * c:BD * (c + 1)],
                                     dwhh[c][:, 0:512],
                                     start=(c == 0), stop=False)
                for c in range(4):
                    nc.tensor.matmul(g1b[:], hdT[:, BD * c:BD * (c + 1)],
                                     dwhh[c][:, 512:1024],
                                     start=(c == 0), stop=False)
                g2b = tp.tile([BD, 512], BF16, tag="g2b")
                nc.vector.tensor_copy(g2b[:], g2[:])
                nc.tensor.matmul(g1a[:], ypT[:], dwyc[:, 0:512],
                                 start=False, stop=True)
                nc.tensor.matmul(g3[:], ypT[:], dwyc[:, 1024:1536],
                                 start=True, stop=True)
                nc.tensor.matmul(g1b[:], ypT[:], dwyc[:, 512:1024],
                                 start=False, stop=True)
                g3b = tp.tile([BD, 512], BF16, tag="g3b")
                nc.vector.tensor_copy(g3b[:], g3[:])
                for dk in range(6):
                    dum = psTR.tile([128, 512], F32, tag="dum")
                    nc.tensor.matmul(dum[:], idb[:],
                                     dwhh[dk % 4][:, 0:512],
                                     start=True, stop=True)

                r_t = tp.tile([BD, 512], BF16, tag="r")
                z_t = tp.tile([BD, 512], BF16, tag="z")
                nc.scalar.activation(r_t[:], g1a[:], AF.Sigmoid)
                nc.scalar.activation(z_t[:], g1b[:], AF.Sigmoid)
                rhn = tp.tile([BD, 512], BF16, tag="rhn")
                nc.vector.tensor_mul(rhn[:], r_t[:], g2b[:])
                npre = tp.tile([BD, 512], BF16, tag="npre")
                nc.vector.tensor_add(npre[:], rhn[:], g3b[:])
                n_t = tp.tile([BD, 512], BF16, tag="n")
                nc.scalar.activation(n_t[:], npre[:], AF.Tanh)
                d_t = tp.tile([BD, 512], BF16, tag="d")
                nc.vector.tensor_sub(d_t[:], hd[:], n_t[:])
                e_t = tp.tile([BD, 512], BF16, tag="e")
                nc.vector.tensor_mul(e_t[:], z_t[:], d_t[:])
                hd_new = st.tile([BD, H], BF16, tag="hd")
                nc.vector.tensor_add(hd_new[:], n_t[:], e_t[:])

                ptr = psTR.tile([128, 4 * BD], BF16, tag="tr")
                for c in range(4):
                    nc.tensor.transpose(ptr[:, BD * c:BD * (c + 1)],
                                        hd_new[:, 128 * c:128 * (c + 1)],
                                        id32b)
                hdT_new = st.tile([128, 4 * BD], BF16, tag="hdT")
                nc.vector.tensor_copy(hdT_new[:, 0:BD], ptr[:, 0:BD])
                nc.vector.tensor_copy(hdT_new[:, BD:4 * BD], ptr[:, BD:4 * BD])
                hdT, hd = hdT_new, hd_new

                m1a = psA.tile([BD, 512], F32, tag="gA1")
                m1b = psA.tile([BD, 512], F32, tag="gA2")
                for c in range(4):
                    nc.tensor.matmul(m1a[:], hdT[:, BD * c:BD * (c + 1)],
                                     dm1[c][:, 0:512],
                                     start=(c == 0), stop=False)
                nc.tensor.matmul(m1a[:], ones_b[0:1, 0:BD],
                                 bias[0:1, B_DM1:B_DM1 + 512],
                                 start=False, stop=True)
                hm1 = tq.tile([BD, M1], BF16, tag="hm1")
                nc.scalar.activation(hm1[:, 0:512], m1a[:], AF.Relu)
                for c in range(4):
                    nc.tensor.matmul(m1b[:],
                                     hdT[:, BD * c:BD * (c + 1)],
                                     dm1[c][:, 512:1024],
                                     start=(c == 0), stop=False)
                nc.tensor.matmul(m1b[:], ones_b[0:1, 0:BD],
                                 bias[0:1, B_DM1 + 512:B_DM1 + 1024],
                                 start=False, stop=True)
                nc.scalar.activation(hm1[:, 512:1024], m1b[:], AF.Relu)
                hm1Ta = tq.tile([128, 4 * BD], BF16, tag="hm1Ta")
                hm1Tb = tq.tile([128, 4 * BD], BF16, tag="hm1Tb")
                p1 = psTR.tile([128, 4 * BD], BF16, tag="tr")
                for c in range(4):
                    nc.tensor.transpose(p1[:, BD * c:BD * (c + 1)],
                                        hm1[:, 128 * c:128 * (c + 1)], id32b)
                nc.vector.tensor_copy(hm1Ta[:], p1[:])
                p1b = psTR.tile([128, 4 * BD], BF16, tag="tr")
                for c in range(4):
                    nc.tensor.transpose(p1b[:, BD * c:BD * (c + 1)],
                                        hm1[:, 512 + 128 * c:640 + 128 * c],
                                        id32b)
                nc.vector.tensor_copy(hm1Tb[:], p1b[:])

                m2 = psB.tile([BD, M2], F32, tag="gB")
                for c in range(8):
                    s = (hm1Ta if c < 4 else hm1Tb)[:, BD * (c % 4):
                                                    BD * (c % 4 + 1)]
                    nc.tensor.matmul(m2[:], s, dm2[c][:],
                                     start=(c == 0), stop=False)
                nc.tensor.matmul(m2[:], ones_b[0:1, 0:BD],
                                 bias[0:1, B_DM2:B_DM2 + 512],
                                 start=False, stop=True)
                hm2 = tq.tile([BD, M2], BF16, tag="hm2")
                nc.scalar.activation(hm2[:], m2[:], AF.Relu)
                p2 = psTR.tile([128, 4 * BD], BF16, tag="tr")
                for c in range(4):
                    nc.tensor.transpose(p2[:, BD * c:BD * (c + 1)],
                                        hm2[:, 128 * c:128 * (c + 1)], id32b)
                hm2T = tq.tile([128, 4 * BD], BF16, tag="hm2T")
                nc.vector.tensor_copy(hm2T[:], p2[:])

                yp_ps = psC.tile([BD, NY], F32, tag="gC")
                for c in range(4):
                    nc.tensor.matmul(yp_ps[:], hm2T[:, BD * c:BD * (c + 1)],
                                     dow[c][:], start=(c == 0), stop=False)
                nc.tensor.matmul(yp_ps[:], ones_b[0:1, 0:BD],
                                 bias[0:1, B_DO:B_DO + NY],
                                 start=False, stop=True)
                y_sb = tp.tile([BD, NY], F32, tag="y_sb")
                nc.vector.tensor_copy(y_sb[:], yp_ps[:])
                nc.sync.dma_start(d_out[:, NY * t:NY * (t + 1)], y_sb[:])
                if t + 1 < hor:
                    yT_ps = psC.tile([NY, BD], F32, tag="gC")
                    for c in range(4):
                        nc.tensor.matmul(yT_ps[:], dow[c][:],
                                         hm2T[:, BD * c:BD * (c + 1)],
                                         start=(c == 0), stop=False)
                    nc.tensor.matmul(yT_ps[:], bias[0:1, B_DO:B_DO + NY],
                                     ones_b[0:1, 0:BD],
                                     start=False, stop=True)
                    ypT_new = ypTs[(t + 1) % 2]
                    nc.vector.tensor_copy(ypT_new[0:NX, :], yT_ps[:])
                    ypT = ypT_new

    nc.compile()
    return nc


# ---------------------------------------------------------------------------
# Host-side sharding
# ---------------------------------------------------------------------------

def shard_inputs(inp, et=100, hor=60):
    f32 = np.float32
    x, y = np.asarray(inp["x"], f32), np.asarray(inp["y"], f32)
    chains = [("xf", False, x), ("xb", True, x),
              ("ef", False, y), ("eb", True, y)]
    in_maps = []
    shared = {}

    def bf(a):
        return np.ascontiguousarray(np.asarray(a, f32)).astype(NPBF)

    def wih_aug(pre):
        wih = np.asarray(inp[pre + "_Wih"], f32)
        bih = np.asarray(inp[pre + "_bih"], f32)
        bhh = np.asarray(inp[pre + "_bhh"], f32)
        aug = np.zeros((66, G), f32)
        aug[0:64, :] = wih.T
        b = bih.copy()
        b[0:2 * H] += bhh[0:2 * H]
        aug[64, :] = b
        aug[65, H:2 * H] = BIG
        return bf(aug)

    d_Wih = np.asarray(inp["d_Wih"], f32)
    d_bih = np.asarray(inp["d_bih"], f32)
    d_bhh = np.asarray(inp["d_bhh"], f32)
    dc_b = d_bih.copy()
    dc_b[0:2 * H] += d_bhh[0:2 * H]

    shared["em_w1t"] = bf(np.asarray(inp["em_W1"], f32).T)
    shared["em_w2t"] = bf(np.asarray(inp["em_W2"], f32).T)
    shared["eo_wt"] = bf(np.asarray(inp["eo_W"], f32).T)
    shared["dc_wt"] = bf(d_Wih[:, 0:2 * H].T)
    shared["dwy_t"] = bf(d_Wih[:, 2 * H:].T)
    shared["dwhh_t"] = bf(np.asarray(inp["d_Whh"], f32).T)
    shared["dm_w1t"] = bf(np.asarray(inp["dm_W1"], f32).T)
    shared["dm_w2t"] = bf(np.asarray(inp["dm_W2"], f32).T)
    shared["do_wt"] = bf(np.asarray(inp["do_W"], f32).T)

    def bias_pack(pre):
        bz = np.zeros((1, BIAS_W), f32)
        bz[0, B_DC:B_DC + G] = dc_b
        bz[0, B_EM1:B_EM1 + M1] = np.asarray(inp["em_b1"], f32)
        bz[0, B_EM2:B_EM2 + M2] = np.asarray(inp["em_b2"], f32)
        bz[0, B_EO:B_EO + H] = np.asarray(inp["eo_b"], f32)
        bz[0, B_BHN:B_BHN + H] = np.asarray(inp[pre + "_bhh"], f32)[2 * H:]
        bz[0, B_DBHN:B_DBHN + H] = d_bhh[2 * H:]
        bz[0, B_DM1:B_DM1 + M1] = np.asarray(inp["dm_b1"], f32)
        bz[0, B_DM2:B_DM2 + M2] = np.asarray(inp["dm_b2"], f32)
        bz[0, B_DO:B_DO + NY] = np.asarray(inp["do_b"], f32)
        return bf(bz)

    for j in range(NCORE):
        chain, half = j // 2, j % 2
        pre, rev, seq = chains[chain]
        T = seq.shape[1]
        s = seq[128 * half:128 * (half + 1)]          # [128, T, 64]
        xin = np.zeros((66, et, BE), f32)
        xin[64, :, :] = 1.0
        if T < et:
            xin[65, T:, :] = 1.0                      # end padding: hold h
        order = np.arange(T)[::-1] if rev else np.arange(T)
        xin[0:64, :T, :] = s[:, order, :].transpose(2, 1, 0)
        m = dict(shared)
        m["xin"] = bf(xin.reshape(66, et * BE))
        m["wih_aug"] = wih_aug(pre)
        m["whh_t"] = bf(np.asarray(inp[pre + "_Whh"], f32).T)
        m["biases"] = bias_pack(pre)
        xl = np.concatenate([x[16 * j:16 * j + 16, -1, :],
                             x[128 + 16 * j:128 + 16 * j + 16, -1, :]])
        m["xlast_t"] = bf(xl.T)
        in_maps.append(m)
    return in_maps


def unshard(results, hor=60):
    out = np.zeros((B, hor, NY), np.float32)
    for j in range(NCORE):
        o = results[j]["out"].reshape(BD, hor, NY)
        out[16 * j:16 * j + 16] = o[0:16]
        out[128 + 16 * j:128 + 16 * j + 16] = o[16:32]
    return out


_NC = None


def kernel(**inputs):
    global _NC
    from concourse.bass_utils import run_bass_kernel_spmd
    if _NC is None:
        _NC = build_nc()
    in_maps = shard_inputs(inputs)
    res = run_bass_kernel_spmd(_NC, in_maps, core_ids=list(range(NCORE)))
    return unshard(res.results)
